# revision 1
# baseline (speedup 1.0000x reference)
"""BlurDegradation kernel for 8x TRN2 NeuronCores.

Math: t[b] successive 11x11 depthwise *circular* convolutions compose into a
single circular convolution whose spectrum is the product of the per-step
spectra. The host composes the (tiny) 20 step-kernels into 21 cumulative
spectra with numpy FFTs and selects per-sample spectrum FK[t[b]]; each device
then computes, per image,  out = Re( F* . (FK o (F x F)) . F* ) / N^2  as four
chained dense matmul stages on the PE array (plus DVE/ACT/GpSimd pointwise
work). Contractions always run over the partition dim, so each stage
implicitly transposes and no PE/DMA transposes are needed.

Optimizations vs the naive 4-stage dense chain:
 - Hermitian half-spectrum: y-frequencies k=0..257 only (258 = even, required
   by fp32r); fold weights (1,2,..,2,1,0) and 1/N^2 live in FK (host-side).
 - Stage 2 is matrix-stationary (resident DFT-matrix weights prefetch cleanly)
   with the k-half axis as the cheap *moving* dim (N=258).
 - Stage 3 is data-stationary with *weight-paired* emission: each fresh
   Z-slice weight load is reused by two consecutive matmuls (the reload is
   skipped), halving the fresh-weight-load penalty.
 - Stage 4 is matrix-stationary with natural [y,x] output; the k-half
   contraction uses three full 128-row tiles (0:128, 128:256, 130:258) with
   the double-counted k=130..255 range pre-halved in FK on the host.
 - H / n1 PSUM tiles are staged to SBUF by the Scalar engine (fast PSUM-bank
   release); the pointwise complex multiply is split across Vector + GpSimd.

Sharding: pure data parallel, 8 samples per core, no cross-core comms.
"""

import numpy as np

N = 512
P = 128
T_STEPS = 20
KS = 11
KP = 258            # padded half-spectrum k-dim (even for fp32r)
NCORES = 8
BATCH = 64
CHANNELS = 3
SPC = BATCH // NCORES  # samples per core
IMGS = SPC * CHANNELS  # images per core

USE_F32R = True

_PROGRAM = None
TRACE = False
LAST_EXEC_NS = None
LAST_TRACE = None


def _build_program():
    import concourse.mybir as mybir
    import concourse.tile as tile
    from concourse import bacc

    f32 = mybir.dt.float32
    f32r = mybir.dt.float32r
    mmdt = f32r if USE_F32R else f32

    nc = bacc.Bacc(
        "TRN2", target_bir_lowering=False, debug=False, num_devices=NCORES
    )
    x_d = nc.dram_tensor("x", [IMGS, N, N], mmdt, kind="ExternalInput").ap()
    fkr_d = nc.dram_tensor("fkr", [SPC, N, KP], f32, kind="ExternalInput").ap()
    fki_d = nc.dram_tensor("fki", [SPC, N, KP], f32, kind="ExternalInput").ap()
    mat_names = ["cmat", "smat", "snmat", "nscmat", "cmsmat"]
    mat_d = {
        nm: nc.dram_tensor(nm, [N, N], mmdt, kind="ExternalInput").ap()
        for nm in mat_names
    }
    out_d = nc.dram_tensor("out", [IMGS, N, N], f32, kind="ExternalOutput").ap()

    with tile.TileContext(nc) as tc:
        with (
            tc.tile_pool(name="mats", bufs=1) as mats,
            tc.tile_pool(name="xsp", bufs=2) as xsp,
            tc.tile_pool(name="outp", bufs=2) as outp,
            tc.tile_pool(name="fkp", bufs=2) as fkp,
            tc.tile_pool(name="apool", bufs=2) as apool,
            tc.tile_pool(name="hpool", bufs=2) as hpool,
            tc.tile_pool(name="zpool", bufs=2) as zpool,
            tc.tile_pool(name="vpool", bufs=2) as vpool,
            tc.tile_pool(name="pw", bufs=3) as pw,
            tc.tile_pool(name="psum", bufs=8, space="PSUM") as psum,
        ):
            # resident DFT matrices, [p, tile, n] layout
            M = {}
            for nm in mat_names:
                mt = mats.tile([P, 4, N], mmdt, name=nm + "_s")
                nc.sync.dma_start(mt[:], mat_d[nm].rearrange("(i p) n -> p i n", p=P))
                M[nm] = mt
            Cs, Ss, Sns = M["cmat"], M["smat"], M["snmat"]
            nSCs, CmSs = M["nscmat"], M["cmsmat"]
            # C/-S rows 130..257, partition-aligned (stage-4 k tail tile)
            Ck2s = mats.tile([P, N], mmdt, name="ck2_s")
            Snk2s = mats.tile([P, N], mmdt, name="snk2_s")
            nc.sync.dma_start(Ck2s[:], mat_d["cmat"][130:258, :])
            nc.sync.dma_start(Snk2s[:], mat_d["snmat"][130:258, :])

            def emit_st4(Vr, Vi, img):
                # ---- Stage 4 (matrix-stationary, natural orientation):
                # out[y,x] = sum_k C[k,y] Vr[k,x] + (-S)[k,y] Vi[k,x]
                # k tiles: 0:128, 128:256, 130:258 (FK pre-halved on the
                # double-counted 130..255 range)
                outs = outp.tile([P, 4, N], f32, tag="outs")
                for ym in range(4):
                    ysl = slice(ym * P, (ym + 1) * P)
                    po = psum.tile([P, N], f32, tag="ps", name="po")
                    nc.tensor.matmul(
                        po[:], Cs[:, 0, ysl], Vr[:, 0, :],
                        start=True, stop=False,
                    )
                    nc.tensor.matmul(
                        po[:], Cs[:, 1, ysl], Vr[:, 1, :],
                        start=False, stop=False,
                    )
                    nc.tensor.matmul(
                        po[:], Ck2s[:, ysl], Vr[:, 2, :],
                        start=False, stop=False,
                    )
                    nc.tensor.matmul(
                        po[:], Sns[:, 0, ysl], Vi[:, 0, :],
                        start=False, stop=False,
                    )
                    nc.tensor.matmul(
                        po[:], Sns[:, 1, ysl], Vi[:, 1, :],
                        start=False, stop=False,
                    )
                    nc.tensor.matmul(
                        po[:], Snk2s[:, ysl], Vi[:, 2, :],
                        start=False, stop=True,
                    )
                    nc.any.tensor_copy(out=outs[:, ym, :], in_=po[:])
                nc.sync.dma_start(
                    out_d[img].rearrange("(i p) n -> p i n", p=P), outs[:]
                )

            pending = None  # (Vr, Vi, img) of the previous image

            for s in range(SPC):
                # per-sample spectrum, transposed [l, k] layout, k cols 0..257
                fktr = fkp.tile([P, 4, KP], f32, tag="fktr")
                fkti = fkp.tile([P, 4, KP], f32, tag="fkti")
                nc.sync.dma_start(
                    fktr[:], fkr_d[s].rearrange("(i p) n -> p i n", p=P)
                )
                nc.sync.dma_start(
                    fkti[:], fki_d[s].rearrange("(i p) n -> p i n", p=P)
                )

                for ch in range(CHANNELS):
                    img = s * CHANNELS + ch
                    xs = xsp.tile([P, 4, N], mmdt, tag="xs")
                    nc.sync.dma_start(
                        xs[:], x_d[img].rearrange("(i p) n -> p i n", p=P)
                    )

                    # ---- Stage 1 (data-stationary, weight-paired):
                    # A_r = x^T C[:, :258] ; A_i = x^T (-S)[:, :258]
                    Ar = apool.tile([P, 4, KP], mmdt, tag="Ar")
                    Ai = apool.tile([P, 4, KP], mmdt, tag="Ai")
                    Apb = apool.tile([P, 4, KP], mmdt, tag="Apb")
                    for m in range(4):
                        msl = slice(m * P, (m + 1) * P)
                        pa = psum.tile([P, N], f32, tag="ps", name="pa")[:, :KP]
                        pb = psum.tile([P, N], f32, tag="ps", name="pb")[:, :KP]
                        for kk in range(4):
                            nc.tensor.matmul(
                                pa[:], xs[:, kk, msl], Cs[:, kk, 0:KP],
                                start=(kk == 0), stop=(kk == 3),
                            )
                            nc.tensor.matmul(
                                pb[:], xs[:, kk, msl], Sns[:, kk, 0:KP],
                                start=(kk == 0), stop=(kk == 3),
                            )
                        nc.scalar.copy(out=Ar[:, m, :], in_=pa[:])
                        nc.scalar.copy(out=Ai[:, m, :], in_=pb[:])
                        nc.gpsimd.tensor_tensor(
                            Apb[:, m, :], Ar[:, m, :], Ai[:, m, :],
                            mybir.AluOpType.add,
                        )

                    # ---- Stage 2 (matrix-stationary, Gauss):
                    # m1 = C.(Ar+Ai) ; m2 = (-S-C).Ar ; m3 = (C-S).Ai
                    # Htr = m1 - m3 ; Hti = m1 + m2 ; pointwise per l-tile
                    Ztr = zpool.tile([P, 4, KP], mmdt, tag="Ztr")
                    Zti = zpool.tile([P, 4, KP], mmdt, tag="Zti")
                    for lm in range(4):
                        lsl = slice(lm * P, (lm + 1) * P)
                        m1 = psum.tile([P, N], f32, tag="ps", name="m1")[:, :KP]
                        m2 = psum.tile([P, N], f32, tag="ps", name="m2")[:, :KP]
                        m3 = psum.tile([P, N], f32, tag="ps", name="m3")[:, :KP]
                        for kk in range(4):
                            nc.tensor.matmul(
                                m1[:], Cs[:, kk, lsl], Apb[:, kk, :],
                                start=(kk == 0), stop=(kk == 3),
                            )
                        for kk in range(4):
                            nc.tensor.matmul(
                                m2[:], nSCs[:, kk, lsl], Ar[:, kk, :],
                                start=(kk == 0), stop=(kk == 3),
                            )
                        for kk in range(4):
                            nc.tensor.matmul(
                                m3[:], CmSs[:, kk, lsl], Ai[:, kk, :],
                                start=(kk == 0), stop=(kk == 3),
                            )
                        # Htr = m1 - m3 ; Hti = m1 + m2
                        m1s = pw.tile([P, KP], f32, tag="m1s")
                        hrs = pw.tile([P, KP], f32, tag="hrs")
                        his = pw.tile([P, KP], f32, tag="his")
                        nc.scalar.copy(out=m1s[:], in_=m1[:])
                        nc.vector.tensor_sub(out=hrs[:], in0=m1s[:], in1=m3[:])
                        nc.vector.tensor_add(out=his[:], in0=m1s[:], in1=m2[:])
                        # pointwise: Ztr = hr o fr - hi o fi
                        #            Zti = hr o fi + hi o fr
                        fr = fktr[:, lm, :]
                        fi = fkti[:, lm, :]
                        tt = pw.tile([P, KP], f32, tag="tt")
                        tu = pw.tile([P, KP], f32, tag="tu")
                        ztr = Ztr[:, lm, :]
                        zti = Zti[:, lm, :]
                        nc.vector.tensor_mul(out=ztr, in0=hrs[:], in1=fr)
                        nc.gpsimd.tensor_tensor(
                            tt[:], his[:], fi, mybir.AluOpType.mult
                        )
                        nc.vector.tensor_sub(out=ztr, in0=ztr, in1=tt[:])
                        nc.gpsimd.tensor_tensor(
                            tu[:], hrs[:], fi, mybir.AluOpType.mult
                        )
                        nc.vector.tensor_mul(out=zti, in0=his[:], in1=fr)
                        nc.vector.tensor_add(out=zti, in0=zti, in1=tu[:])

                    # fill the pointwise-latency bubble with the previous
                    # image's stage 4 (independent PE work)
                    if pending is not None:
                        emit_st4(*pending)
                        pending = None

                    # ---- Stage 3 (data-stationary, weight-paired direct):
                    # V_r = Ztr^T C + Zti^T (-S) ; V_i = Ztr^T S + Zti^T C
                    # k M-tiles: 0:128, 128:256, 130:258
                    Vr = vpool.tile([P, 3, N], mmdt, tag="Vr")
                    Vi = vpool.tile([P, 3, N], mmdt, tag="Vi")
                    for km in range(3):
                        koff = (0, 128, 130)[km]
                        ksl = slice(koff, koff + P)
                        nvr = psum.tile([P, N], f32, tag="ps", name="nvr")
                        nvi = psum.tile([P, N], f32, tag="ps", name="nvi")
                        for lt in range(4):
                            nc.tensor.matmul(
                                nvr[:], Ztr[:, lt, ksl], Cs[:, lt, :],
                                start=(lt == 0), stop=False,
                            )
                            nc.tensor.matmul(
                                nvi[:], Ztr[:, lt, ksl], Ss[:, lt, :],
                                start=(lt == 0), stop=False,
                            )
                        for lt in range(4):
                            nc.tensor.matmul(
                                nvr[:], Zti[:, lt, ksl], Sns[:, lt, :],
                                start=False, stop=(lt == 3),
                            )
                            nc.tensor.matmul(
                                nvi[:], Zti[:, lt, ksl], Cs[:, lt, :],
                                start=False, stop=(lt == 3),
                            )
                        nc.any.tensor_copy(out=Vr[:, km, :], in_=nvr[:])
                        nc.any.tensor_copy(out=Vi[:, km, :], in_=nvi[:])

                    pending = (Vr, Vi, img)


            if pending is not None:
                emit_st4(*pending)

    nc.compile()
    return nc


def _host_spectra(kernels):
    """Compose step kernels into 21 cumulative half-spectra, transposed to
    [l, k] layout with Hermitian weights, 1/N^2, and the stage-4
    double-count halving folded in. Returns (FKtr, FKti) f32 [21, 512, KP]."""
    kernels = np.asarray(kernels, dtype=np.float64)
    h = np.zeros((T_STEPS, N, N), np.float64)
    idx = (KS // 2 - np.arange(KS)) % N
    h[:, idx[:, None], idx[None, :]] = kernels
    s_step = np.fft.fft2(h)
    cum = np.ones((T_STEPS + 1, N, N), np.complex128)
    for i in range(1, T_STEPS + 1):
        cum[i] = cum[i - 1] * s_step[i - 1]
    w = np.zeros(KP)
    w[: N // 2 + 1] = 2.0
    w[0] = w[N // 2] = 1.0
    fkt = (cum[:, :KP, :] * w[None, :, None] / float(N * N)).transpose(0, 2, 1)
    half = np.ones(KP)
    half[130:256] = 0.5  # k rows 130..255 appear in both stage-4 k-tiles
    fkt = fkt * half[None, None, :]
    return (
        np.ascontiguousarray(fkt.real.astype(np.float32)),
        np.ascontiguousarray(fkt.imag.astype(np.float32)),
    )


def _dft_mats():
    j = np.arange(N)
    ang = 2.0 * np.pi * (np.outer(j, j) % N) / N
    cm = np.cos(ang).astype(np.float32)
    sm = np.sin(ang).astype(np.float32)
    return {
        "cmat": cm,
        "smat": sm,
        "snmat": np.ascontiguousarray(-sm),
        "nscmat": np.ascontiguousarray(-sm - cm),
        "cmsmat": np.ascontiguousarray(cm - sm),
    }


def kernel(x0, t, kernels):
    global _PROGRAM, LAST_EXEC_NS, LAST_TRACE
    from concourse import bass_utils

    x0 = np.ascontiguousarray(np.asarray(x0), dtype=np.float32)
    tt = np.asarray(t).astype(np.int64)
    fktr_all, fkti_all = _host_spectra(kernels)
    mats = _dft_mats()

    if _PROGRAM is None:
        _PROGRAM = _build_program()
    nc = _PROGRAM

    in_maps = []
    for c in range(NCORES):
        sl = slice(c * SPC, (c + 1) * SPC)
        ts = tt[sl]
        im = {
            "x": np.ascontiguousarray(x0[sl].reshape(IMGS, N, N)),
            "fkr": np.ascontiguousarray(fktr_all[ts]),
            "fki": np.ascontiguousarray(fkti_all[ts]),
        }
        im.update(mats)
        in_maps.append(im)

    res = bass_utils.run_bass_kernel_spmd(
        nc, in_maps, core_ids=list(range(NCORES)), trace=TRACE
    )
    LAST_EXEC_NS = res.exec_time_ns
    if res.instructions_and_trace is not None:
        LAST_TRACE = res.instructions_and_trace[1]
    out = np.empty((BATCH, CHANNELS, N, N), np.float32)
    for c in range(NCORES):
        out[c * SPC : (c + 1) * SPC] = res.results[c]["out"].reshape(
            SPC, CHANNELS, N, N
        )
    return out



# revision 2
# speedup vs baseline: 3.7874x; 3.7874x over previous
"""BlurDegradation kernel for 8x TRN2 NeuronCores.

Math: t[b] successive 11x11 depthwise *circular* convolutions compose into a
single circular convolution with kernel k_t = h_1 (*) h_2 (*) ... (*) h_t
(circular 2D convolution of the per-step impulse responses). The host
composes k_t exactly with FFTs.

Fast path (separable): when every composed k_t is numerically rank-1
(k_t = outer(a_t, b_t) -- always true for the constant-Gaussian blur routine,
whose steps are separable), the 2D blur factors into a column circular conv
by a_t followed by a row circular conv by b_t. Each is one dense circulant
matmul, so an image costs just two 512x512x512 matmul chains on the PE array
(268M MACs vs 941M for the spectral pipeline). Both stages are
data-stationary so outputs chain orientation [h,w] -> [w,v] -> [v,x] with no
transposes:
  stage 1: T1[w,v] = sum_h X[h,w] * Gc[h,v],  Gc[h,v] = a[(v-h) mod N]
  stage 2: Z [v,x] = sum_w T1[w,v] * Gr[w,x], Gr[w,x] = b[(x-w) mod N]
Matmuls run in bf16 (1 cycle/row + fast weight load) with fp32 PSUM
accumulation; stage-2 of image i is emitted after stage-1 of image i+1 so the
PSUM->SBUF cast copies never stall the PE.

Fallback path (general kernels): the original spectral pipeline -- per-sample
cumulative half-spectra multiply between dense DFT matmuls.

Sharding: pure data parallel, 8 samples per core, no cross-core comms.
"""

import numpy as np

N = 512
P = 128
T_STEPS = 20
KS = 11
KP = 258            # padded half-spectrum k-dim (even for fp32r)
NCORES = 8
BATCH = 64
CHANNELS = 3
SPC = BATCH // NCORES  # samples per core
IMGS = SPC * CHANNELS  # images per core

USE_F32R = True

_PROGRAMS = {}
TRACE = False
LAST_EXEC_NS = None
LAST_TRACE = None


# --------------------------------------------------------------------------
# Separable fast path
# --------------------------------------------------------------------------

def _build_program_sep():
    import concourse.mybir as mybir
    import concourse.tile as tile
    from concourse import bacc

    f32 = mybir.dt.float32
    bf16 = mybir.dt.bfloat16

    nc = bacc.Bacc(
        "TRN2", target_bir_lowering=False, debug=False, num_devices=NCORES
    )
    x_d = nc.dram_tensor("x", [IMGS, N, N], bf16, kind="ExternalInput").ap()
    gc_d = nc.dram_tensor("gc", [SPC, N, N], bf16, kind="ExternalInput").ap()
    gr_d = nc.dram_tensor("gr", [SPC, N, N], bf16, kind="ExternalInput").ap()
    out_d = nc.dram_tensor("out", [IMGS, N, N], f32, kind="ExternalOutput").ap()

    with tile.TileContext(nc) as tc:
        with (
            tc.tile_pool(name="gp", bufs=2) as gp,
            tc.tile_pool(name="xsp", bufs=3) as xsp,
            tc.tile_pool(name="t1p", bufs=3) as t1p,
            tc.tile_pool(name="outp", bufs=3) as outp,
            tc.tile_pool(name="psum", bufs=8, space="PSUM") as psum,
        ):
            def emit_st2(T1, grs, img):
                # stage 2 (row conv): Z[v,x] = sum_w T1[w,v] Gr[w,x]
                outs = outp.tile([P, 4, N], f32, tag="outs")
                for m in range(4):
                    msl = slice(m * P, (m + 1) * P)
                    pb = psum.tile([P, N], f32, tag="ps", name="pb")
                    for kk in range(4):
                        nc.tensor.matmul(
                            pb[:], T1[:, kk, msl], grs[:, kk, :],
                            start=(kk == 0), stop=(kk == 3),
                        )
                    nc.any.tensor_copy(out=outs[:, m, :], in_=pb[:])
                nc.sync.dma_start(
                    out_d[img].rearrange("(i p) n -> p i n", p=P), outs[:]
                )

            pending = None  # (T1, grs, img) of the previous image

            for s in range(SPC):
                gcs = gp.tile([P, 4, N], bf16, tag="gcs")
                grs = gp.tile([P, 4, N], bf16, tag="grs")
                nc.sync.dma_start(
                    gcs[:], gc_d[s].rearrange("(i p) n -> p i n", p=P)
                )
                nc.sync.dma_start(
                    grs[:], gr_d[s].rearrange("(i p) n -> p i n", p=P)
                )
                for ch in range(CHANNELS):
                    img = s * CHANNELS + ch
                    xs = xsp.tile([P, 4, N], bf16, tag="xs")
                    nc.sync.dma_start(
                        xs[:], x_d[img].rearrange("(i p) n -> p i n", p=P)
                    )
                    # stage 1 (col conv): T1[w,v] = sum_h X[h,w] Gc[h,v]
                    T1 = t1p.tile([P, 4, N], bf16, tag="T1")
                    for m in range(4):
                        msl = slice(m * P, (m + 1) * P)
                        pa = psum.tile([P, N], f32, tag="ps", name="pa")
                        for kk in range(4):
                            nc.tensor.matmul(
                                pa[:], xs[:, kk, msl], gcs[:, kk, :],
                                start=(kk == 0), stop=(kk == 3),
                            )
                        nc.scalar.copy(out=T1[:, m, :], in_=pa[:])
                    # fill the copy-latency bubble with the previous image's
                    # stage 2 (independent PE work)
                    if pending is not None:
                        emit_st2(*pending)
                    pending = (T1, grs, img)

            if pending is not None:
                emit_st2(*pending)

    nc.compile()
    return nc


def _composed_kernels(kernels):
    """Exact composed spatial kernels k_t, [T+1, N, N] float64 (k_0 = delta).
    out_t = k_t (*) x as a 2D circular convolution."""
    kernels = np.asarray(kernels, dtype=np.float64)
    h = np.zeros((T_STEPS, N, N), np.float64)
    idx = (KS // 2 - np.arange(KS)) % N
    h[:, idx[:, None], idx[None, :]] = kernels
    s_step = np.fft.fft2(h)
    cum = np.empty((T_STEPS + 1, N, N), np.complex128)
    cum[0] = 1.0
    for i in range(1, T_STEPS + 1):
        cum[i] = cum[i - 1] * s_step[i - 1]
    return np.fft.ifft2(cum).real


def _rank1_factors(k_sp):
    """If every composed kernel is rank-1, return (A, B) with
    k_sp[t] == outer(A[t], B[t]); else None."""
    A = np.zeros((T_STEPS + 1, N))
    B = np.zeros((T_STEPS + 1, N))
    for tl in range(T_STEPS + 1):
        K = k_sp[tl]
        am = np.abs(K).max()
        if am == 0.0:
            return None
        i0, j0 = np.unravel_index(np.abs(K).argmax(), K.shape)
        a = K[:, j0] / K[i0, j0]
        b = K[i0, :]
        if np.abs(K - np.outer(a, b)).max() > 1e-6 * am:
            return None
        A[tl] = a
        B[tl] = b
    return A, B


_SHIFT_IDX = (np.arange(N)[None, :] - np.arange(N)[:, None]) % N


def _circulant(vec):
    # M[h, v] = vec[(v - h) mod N]
    return vec[_SHIFT_IDX]


def _kernel_separable(x0, tt, A, B):
    global LAST_EXEC_NS, LAST_TRACE
    from concourse import bass_utils
    import ml_dtypes

    if "sep" not in _PROGRAMS:
        _PROGRAMS["sep"] = _build_program_sep()
    nc = _PROGRAMS["sep"]

    bf = ml_dtypes.bfloat16
    GC = {}
    GR = {}
    for tv in np.unique(tt):
        tv = int(tv)
        GC[tv] = _circulant(A[tv]).astype(bf)
        GR[tv] = _circulant(B[tv]).astype(bf)
    xb = np.asarray(x0, dtype=np.float32).astype(bf)

    in_maps = []
    for c in range(NCORES):
        sl = slice(c * SPC, (c + 1) * SPC)
        ts = tt[sl]
        in_maps.append({
            "x": np.ascontiguousarray(xb[sl].reshape(IMGS, N, N)),
            "gc": np.stack([GC[int(tv)] for tv in ts]),
            "gr": np.stack([GR[int(tv)] for tv in ts]),
        })

    res = bass_utils.run_bass_kernel_spmd(
        nc, in_maps, core_ids=list(range(NCORES)), trace=TRACE
    )
    LAST_EXEC_NS = res.exec_time_ns
    if res.instructions_and_trace is not None:
        LAST_TRACE = res.instructions_and_trace[1]
    out = np.empty((BATCH, CHANNELS, N, N), np.float32)
    for c in range(NCORES):
        out[c * SPC : (c + 1) * SPC] = res.results[c]["out"].reshape(
            SPC, CHANNELS, N, N
        )
    return out


# --------------------------------------------------------------------------
# Spectral fallback (general, possibly non-separable kernels)
# --------------------------------------------------------------------------

def _build_program_spec():
    import concourse.mybir as mybir
    import concourse.tile as tile
    from concourse import bacc

    f32 = mybir.dt.float32
    f32r = mybir.dt.float32r
    mmdt = f32r if USE_F32R else f32

    nc = bacc.Bacc(
        "TRN2", target_bir_lowering=False, debug=False, num_devices=NCORES
    )
    x_d = nc.dram_tensor("x", [IMGS, N, N], mmdt, kind="ExternalInput").ap()
    fkr_d = nc.dram_tensor("fkr", [SPC, N, KP], f32, kind="ExternalInput").ap()
    fki_d = nc.dram_tensor("fki", [SPC, N, KP], f32, kind="ExternalInput").ap()
    mat_names = ["cmat", "smat", "snmat", "nscmat", "cmsmat"]
    mat_d = {
        nm: nc.dram_tensor(nm, [N, N], mmdt, kind="ExternalInput").ap()
        for nm in mat_names
    }
    out_d = nc.dram_tensor("out", [IMGS, N, N], f32, kind="ExternalOutput").ap()

    with tile.TileContext(nc) as tc:
        with (
            tc.tile_pool(name="mats", bufs=1) as mats,
            tc.tile_pool(name="xsp", bufs=2) as xsp,
            tc.tile_pool(name="outp", bufs=2) as outp,
            tc.tile_pool(name="fkp", bufs=2) as fkp,
            tc.tile_pool(name="apool", bufs=2) as apool,
            tc.tile_pool(name="hpool", bufs=2) as hpool,
            tc.tile_pool(name="zpool", bufs=2) as zpool,
            tc.tile_pool(name="vpool", bufs=2) as vpool,
            tc.tile_pool(name="pw", bufs=3) as pw,
            tc.tile_pool(name="psum", bufs=8, space="PSUM") as psum,
        ):
            # resident DFT matrices, [p, tile, n] layout
            M = {}
            for nm in mat_names:
                mt = mats.tile([P, 4, N], mmdt, name=nm + "_s")
                nc.sync.dma_start(mt[:], mat_d[nm].rearrange("(i p) n -> p i n", p=P))
                M[nm] = mt
            Cs, Ss, Sns = M["cmat"], M["smat"], M["snmat"]
            nSCs, CmSs = M["nscmat"], M["cmsmat"]
            # C/-S rows 130..257, partition-aligned (stage-4 k tail tile)
            Ck2s = mats.tile([P, N], mmdt, name="ck2_s")
            Snk2s = mats.tile([P, N], mmdt, name="snk2_s")
            nc.sync.dma_start(Ck2s[:], mat_d["cmat"][130:258, :])
            nc.sync.dma_start(Snk2s[:], mat_d["snmat"][130:258, :])

            def emit_st4(Vr, Vi, img):
                # ---- Stage 4 (matrix-stationary, natural orientation):
                # out[y,x] = sum_k C[k,y] Vr[k,x] + (-S)[k,y] Vi[k,x]
                # k tiles: 0:128, 128:256, 130:258 (FK pre-halved on the
                # double-counted 130..255 range)
                outs = outp.tile([P, 4, N], f32, tag="outs")
                for ym in range(4):
                    ysl = slice(ym * P, (ym + 1) * P)
                    po = psum.tile([P, N], f32, tag="ps", name="po")
                    nc.tensor.matmul(
                        po[:], Cs[:, 0, ysl], Vr[:, 0, :],
                        start=True, stop=False,
                    )
                    nc.tensor.matmul(
                        po[:], Cs[:, 1, ysl], Vr[:, 1, :],
                        start=False, stop=False,
                    )
                    nc.tensor.matmul(
                        po[:], Ck2s[:, ysl], Vr[:, 2, :],
                        start=False, stop=False,
                    )
                    nc.tensor.matmul(
                        po[:], Sns[:, 0, ysl], Vi[:, 0, :],
                        start=False, stop=False,
                    )
                    nc.tensor.matmul(
                        po[:], Sns[:, 1, ysl], Vi[:, 1, :],
                        start=False, stop=False,
                    )
                    nc.tensor.matmul(
                        po[:], Snk2s[:, ysl], Vi[:, 2, :],
                        start=False, stop=True,
                    )
                    nc.any.tensor_copy(out=outs[:, ym, :], in_=po[:])
                nc.sync.dma_start(
                    out_d[img].rearrange("(i p) n -> p i n", p=P), outs[:]
                )

            pending = None  # (Vr, Vi, img) of the previous image

            for s in range(SPC):
                # per-sample spectrum, transposed [l, k] layout, k cols 0..257
                fktr = fkp.tile([P, 4, KP], f32, tag="fktr")
                fkti = fkp.tile([P, 4, KP], f32, tag="fkti")
                nc.sync.dma_start(
                    fktr[:], fkr_d[s].rearrange("(i p) n -> p i n", p=P)
                )
                nc.sync.dma_start(
                    fkti[:], fki_d[s].rearrange("(i p) n -> p i n", p=P)
                )

                for ch in range(CHANNELS):
                    img = s * CHANNELS + ch
                    xs = xsp.tile([P, 4, N], mmdt, tag="xs")
                    nc.sync.dma_start(
                        xs[:], x_d[img].rearrange("(i p) n -> p i n", p=P)
                    )

                    # ---- Stage 1 (data-stationary, weight-paired):
                    # A_r = x^T C[:, :258] ; A_i = x^T (-S)[:, :258]
                    Ar = apool.tile([P, 4, KP], mmdt, tag="Ar")
                    Ai = apool.tile([P, 4, KP], mmdt, tag="Ai")
                    Apb = apool.tile([P, 4, KP], mmdt, tag="Apb")
                    for m in range(4):
                        msl = slice(m * P, (m + 1) * P)
                        pa = psum.tile([P, N], f32, tag="ps", name="pa")[:, :KP]
                        pb = psum.tile([P, N], f32, tag="ps", name="pb")[:, :KP]
                        for kk in range(4):
                            nc.tensor.matmul(
                                pa[:], xs[:, kk, msl], Cs[:, kk, 0:KP],
                                start=(kk == 0), stop=(kk == 3),
                            )
                            nc.tensor.matmul(
                                pb[:], xs[:, kk, msl], Sns[:, kk, 0:KP],
                                start=(kk == 0), stop=(kk == 3),
                            )
                        nc.scalar.copy(out=Ar[:, m, :], in_=pa[:])
                        nc.scalar.copy(out=Ai[:, m, :], in_=pb[:])
                        nc.gpsimd.tensor_tensor(
                            Apb[:, m, :], Ar[:, m, :], Ai[:, m, :],
                            mybir.AluOpType.add,
                        )

                    # ---- Stage 2 (matrix-stationary, Gauss):
                    # m1 = C.(Ar+Ai) ; m2 = (-S-C).Ar ; m3 = (C-S).Ai
                    # Htr = m1 - m3 ; Hti = m1 + m2 ; pointwise per l-tile
                    Ztr = zpool.tile([P, 4, KP], mmdt, tag="Ztr")
                    Zti = zpool.tile([P, 4, KP], mmdt, tag="Zti")
                    for lm in range(4):
                        lsl = slice(lm * P, (lm + 1) * P)
                        m1 = psum.tile([P, N], f32, tag="ps", name="m1")[:, :KP]
                        m2 = psum.tile([P, N], f32, tag="ps", name="m2")[:, :KP]
                        m3 = psum.tile([P, N], f32, tag="ps", name="m3")[:, :KP]
                        for kk in range(4):
                            nc.tensor.matmul(
                                m1[:], Cs[:, kk, lsl], Apb[:, kk, :],
                                start=(kk == 0), stop=(kk == 3),
                            )
                        for kk in range(4):
                            nc.tensor.matmul(
                                m2[:], nSCs[:, kk, lsl], Ar[:, kk, :],
                                start=(kk == 0), stop=(kk == 3),
                            )
                        for kk in range(4):
                            nc.tensor.matmul(
                                m3[:], CmSs[:, kk, lsl], Ai[:, kk, :],
                                start=(kk == 0), stop=(kk == 3),
                            )
                        # Htr = m1 - m3 ; Hti = m1 + m2
                        m1s = pw.tile([P, KP], f32, tag="m1s")
                        hrs = pw.tile([P, KP], f32, tag="hrs")
                        his = pw.tile([P, KP], f32, tag="his")
                        nc.scalar.copy(out=m1s[:], in_=m1[:])
                        nc.vector.tensor_sub(out=hrs[:], in0=m1s[:], in1=m3[:])
                        nc.vector.tensor_add(out=his[:], in0=m1s[:], in1=m2[:])
                        # pointwise: Ztr = hr o fr - hi o fi
                        #            Zti = hr o fi + hi o fr
                        fr = fktr[:, lm, :]
                        fi = fkti[:, lm, :]
                        tt = pw.tile([P, KP], f32, tag="tt")
                        tu = pw.tile([P, KP], f32, tag="tu")
                        ztr = Ztr[:, lm, :]
                        zti = Zti[:, lm, :]
                        nc.vector.tensor_mul(out=ztr, in0=hrs[:], in1=fr)
                        nc.gpsimd.tensor_tensor(
                            tt[:], his[:], fi, mybir.AluOpType.mult
                        )
                        nc.vector.tensor_sub(out=ztr, in0=ztr, in1=tt[:])
                        nc.gpsimd.tensor_tensor(
                            tu[:], hrs[:], fi, mybir.AluOpType.mult
                        )
                        nc.vector.tensor_mul(out=zti, in0=his[:], in1=fr)
                        nc.vector.tensor_add(out=zti, in0=zti, in1=tu[:])

                    # fill the pointwise-latency bubble with the previous
                    # image's stage 4 (independent PE work)
                    if pending is not None:
                        emit_st4(*pending)
                        pending = None

                    # ---- Stage 3 (data-stationary, weight-paired direct):
                    # V_r = Ztr^T C + Zti^T (-S) ; V_i = Ztr^T S + Zti^T C
                    # k M-tiles: 0:128, 128:256, 130:258
                    Vr = vpool.tile([P, 3, N], mmdt, tag="Vr")
                    Vi = vpool.tile([P, 3, N], mmdt, tag="Vi")
                    for km in range(3):
                        koff = (0, 128, 130)[km]
                        ksl = slice(koff, koff + P)
                        nvr = psum.tile([P, N], f32, tag="ps", name="nvr")
                        nvi = psum.tile([P, N], f32, tag="ps", name="nvi")
                        for lt in range(4):
                            nc.tensor.matmul(
                                nvr[:], Ztr[:, lt, ksl], Cs[:, lt, :],
                                start=(lt == 0), stop=False,
                            )
                            nc.tensor.matmul(
                                nvi[:], Ztr[:, lt, ksl], Ss[:, lt, :],
                                start=(lt == 0), stop=False,
                            )
                        for lt in range(4):
                            nc.tensor.matmul(
                                nvr[:], Zti[:, lt, ksl], Sns[:, lt, :],
                                start=False, stop=(lt == 3),
                            )
                            nc.tensor.matmul(
                                nvi[:], Zti[:, lt, ksl], Cs[:, lt, :],
                                start=False, stop=(lt == 3),
                            )
                        nc.any.tensor_copy(out=Vr[:, km, :], in_=nvr[:])
                        nc.any.tensor_copy(out=Vi[:, km, :], in_=nvi[:])

                    pending = (Vr, Vi, img)


            if pending is not None:
                emit_st4(*pending)

    nc.compile()
    return nc


def _host_spectra(kernels):
    """Compose step kernels into 21 cumulative half-spectra, transposed to
    [l, k] layout with Hermitian weights, 1/N^2, and the stage-4
    double-count halving folded in. Returns (FKtr, FKti) f32 [21, 512, KP]."""
    kernels = np.asarray(kernels, dtype=np.float64)
    h = np.zeros((T_STEPS, N, N), np.float64)
    idx = (KS // 2 - np.arange(KS)) % N
    h[:, idx[:, None], idx[None, :]] = kernels
    s_step = np.fft.fft2(h)
    cum = np.ones((T_STEPS + 1, N, N), np.complex128)
    for i in range(1, T_STEPS + 1):
        cum[i] = cum[i - 1] * s_step[i - 1]
    w = np.zeros(KP)
    w[: N // 2 + 1] = 2.0
    w[0] = w[N // 2] = 1.0
    fkt = (cum[:, :KP, :] * w[None, :, None] / float(N * N)).transpose(0, 2, 1)
    half = np.ones(KP)
    half[130:256] = 0.5  # k rows 130..255 appear in both stage-4 k-tiles
    fkt = fkt * half[None, None, :]
    return (
        np.ascontiguousarray(fkt.real.astype(np.float32)),
        np.ascontiguousarray(fkt.imag.astype(np.float32)),
    )


def _dft_mats():
    j = np.arange(N)
    ang = 2.0 * np.pi * (np.outer(j, j) % N) / N
    cm = np.cos(ang).astype(np.float32)
    sm = np.sin(ang).astype(np.float32)
    return {
        "cmat": cm,
        "smat": sm,
        "snmat": np.ascontiguousarray(-sm),
        "nscmat": np.ascontiguousarray(-sm - cm),
        "cmsmat": np.ascontiguousarray(cm - sm),
    }


def _kernel_spectral(x0, tt, kernels):
    global LAST_EXEC_NS, LAST_TRACE
    from concourse import bass_utils

    x0 = np.ascontiguousarray(np.asarray(x0), dtype=np.float32)
    fktr_all, fkti_all = _host_spectra(kernels)
    mats = _dft_mats()

    if "spec" not in _PROGRAMS:
        _PROGRAMS["spec"] = _build_program_spec()
    nc = _PROGRAMS["spec"]

    in_maps = []
    for c in range(NCORES):
        sl = slice(c * SPC, (c + 1) * SPC)
        ts = tt[sl]
        im = {
            "x": np.ascontiguousarray(x0[sl].reshape(IMGS, N, N)),
            "fkr": np.ascontiguousarray(fktr_all[ts]),
            "fki": np.ascontiguousarray(fkti_all[ts]),
        }
        im.update(mats)
        in_maps.append(im)

    res = bass_utils.run_bass_kernel_spmd(
        nc, in_maps, core_ids=list(range(NCORES)), trace=TRACE
    )
    LAST_EXEC_NS = res.exec_time_ns
    if res.instructions_and_trace is not None:
        LAST_TRACE = res.instructions_and_trace[1]
    out = np.empty((BATCH, CHANNELS, N, N), np.float32)
    for c in range(NCORES):
        out[c * SPC : (c + 1) * SPC] = res.results[c]["out"].reshape(
            SPC, CHANNELS, N, N
        )
    return out


def kernel(x0, t, kernels):
    tt = np.asarray(t).astype(np.int64)
    k_sp = _composed_kernels(kernels)
    fac = _rank1_factors(k_sp)
    if fac is not None:
        return _kernel_separable(x0, tt, fac[0], fac[1])
    return _kernel_spectral(x0, tt, kernels)


# revision 9
# speedup vs baseline: 5.2621x; 1.3894x over previous
"""BlurDegradation kernel for 8x TRN2 NeuronCores.

Math: t[b] successive 11x11 depthwise *circular* convolutions compose into a
single circular convolution with kernel k_t = h_1 (*) h_2 (*) ... (*) h_t
(circular 2D convolution of the per-step impulse responses). The host
composes k_t exactly with FFTs.

Fast path (separable): when every composed k_t is numerically rank-1
(k_t = outer(a_t, b_t) -- always true for the constant-Gaussian blur routine,
whose steps are separable), the 2D blur factors into a column circular conv
by a_t followed by a row circular conv by b_t. Each is one dense circulant
matmul, so an image costs just two 512x512x512 matmul chains on the PE array
(268M MACs vs 941M for the spectral pipeline). Both stages are
data-stationary so outputs chain orientation [h,w] -> [w,v] -> [v,x] with no
transposes:
  stage 1: T1[w,v] = sum_h X[h,w] * Gc[h,v],  Gc[h,v] = a[(v-h) mod N]
  stage 2: Z [v,x] = sum_w T1[w,v] * Gr[w,x], Gr[w,x] = b[(x-w) mod N]
Matmuls run in bf16 (1 cycle/row + fast weight load) with fp32 PSUM
accumulation; stage-2 of image i is emitted after stage-1 of image i+1 so the
PSUM->SBUF cast copies never stall the PE.

Fallback path (general kernels): the original spectral pipeline -- per-sample
cumulative half-spectra multiply between dense DFT matmuls.

Sharding: pure data parallel, 8 samples per core, no cross-core comms.
"""

import numpy as np

N = 512
P = 128
T_STEPS = 20
KS = 11
KP = 258            # padded half-spectrum k-dim (even for fp32r)
NCORES = 8
BATCH = 64
CHANNELS = 3
SPC = BATCH // NCORES  # samples per core
IMGS = SPC * CHANNELS  # images per core

USE_F32R = True

_PROGRAMS = {}
TRACE = False
LAST_EXEC_NS = None
LAST_TRACE = None


# --------------------------------------------------------------------------
# Separable fast path
# --------------------------------------------------------------------------

# The composed blur kernel after t steps has support <= +-5t, so the
# circulant G[a, b] = vec[(b-a) mod 512] is band-limited: the [128 x 512]
# tile for contraction block kk has nonzero columns only in the contiguous
# (mod 512) span [128kk - W, 128(kk+1) + W). Samples are globally sorted by
# t so that slot j on every core shares a compiled band halfwidth W_j.
# PSUM's per-element has_written bit makes partial-column accumulation work:
# the group's first matmul (start=True) clears the whole bank's bits, later
# matmuls overwrite where unwritten and accumulate where written.
def _plan_spans(W):
    """Per contraction tile kk: fused contiguous column ranges of its band
    (split only at the mod-512 wrap). Returns [(kk, c0, c1), ...]."""
    plans = []
    for kk in range(4):
        c0 = (P * kk - W) % N
        ln = P + 2 * W
        if ln >= N:
            plans.append((kk, 0, N))
        elif c0 + ln <= N:
            plans.append((kk, c0, c0 + ln))
        else:
            plans.append((kk, c0, N))
            plans.append((kk, 0, c0 + ln - N))
    return plans


def _build_program_sep(Wj):
    import concourse.mybir as mybir
    import concourse.tile as tile
    from concourse import bacc

    f32 = mybir.dt.float32
    bf16 = mybir.dt.bfloat16

    nc = bacc.Bacc(
        "TRN2", target_bir_lowering=False, debug=False, num_devices=NCORES
    )
    x_d = nc.dram_tensor("x", [IMGS, N, N], bf16, kind="ExternalInput").ap()
    gc_d = nc.dram_tensor("gc", [SPC, N, N], bf16, kind="ExternalInput").ap()
    gr_d = nc.dram_tensor("gr", [SPC, N, N], bf16, kind="ExternalInput").ap()
    out_d = nc.dram_tensor("out", [IMGS, N, N], bf16, kind="ExternalOutput").ap()

    with tile.TileContext(nc) as tc:
        with (
            tc.tile_pool(name="gp", bufs=2) as gp,
            tc.tile_pool(name="xsp", bufs=3) as xsp,
            tc.tile_pool(name="t1p", bufs=3) as t1p,
            tc.tile_pool(name="outp", bufs=3) as outp,
            tc.tile_pool(name="psum", bufs=8, space="PSUM") as psum,
        ):
            def banded_mms(pa, data, msl, g, plans):
                # fused band spans; only the group's first matmul starts the
                # bank (whole-bank has_written clear), the rest accumulate
                # per-element. Consecutive same-kk matmuls share LDWEIGHTS.
                last = len(plans) - 1
                for i, (kk, c0, c1) in enumerate(plans):
                    nc.tensor.matmul(
                        pa[:, c0:c1], data[:, kk, msl], g[:, kk, c0:c1],
                        start=(i == 0), stop=(i == last),
                    )

            def emit_st2(T1, grs, img, plans):
                # stage 2 (row conv): Z[v,x] = sum_w T1[w,v] Gr[w,x]
                outs = outp.tile([P, 4, N], bf16, tag="outs")
                for m in range(4):
                    msl = slice(m * P, (m + 1) * P)
                    pb = psum.tile([P, N], f32, tag="ps", name="pb")
                    banded_mms(pb, T1, msl, grs, plans)
                    if m % 2 == 0:
                        nc.vector.tensor_copy(out=outs[:, m, :], in_=pb[:])
                    else:
                        nc.scalar.copy(out=outs[:, m, :], in_=pb[:])
                nc.sync.dma_start(
                    out_d[img].rearrange("(i p) n -> p i n", p=P), outs[:]
                )

            pending = None  # (T1, grs, img, plans) of the previous image

            for s in range(SPC):
                plans = _plan_spans(Wj[s])
                gcs = gp.tile([P, 4, N], bf16, tag="gcs")
                grs = gp.tile([P, 4, N], bf16, tag="grs")
                nc.sync.dma_start(
                    gcs[:], gc_d[s].rearrange("(i p) n -> p i n", p=P)
                )
                nc.sync.dma_start(
                    grs[:], gr_d[s].rearrange("(i p) n -> p i n", p=P)
                )
                for ch in range(CHANNELS):
                    img = s * CHANNELS + ch
                    xs = xsp.tile([P, 4, N], bf16, tag="xs")
                    nc.sync.dma_start(
                        xs[:], x_d[img].rearrange("(i p) n -> p i n", p=P)
                    )
                    # stage 1 (col conv): T1[w,v] = sum_h X[h,w] Gc[h,v]
                    T1 = t1p.tile([P, 4, N], bf16, tag="T1")
                    for m in range(4):
                        msl = slice(m * P, (m + 1) * P)
                        pa = psum.tile([P, N], f32, tag="ps", name="pa")
                        banded_mms(pa, xs, msl, gcs, plans)
                        if m % 2 == 0:
                            nc.vector.tensor_copy(out=T1[:, m, :], in_=pa[:])
                        else:
                            nc.scalar.copy(out=T1[:, m, :], in_=pa[:])
                    # fill the copy-latency bubble with the previous image's
                    # stage 2 (independent PE work)
                    if pending is not None:
                        emit_st2(*pending)
                    pending = (T1, grs, img, plans)

            if pending is not None:
                emit_st2(*pending)

    nc.compile()
    return nc


def _composed_kernels(kernels):
    """Exact composed spatial kernels k_t, [T+1, N, N] float64 (k_0 = delta).
    out_t = k_t (*) x as a 2D circular convolution."""
    kernels = np.asarray(kernels, dtype=np.float64)
    h = np.zeros((T_STEPS, N, N), np.float64)
    idx = (KS // 2 - np.arange(KS)) % N
    h[:, idx[:, None], idx[None, :]] = kernels
    s_step = np.fft.fft2(h)
    cum = np.empty((T_STEPS + 1, N, N), np.complex128)
    cum[0] = 1.0
    for i in range(1, T_STEPS + 1):
        cum[i] = cum[i - 1] * s_step[i - 1]
    return np.fft.ifft2(cum).real


def _rank1_factors(k_sp):
    """If every composed kernel is rank-1, return (A, B) with
    k_sp[t] == outer(A[t], B[t]); else None."""
    A = np.zeros((T_STEPS + 1, N))
    B = np.zeros((T_STEPS + 1, N))
    for tl in range(T_STEPS + 1):
        K = k_sp[tl]
        am = np.abs(K).max()
        if am == 0.0:
            return None
        i0, j0 = np.unravel_index(np.abs(K).argmax(), K.shape)
        a = K[:, j0] / K[i0, j0]
        b = K[i0, :]
        if np.abs(K - np.outer(a, b)).max() > 1e-6 * am:
            return None
        A[tl] = a
        B[tl] = b
    return A, B


_SHIFT_IDX = (np.arange(N)[None, :] - np.arange(N)[:, None]) % N


def _circulant(vec):
    # M[h, v] = vec[(v - h) mod N]
    return vec[_SHIFT_IDX]


def _kernel_separable(x0, tt, A, B):
    global LAST_EXEC_NS, LAST_TRACE
    from concourse import bass_utils
    import ml_dtypes

    # sort samples by t; core c slot j <- sorted position 8j + c, so every
    # core's slot j shares the compiled band halfwidth W_j = 5 * max t
    order = np.argsort(tt, kind="stable")
    Wj = tuple(min(5 * int(tt[order[NCORES * j + NCORES - 1]]), P - 1)
               for j in range(SPC))

    key = ("sep", Wj)
    if key not in _PROGRAMS:
        _PROGRAMS[key] = _build_program_sep(Wj)
    nc = _PROGRAMS[key]

    bf = ml_dtypes.bfloat16
    GC = {}
    GR = {}
    for tv in np.unique(tt):
        tv = int(tv)
        W = min(5 * tv, P - 1)
        a = A[tv].copy()
        b = B[tv].copy()
        a[W + 1 : N - W] = 0.0  # drop FFT noise outside the structural band
        b[W + 1 : N - W] = 0.0
        GC[tv] = _circulant(a).astype(bf)
        GR[tv] = _circulant(b).astype(bf)
    xb = np.asarray(x0, dtype=np.float32).astype(bf)

    in_maps = []
    origs = []
    for c in range(NCORES):
        orig = order[np.arange(SPC) * NCORES + c]  # slot j -> original sample
        origs.append(orig)
        ts = tt[orig]
        in_maps.append({
            "x": np.ascontiguousarray(xb[orig].reshape(IMGS, N, N)),
            "gc": np.stack([GC[int(tv)] for tv in ts]),
            "gr": np.stack([GR[int(tv)] for tv in ts]),
        })

    res = bass_utils.run_bass_kernel_spmd(
        nc, in_maps, core_ids=list(range(NCORES)), trace=TRACE
    )
    LAST_EXEC_NS = res.exec_time_ns
    if res.instructions_and_trace is not None:
        LAST_TRACE = res.instructions_and_trace[1]
    out = np.empty((BATCH, CHANNELS, N, N), np.float32)
    for c in range(NCORES):
        out[origs[c]] = (
            np.asarray(res.results[c]["out"])
            .astype(np.float32)
            .reshape(SPC, CHANNELS, N, N)
        )
    return out


# --------------------------------------------------------------------------
# Spectral fallback (general, possibly non-separable kernels)
# --------------------------------------------------------------------------

def _build_program_spec():
    import concourse.mybir as mybir
    import concourse.tile as tile
    from concourse import bacc

    f32 = mybir.dt.float32
    f32r = mybir.dt.float32r
    mmdt = f32r if USE_F32R else f32

    nc = bacc.Bacc(
        "TRN2", target_bir_lowering=False, debug=False, num_devices=NCORES
    )
    x_d = nc.dram_tensor("x", [IMGS, N, N], mmdt, kind="ExternalInput").ap()
    fkr_d = nc.dram_tensor("fkr", [SPC, N, KP], f32, kind="ExternalInput").ap()
    fki_d = nc.dram_tensor("fki", [SPC, N, KP], f32, kind="ExternalInput").ap()
    mat_names = ["cmat", "smat", "snmat", "nscmat", "cmsmat"]
    mat_d = {
        nm: nc.dram_tensor(nm, [N, N], mmdt, kind="ExternalInput").ap()
        for nm in mat_names
    }
    out_d = nc.dram_tensor("out", [IMGS, N, N], f32, kind="ExternalOutput").ap()

    with tile.TileContext(nc) as tc:
        with (
            tc.tile_pool(name="mats", bufs=1) as mats,
            tc.tile_pool(name="xsp", bufs=2) as xsp,
            tc.tile_pool(name="outp", bufs=2) as outp,
            tc.tile_pool(name="fkp", bufs=2) as fkp,
            tc.tile_pool(name="apool", bufs=2) as apool,
            tc.tile_pool(name="hpool", bufs=2) as hpool,
            tc.tile_pool(name="zpool", bufs=2) as zpool,
            tc.tile_pool(name="vpool", bufs=2) as vpool,
            tc.tile_pool(name="pw", bufs=3) as pw,
            tc.tile_pool(name="psum", bufs=8, space="PSUM") as psum,
        ):
            # resident DFT matrices, [p, tile, n] layout
            M = {}
            for nm in mat_names:
                mt = mats.tile([P, 4, N], mmdt, name=nm + "_s")
                nc.sync.dma_start(mt[:], mat_d[nm].rearrange("(i p) n -> p i n", p=P))
                M[nm] = mt
            Cs, Ss, Sns = M["cmat"], M["smat"], M["snmat"]
            nSCs, CmSs = M["nscmat"], M["cmsmat"]
            # C/-S rows 130..257, partition-aligned (stage-4 k tail tile)
            Ck2s = mats.tile([P, N], mmdt, name="ck2_s")
            Snk2s = mats.tile([P, N], mmdt, name="snk2_s")
            nc.sync.dma_start(Ck2s[:], mat_d["cmat"][130:258, :])
            nc.sync.dma_start(Snk2s[:], mat_d["snmat"][130:258, :])

            def emit_st4(Vr, Vi, img):
                # ---- Stage 4 (matrix-stationary, natural orientation):
                # out[y,x] = sum_k C[k,y] Vr[k,x] + (-S)[k,y] Vi[k,x]
                # k tiles: 0:128, 128:256, 130:258 (FK pre-halved on the
                # double-counted 130..255 range)
                outs = outp.tile([P, 4, N], f32, tag="outs")
                for ym in range(4):
                    ysl = slice(ym * P, (ym + 1) * P)
                    po = psum.tile([P, N], f32, tag="ps", name="po")
                    nc.tensor.matmul(
                        po[:], Cs[:, 0, ysl], Vr[:, 0, :],
                        start=True, stop=False,
                    )
                    nc.tensor.matmul(
                        po[:], Cs[:, 1, ysl], Vr[:, 1, :],
                        start=False, stop=False,
                    )
                    nc.tensor.matmul(
                        po[:], Ck2s[:, ysl], Vr[:, 2, :],
                        start=False, stop=False,
                    )
                    nc.tensor.matmul(
                        po[:], Sns[:, 0, ysl], Vi[:, 0, :],
                        start=False, stop=False,
                    )
                    nc.tensor.matmul(
                        po[:], Sns[:, 1, ysl], Vi[:, 1, :],
                        start=False, stop=False,
                    )
                    nc.tensor.matmul(
                        po[:], Snk2s[:, ysl], Vi[:, 2, :],
                        start=False, stop=True,
                    )
                    nc.any.tensor_copy(out=outs[:, ym, :], in_=po[:])
                nc.sync.dma_start(
                    out_d[img].rearrange("(i p) n -> p i n", p=P), outs[:]
                )

            pending = None  # (Vr, Vi, img) of the previous image

            for s in range(SPC):
                # per-sample spectrum, transposed [l, k] layout, k cols 0..257
                fktr = fkp.tile([P, 4, KP], f32, tag="fktr")
                fkti = fkp.tile([P, 4, KP], f32, tag="fkti")
                nc.sync.dma_start(
                    fktr[:], fkr_d[s].rearrange("(i p) n -> p i n", p=P)
                )
                nc.sync.dma_start(
                    fkti[:], fki_d[s].rearrange("(i p) n -> p i n", p=P)
                )

                for ch in range(CHANNELS):
                    img = s * CHANNELS + ch
                    xs = xsp.tile([P, 4, N], mmdt, tag="xs")
                    nc.sync.dma_start(
                        xs[:], x_d[img].rearrange("(i p) n -> p i n", p=P)
                    )

                    # ---- Stage 1 (data-stationary, weight-paired):
                    # A_r = x^T C[:, :258] ; A_i = x^T (-S)[:, :258]
                    Ar = apool.tile([P, 4, KP], mmdt, tag="Ar")
                    Ai = apool.tile([P, 4, KP], mmdt, tag="Ai")
                    Apb = apool.tile([P, 4, KP], mmdt, tag="Apb")
                    for m in range(4):
                        msl = slice(m * P, (m + 1) * P)
                        pa = psum.tile([P, N], f32, tag="ps", name="pa")[:, :KP]
                        pb = psum.tile([P, N], f32, tag="ps", name="pb")[:, :KP]
                        for kk in range(4):
                            nc.tensor.matmul(
                                pa[:], xs[:, kk, msl], Cs[:, kk, 0:KP],
                                start=(kk == 0), stop=(kk == 3),
                            )
                            nc.tensor.matmul(
                                pb[:], xs[:, kk, msl], Sns[:, kk, 0:KP],
                                start=(kk == 0), stop=(kk == 3),
                            )
                        nc.scalar.copy(out=Ar[:, m, :], in_=pa[:])
                        nc.scalar.copy(out=Ai[:, m, :], in_=pb[:])
                        nc.gpsimd.tensor_tensor(
                            Apb[:, m, :], Ar[:, m, :], Ai[:, m, :],
                            mybir.AluOpType.add,
                        )

                    # ---- Stage 2 (matrix-stationary, Gauss):
                    # m1 = C.(Ar+Ai) ; m2 = (-S-C).Ar ; m3 = (C-S).Ai
                    # Htr = m1 - m3 ; Hti = m1 + m2 ; pointwise per l-tile
                    Ztr = zpool.tile([P, 4, KP], mmdt, tag="Ztr")
                    Zti = zpool.tile([P, 4, KP], mmdt, tag="Zti")
                    for lm in range(4):
                        lsl = slice(lm * P, (lm + 1) * P)
                        m1 = psum.tile([P, N], f32, tag="ps", name="m1")[:, :KP]
                        m2 = psum.tile([P, N], f32, tag="ps", name="m2")[:, :KP]
                        m3 = psum.tile([P, N], f32, tag="ps", name="m3")[:, :KP]
                        for kk in range(4):
                            nc.tensor.matmul(
                                m1[:], Cs[:, kk, lsl], Apb[:, kk, :],
                                start=(kk == 0), stop=(kk == 3),
                            )
                        for kk in range(4):
                            nc.tensor.matmul(
                                m2[:], nSCs[:, kk, lsl], Ar[:, kk, :],
                                start=(kk == 0), stop=(kk == 3),
                            )
                        for kk in range(4):
                            nc.tensor.matmul(
                                m3[:], CmSs[:, kk, lsl], Ai[:, kk, :],
                                start=(kk == 0), stop=(kk == 3),
                            )
                        # Htr = m1 - m3 ; Hti = m1 + m2
                        m1s = pw.tile([P, KP], f32, tag="m1s")
                        hrs = pw.tile([P, KP], f32, tag="hrs")
                        his = pw.tile([P, KP], f32, tag="his")
                        nc.scalar.copy(out=m1s[:], in_=m1[:])
                        nc.vector.tensor_sub(out=hrs[:], in0=m1s[:], in1=m3[:])
                        nc.vector.tensor_add(out=his[:], in0=m1s[:], in1=m2[:])
                        # pointwise: Ztr = hr o fr - hi o fi
                        #            Zti = hr o fi + hi o fr
                        fr = fktr[:, lm, :]
                        fi = fkti[:, lm, :]
                        tt = pw.tile([P, KP], f32, tag="tt")
                        tu = pw.tile([P, KP], f32, tag="tu")
                        ztr = Ztr[:, lm, :]
                        zti = Zti[:, lm, :]
                        nc.vector.tensor_mul(out=ztr, in0=hrs[:], in1=fr)
                        nc.gpsimd.tensor_tensor(
                            tt[:], his[:], fi, mybir.AluOpType.mult
                        )
                        nc.vector.tensor_sub(out=ztr, in0=ztr, in1=tt[:])
                        nc.gpsimd.tensor_tensor(
                            tu[:], hrs[:], fi, mybir.AluOpType.mult
                        )
                        nc.vector.tensor_mul(out=zti, in0=his[:], in1=fr)
                        nc.vector.tensor_add(out=zti, in0=zti, in1=tu[:])

                    # fill the pointwise-latency bubble with the previous
                    # image's stage 4 (independent PE work)
                    if pending is not None:
                        emit_st4(*pending)
                        pending = None

                    # ---- Stage 3 (data-stationary, weight-paired direct):
                    # V_r = Ztr^T C + Zti^T (-S) ; V_i = Ztr^T S + Zti^T C
                    # k M-tiles: 0:128, 128:256, 130:258
                    Vr = vpool.tile([P, 3, N], mmdt, tag="Vr")
                    Vi = vpool.tile([P, 3, N], mmdt, tag="Vi")
                    for km in range(3):
                        koff = (0, 128, 130)[km]
                        ksl = slice(koff, koff + P)
                        nvr = psum.tile([P, N], f32, tag="ps", name="nvr")
                        nvi = psum.tile([P, N], f32, tag="ps", name="nvi")
                        for lt in range(4):
                            nc.tensor.matmul(
                                nvr[:], Ztr[:, lt, ksl], Cs[:, lt, :],
                                start=(lt == 0), stop=False,
                            )
                            nc.tensor.matmul(
                                nvi[:], Ztr[:, lt, ksl], Ss[:, lt, :],
                                start=(lt == 0), stop=False,
                            )
                        for lt in range(4):
                            nc.tensor.matmul(
                                nvr[:], Zti[:, lt, ksl], Sns[:, lt, :],
                                start=False, stop=(lt == 3),
                            )
                            nc.tensor.matmul(
                                nvi[:], Zti[:, lt, ksl], Cs[:, lt, :],
                                start=False, stop=(lt == 3),
                            )
                        nc.any.tensor_copy(out=Vr[:, km, :], in_=nvr[:])
                        nc.any.tensor_copy(out=Vi[:, km, :], in_=nvi[:])

                    pending = (Vr, Vi, img)


            if pending is not None:
                emit_st4(*pending)

    nc.compile()
    return nc


def _host_spectra(kernels):
    """Compose step kernels into 21 cumulative half-spectra, transposed to
    [l, k] layout with Hermitian weights, 1/N^2, and the stage-4
    double-count halving folded in. Returns (FKtr, FKti) f32 [21, 512, KP]."""
    kernels = np.asarray(kernels, dtype=np.float64)
    h = np.zeros((T_STEPS, N, N), np.float64)
    idx = (KS // 2 - np.arange(KS)) % N
    h[:, idx[:, None], idx[None, :]] = kernels
    s_step = np.fft.fft2(h)
    cum = np.ones((T_STEPS + 1, N, N), np.complex128)
    for i in range(1, T_STEPS + 1):
        cum[i] = cum[i - 1] * s_step[i - 1]
    w = np.zeros(KP)
    w[: N // 2 + 1] = 2.0
    w[0] = w[N // 2] = 1.0
    fkt = (cum[:, :KP, :] * w[None, :, None] / float(N * N)).transpose(0, 2, 1)
    half = np.ones(KP)
    half[130:256] = 0.5  # k rows 130..255 appear in both stage-4 k-tiles
    fkt = fkt * half[None, None, :]
    return (
        np.ascontiguousarray(fkt.real.astype(np.float32)),
        np.ascontiguousarray(fkt.imag.astype(np.float32)),
    )


def _dft_mats():
    j = np.arange(N)
    ang = 2.0 * np.pi * (np.outer(j, j) % N) / N
    cm = np.cos(ang).astype(np.float32)
    sm = np.sin(ang).astype(np.float32)
    return {
        "cmat": cm,
        "smat": sm,
        "snmat": np.ascontiguousarray(-sm),
        "nscmat": np.ascontiguousarray(-sm - cm),
        "cmsmat": np.ascontiguousarray(cm - sm),
    }


def _kernel_spectral(x0, tt, kernels):
    global LAST_EXEC_NS, LAST_TRACE
    from concourse import bass_utils

    x0 = np.ascontiguousarray(np.asarray(x0), dtype=np.float32)
    fktr_all, fkti_all = _host_spectra(kernels)
    mats = _dft_mats()

    if "spec" not in _PROGRAMS:
        _PROGRAMS["spec"] = _build_program_spec()
    nc = _PROGRAMS["spec"]

    in_maps = []
    for c in range(NCORES):
        sl = slice(c * SPC, (c + 1) * SPC)
        ts = tt[sl]
        im = {
            "x": np.ascontiguousarray(x0[sl].reshape(IMGS, N, N)),
            "fkr": np.ascontiguousarray(fktr_all[ts]),
            "fki": np.ascontiguousarray(fkti_all[ts]),
        }
        im.update(mats)
        in_maps.append(im)

    res = bass_utils.run_bass_kernel_spmd(
        nc, in_maps, core_ids=list(range(NCORES)), trace=TRACE
    )
    LAST_EXEC_NS = res.exec_time_ns
    if res.instructions_and_trace is not None:
        LAST_TRACE = res.instructions_and_trace[1]
    out = np.empty((BATCH, CHANNELS, N, N), np.float32)
    for c in range(NCORES):
        out[c * SPC : (c + 1) * SPC] = res.results[c]["out"].reshape(
            SPC, CHANNELS, N, N
        )
    return out


def kernel(x0, t, kernels):
    tt = np.asarray(t).astype(np.int64)
    k_sp = _composed_kernels(kernels)
    fac = _rank1_factors(k_sp)
    if fac is not None:
        return _kernel_separable(x0, tt, fac[0], fac[1])
    return _kernel_spectral(x0, tt, kernels)


# revision 14
# speedup vs baseline: 6.5180x; 1.2387x over previous
"""BlurDegradation kernel for 8x TRN2 NeuronCores.

Math: t[b] successive 11x11 depthwise *circular* convolutions compose into a
single circular convolution with kernel k_t = h_1 (*) h_2 (*) ... (*) h_t
(circular 2D convolution of the per-step impulse responses). The host
composes k_t exactly with FFTs.

Fast path (separable): when every composed k_t is numerically rank-1
(k_t = outer(a_t, b_t) -- always true for the constant-Gaussian blur routine,
whose steps are separable), the 2D blur factors into a column circular conv
by a_t followed by a row circular conv by b_t. Each is one dense circulant
matmul, so an image costs just two 512x512x512 matmul chains on the PE array
(268M MACs vs 941M for the spectral pipeline). Both stages are
data-stationary so outputs chain orientation [h,w] -> [w,v] -> [v,x] with no
transposes:
  stage 1: T1[w,v] = sum_h X[h,w] * Gc[h,v],  Gc[h,v] = a[(v-h) mod N]
  stage 2: Z [v,x] = sum_w T1[w,v] * Gr[w,x], Gr[w,x] = b[(x-w) mod N]
Matmuls run in bf16 (1 cycle/row + fast weight load) with fp32 PSUM
accumulation; stage-2 of image i is emitted after stage-1 of image i+1 so the
PSUM->SBUF cast copies never stall the PE.

Fallback path (general kernels): the original spectral pipeline -- per-sample
cumulative half-spectra multiply between dense DFT matmuls.

Sharding: pure data parallel, 8 samples per core, no cross-core comms.
"""

import numpy as np

N = 512
P = 128
T_STEPS = 20
KS = 11
KP = 258            # padded half-spectrum k-dim (even for fp32r)
NCORES = 8
BATCH = 64
CHANNELS = 3
SPC = BATCH // NCORES  # samples per core
IMGS = SPC * CHANNELS  # images per core

USE_F32R = True

_PROGRAMS = {}
TRACE = False
LAST_EXEC_NS = None
LAST_TRACE = None


# --------------------------------------------------------------------------
# Separable fast path
# --------------------------------------------------------------------------

# The composed blur kernel after t steps has support <= +-5t, so the
# circulant G[a, b] = vec[(b-a) mod 512] is band-limited: the [128 x 512]
# tile for contraction block kk has nonzero columns only in the contiguous
# (mod 512) span [128kk - W, 128(kk+1) + W). Samples are globally sorted by
# t so that slot j on every core shares a compiled band halfwidth W_j.
# PSUM's per-element has_written bit makes partial-column accumulation work:
# the group's first matmul (start=True) clears the whole bank's bits, later
# matmuls overwrite where unwritten and accumulate where written.
def _plan_spans(W):
    """Per contraction tile kk: fused contiguous column ranges of its band
    (split only at the mod-512 wrap). Returns [(kk, c0, c1), ...]."""
    plans = []
    for kk in range(4):
        c0 = (P * kk - W) % N
        ln = P + 2 * W
        if ln >= N:
            plans.append((kk, 0, N))
        elif c0 + ln <= N:
            plans.append((kk, c0, c0 + ln))
        else:
            plans.append((kk, c0, N))
            plans.append((kk, 0, c0 + ln - N))
    return plans


def _build_program_sep(Wj, shared):
    import concourse.mybir as mybir
    import concourse.tile as tile
    from concourse import bacc

    f32 = mybir.dt.float32
    bf16 = mybir.dt.bfloat16

    nc = bacc.Bacc(
        "TRN2", target_bir_lowering=False, debug=False, num_devices=NCORES
    )
    x_d = nc.dram_tensor("x", [IMGS, N, N], bf16, kind="ExternalInput").ap()
    gc_d = nc.dram_tensor("gc", [SPC, N, N], bf16, kind="ExternalInput").ap()
    gr_d = None
    if not shared:
        gr_d = nc.dram_tensor("gr", [SPC, N, N], bf16, kind="ExternalInput").ap()
    out_d = nc.dram_tensor("out", [IMGS, N, N], bf16, kind="ExternalOutput").ap()

    with tile.TileContext(nc) as tc:
        with (
            tc.tile_pool(name="gp", bufs=4 if shared else 3) as gp,
            tc.tile_pool(name="xsp", bufs=4) as xsp,
            tc.tile_pool(name="t1p", bufs=3) as t1p,
            tc.tile_pool(name="outp", bufs=3) as outp,
            tc.tile_pool(name="psum", bufs=8, space="PSUM") as psum,
        ):
            def banded_mms(pa, data, msl, g, plans):
                # fused band spans; only the group's first matmul starts the
                # bank (whole-bank has_written clear), the rest accumulate
                # per-element. Consecutive same-kk matmuls share LDWEIGHTS.
                last = len(plans) - 1
                for i, (kk, c0, c1) in enumerate(plans):
                    nc.tensor.matmul(
                        pa[:, c0:c1], data[:, kk, msl], g[:, kk, c0:c1],
                        start=(i == 0), stop=(i == last),
                    )

            def emit_st2(T1, grs, img, plans):
                # stage 2 (row conv): Z[v,x] = sum_w T1[w,v] Gr[w,x]
                outs = outp.tile([P, 4, N], bf16, tag="outs")
                for m in range(4):
                    msl = slice(m * P, (m + 1) * P)
                    pb = psum.tile([P, N], f32, tag="ps", name="pb")
                    banded_mms(pb, T1, msl, grs, plans)
                    if m % 2 == 0:
                        nc.vector.tensor_copy(out=outs[:, m, :], in_=pb[:])
                    else:
                        nc.scalar.copy(out=outs[:, m, :], in_=pb[:])
                nc.sync.dma_start(
                    out_d[img].rearrange("(i p) n -> p i n", p=P), outs[:]
                )

            pending = None  # (T1, grs, img, plans) of the previous image

            for s in range(SPC):
                plans = _plan_spans(Wj[s])
                gcs = gp.tile([P, 4, N], bf16, tag="gcs")
                nc.sync.dma_start(
                    gcs[:], gc_d[s].rearrange("(i p) n -> p i n", p=P)
                )
                if shared:
                    grs = gcs
                else:
                    grs = gp.tile([P, 4, N], bf16, tag="grs")
                    nc.sync.dma_start(
                        grs[:], gr_d[s].rearrange("(i p) n -> p i n", p=P)
                    )
                for ch in range(CHANNELS):
                    img = s * CHANNELS + ch
                    xs = xsp.tile([P, 4, N], bf16, tag="xs")
                    nc.sync.dma_start(
                        xs[:], x_d[img].rearrange("(i p) n -> p i n", p=P)
                    )
                    # stage 1 (col conv): T1[w,v] = sum_h X[h,w] Gc[h,v]
                    T1 = t1p.tile([P, 4, N], bf16, tag="T1")
                    for m in range(4):
                        msl = slice(m * P, (m + 1) * P)
                        pa = psum.tile([P, N], f32, tag="ps", name="pa")
                        banded_mms(pa, xs, msl, gcs, plans)
                        if m % 2 == 0:
                            nc.vector.tensor_copy(out=T1[:, m, :], in_=pa[:])
                        else:
                            nc.scalar.copy(out=T1[:, m, :], in_=pa[:])
                    # fill the copy-latency bubble with the previous image's
                    # stage 2 (independent PE work)
                    if pending is not None:
                        emit_st2(*pending)
                    pending = (T1, grs, img, plans)

            if pending is not None:
                emit_st2(*pending)

    nc.compile()
    return nc


def _composed_kernels(kernels):
    """Exact composed spatial kernels k_t, [T+1, N, N] float64 (k_0 = delta).
    out_t = k_t (*) x as a 2D circular convolution."""
    kernels = np.asarray(kernels, dtype=np.float64)
    h = np.zeros((T_STEPS, N, N), np.float64)
    idx = (KS // 2 - np.arange(KS)) % N
    h[:, idx[:, None], idx[None, :]] = kernels
    s_step = np.fft.fft2(h)
    cum = np.empty((T_STEPS + 1, N, N), np.complex128)
    cum[0] = 1.0
    for i in range(1, T_STEPS + 1):
        cum[i] = cum[i - 1] * s_step[i - 1]
    return np.fft.ifft2(cum).real


def _rank1_factors(k_sp):
    """If every composed kernel is rank-1, return (A, B, shared) with
    k_sp[t] == outer(A[t], B[t]); shared=True when A==B for all t (symmetric
    kernels -- lets the device reuse one circulant for both stages).
    Returns None if any level is not rank-1."""
    A = np.zeros((T_STEPS + 1, N))
    B = np.zeros((T_STEPS + 1, N))
    shared = True
    for tl in range(T_STEPS + 1):
        K = k_sp[tl]
        am = np.abs(K).max()
        if am == 0.0:
            return None
        sym = False
        if np.abs(K - K.T).max() <= 1e-9 * am:
            i0 = int(np.argmax(np.diagonal(K)))
            piv = K[i0, i0]
            if piv > 1e-12 * am:
                a = K[:, i0] / np.sqrt(piv)
                b = a
                sym = True
        if not sym:
            shared = False
            i0, j0 = np.unravel_index(np.abs(K).argmax(), K.shape)
            a = K[:, j0] / K[i0, j0]
            b = K[i0, :]
        if np.abs(K - np.outer(a, b)).max() > 1e-6 * am:
            return None
        A[tl] = a
        B[tl] = b
    return A, B, shared


_SHIFT_IDX = (np.arange(N)[None, :] - np.arange(N)[:, None]) % N


def _circulant(vec):
    # M[h, v] = vec[(v - h) mod N]
    return vec[_SHIFT_IDX]


def _kernel_separable(x0, tt, A, B, shared):
    global LAST_EXEC_NS, LAST_TRACE
    from concourse import bass_utils
    import ml_dtypes

    # sort samples by descending t; core c slot j <- sorted position 8j + c,
    # so every core's slot j shares the compiled band halfwidth
    # W_j = 5 * max t. Heavy slots run first (DMA prefetch warms up behind
    # long PE bursts), cheap slots drain fast at the tail.
    order = np.argsort(-tt, kind="stable")
    Wj = tuple(min(5 * int(tt[order[NCORES * j]]), P - 1) for j in range(SPC))

    key = ("sep", Wj, shared)
    if key not in _PROGRAMS:
        _PROGRAMS[key] = _build_program_sep(Wj, shared)
    nc = _PROGRAMS[key]

    bf = ml_dtypes.bfloat16
    GC = {}
    GR = {}
    for tv in np.unique(tt):
        tv = int(tv)
        W = min(5 * tv, P - 1)
        a = A[tv].copy()
        a[W + 1 : N - W] = 0.0  # drop FFT noise outside the structural band
        GC[tv] = _circulant(a).astype(bf)
        if not shared:
            b = B[tv].copy()
            b[W + 1 : N - W] = 0.0
            GR[tv] = _circulant(b).astype(bf)
    xb = np.asarray(x0, dtype=np.float32).astype(bf)

    in_maps = []
    origs = []
    for c in range(NCORES):
        orig = order[np.arange(SPC) * NCORES + c]  # slot j -> original sample
        origs.append(orig)
        ts = tt[orig]
        im = {
            "x": np.ascontiguousarray(xb[orig].reshape(IMGS, N, N)),
            "gc": np.stack([GC[int(tv)] for tv in ts]),
        }
        if not shared:
            im["gr"] = np.stack([GR[int(tv)] for tv in ts])
        in_maps.append(im)

    res = bass_utils.run_bass_kernel_spmd(
        nc, in_maps, core_ids=list(range(NCORES)), trace=TRACE
    )
    LAST_EXEC_NS = res.exec_time_ns
    if res.instructions_and_trace is not None:
        LAST_TRACE = res.instructions_and_trace[1]
    out = np.empty((BATCH, CHANNELS, N, N), np.float32)
    for c in range(NCORES):
        out[origs[c]] = (
            np.asarray(res.results[c]["out"])
            .astype(np.float32)
            .reshape(SPC, CHANNELS, N, N)
        )
    return out


# --------------------------------------------------------------------------
# Spectral fallback (general, possibly non-separable kernels)
# --------------------------------------------------------------------------

def _build_program_spec():
    import concourse.mybir as mybir
    import concourse.tile as tile
    from concourse import bacc

    f32 = mybir.dt.float32
    f32r = mybir.dt.float32r
    mmdt = f32r if USE_F32R else f32

    nc = bacc.Bacc(
        "TRN2", target_bir_lowering=False, debug=False, num_devices=NCORES
    )
    x_d = nc.dram_tensor("x", [IMGS, N, N], mmdt, kind="ExternalInput").ap()
    fkr_d = nc.dram_tensor("fkr", [SPC, N, KP], f32, kind="ExternalInput").ap()
    fki_d = nc.dram_tensor("fki", [SPC, N, KP], f32, kind="ExternalInput").ap()
    mat_names = ["cmat", "smat", "snmat", "nscmat", "cmsmat"]
    mat_d = {
        nm: nc.dram_tensor(nm, [N, N], mmdt, kind="ExternalInput").ap()
        for nm in mat_names
    }
    out_d = nc.dram_tensor("out", [IMGS, N, N], f32, kind="ExternalOutput").ap()

    with tile.TileContext(nc) as tc:
        with (
            tc.tile_pool(name="mats", bufs=1) as mats,
            tc.tile_pool(name="xsp", bufs=2) as xsp,
            tc.tile_pool(name="outp", bufs=2) as outp,
            tc.tile_pool(name="fkp", bufs=2) as fkp,
            tc.tile_pool(name="apool", bufs=2) as apool,
            tc.tile_pool(name="hpool", bufs=2) as hpool,
            tc.tile_pool(name="zpool", bufs=2) as zpool,
            tc.tile_pool(name="vpool", bufs=2) as vpool,
            tc.tile_pool(name="pw", bufs=3) as pw,
            tc.tile_pool(name="psum", bufs=8, space="PSUM") as psum,
        ):
            # resident DFT matrices, [p, tile, n] layout
            M = {}
            for nm in mat_names:
                mt = mats.tile([P, 4, N], mmdt, name=nm + "_s")
                nc.sync.dma_start(mt[:], mat_d[nm].rearrange("(i p) n -> p i n", p=P))
                M[nm] = mt
            Cs, Ss, Sns = M["cmat"], M["smat"], M["snmat"]
            nSCs, CmSs = M["nscmat"], M["cmsmat"]
            # C/-S rows 130..257, partition-aligned (stage-4 k tail tile)
            Ck2s = mats.tile([P, N], mmdt, name="ck2_s")
            Snk2s = mats.tile([P, N], mmdt, name="snk2_s")
            nc.sync.dma_start(Ck2s[:], mat_d["cmat"][130:258, :])
            nc.sync.dma_start(Snk2s[:], mat_d["snmat"][130:258, :])

            def emit_st4(Vr, Vi, img):
                # ---- Stage 4 (matrix-stationary, natural orientation):
                # out[y,x] = sum_k C[k,y] Vr[k,x] + (-S)[k,y] Vi[k,x]
                # k tiles: 0:128, 128:256, 130:258 (FK pre-halved on the
                # double-counted 130..255 range)
                outs = outp.tile([P, 4, N], f32, tag="outs")
                for ym in range(4):
                    ysl = slice(ym * P, (ym + 1) * P)
                    po = psum.tile([P, N], f32, tag="ps", name="po")
                    nc.tensor.matmul(
                        po[:], Cs[:, 0, ysl], Vr[:, 0, :],
                        start=True, stop=False,
                    )
                    nc.tensor.matmul(
                        po[:], Cs[:, 1, ysl], Vr[:, 1, :],
                        start=False, stop=False,
                    )
                    nc.tensor.matmul(
                        po[:], Ck2s[:, ysl], Vr[:, 2, :],
                        start=False, stop=False,
                    )
                    nc.tensor.matmul(
                        po[:], Sns[:, 0, ysl], Vi[:, 0, :],
                        start=False, stop=False,
                    )
                    nc.tensor.matmul(
                        po[:], Sns[:, 1, ysl], Vi[:, 1, :],
                        start=False, stop=False,
                    )
                    nc.tensor.matmul(
                        po[:], Snk2s[:, ysl], Vi[:, 2, :],
                        start=False, stop=True,
                    )
                    nc.any.tensor_copy(out=outs[:, ym, :], in_=po[:])
                nc.sync.dma_start(
                    out_d[img].rearrange("(i p) n -> p i n", p=P), outs[:]
                )

            pending = None  # (Vr, Vi, img) of the previous image

            for s in range(SPC):
                # per-sample spectrum, transposed [l, k] layout, k cols 0..257
                fktr = fkp.tile([P, 4, KP], f32, tag="fktr")
                fkti = fkp.tile([P, 4, KP], f32, tag="fkti")
                nc.sync.dma_start(
                    fktr[:], fkr_d[s].rearrange("(i p) n -> p i n", p=P)
                )
                nc.sync.dma_start(
                    fkti[:], fki_d[s].rearrange("(i p) n -> p i n", p=P)
                )

                for ch in range(CHANNELS):
                    img = s * CHANNELS + ch
                    xs = xsp.tile([P, 4, N], mmdt, tag="xs")
                    nc.sync.dma_start(
                        xs[:], x_d[img].rearrange("(i p) n -> p i n", p=P)
                    )

                    # ---- Stage 1 (data-stationary, weight-paired):
                    # A_r = x^T C[:, :258] ; A_i = x^T (-S)[:, :258]
                    Ar = apool.tile([P, 4, KP], mmdt, tag="Ar")
                    Ai = apool.tile([P, 4, KP], mmdt, tag="Ai")
                    Apb = apool.tile([P, 4, KP], mmdt, tag="Apb")
                    for m in range(4):
                        msl = slice(m * P, (m + 1) * P)
                        pa = psum.tile([P, N], f32, tag="ps", name="pa")[:, :KP]
                        pb = psum.tile([P, N], f32, tag="ps", name="pb")[:, :KP]
                        for kk in range(4):
                            nc.tensor.matmul(
                                pa[:], xs[:, kk, msl], Cs[:, kk, 0:KP],
                                start=(kk == 0), stop=(kk == 3),
                            )
                            nc.tensor.matmul(
                                pb[:], xs[:, kk, msl], Sns[:, kk, 0:KP],
                                start=(kk == 0), stop=(kk == 3),
                            )
                        nc.scalar.copy(out=Ar[:, m, :], in_=pa[:])
                        nc.scalar.copy(out=Ai[:, m, :], in_=pb[:])
                        nc.gpsimd.tensor_tensor(
                            Apb[:, m, :], Ar[:, m, :], Ai[:, m, :],
                            mybir.AluOpType.add,
                        )

                    # ---- Stage 2 (matrix-stationary, Gauss):
                    # m1 = C.(Ar+Ai) ; m2 = (-S-C).Ar ; m3 = (C-S).Ai
                    # Htr = m1 - m3 ; Hti = m1 + m2 ; pointwise per l-tile
                    Ztr = zpool.tile([P, 4, KP], mmdt, tag="Ztr")
                    Zti = zpool.tile([P, 4, KP], mmdt, tag="Zti")
                    for lm in range(4):
                        lsl = slice(lm * P, (lm + 1) * P)
                        m1 = psum.tile([P, N], f32, tag="ps", name="m1")[:, :KP]
                        m2 = psum.tile([P, N], f32, tag="ps", name="m2")[:, :KP]
                        m3 = psum.tile([P, N], f32, tag="ps", name="m3")[:, :KP]
                        for kk in range(4):
                            nc.tensor.matmul(
                                m1[:], Cs[:, kk, lsl], Apb[:, kk, :],
                                start=(kk == 0), stop=(kk == 3),
                            )
                        for kk in range(4):
                            nc.tensor.matmul(
                                m2[:], nSCs[:, kk, lsl], Ar[:, kk, :],
                                start=(kk == 0), stop=(kk == 3),
                            )
                        for kk in range(4):
                            nc.tensor.matmul(
                                m3[:], CmSs[:, kk, lsl], Ai[:, kk, :],
                                start=(kk == 0), stop=(kk == 3),
                            )
                        # Htr = m1 - m3 ; Hti = m1 + m2
                        m1s = pw.tile([P, KP], f32, tag="m1s")
                        hrs = pw.tile([P, KP], f32, tag="hrs")
                        his = pw.tile([P, KP], f32, tag="his")
                        nc.scalar.copy(out=m1s[:], in_=m1[:])
                        nc.vector.tensor_sub(out=hrs[:], in0=m1s[:], in1=m3[:])
                        nc.vector.tensor_add(out=his[:], in0=m1s[:], in1=m2[:])
                        # pointwise: Ztr = hr o fr - hi o fi
                        #            Zti = hr o fi + hi o fr
                        fr = fktr[:, lm, :]
                        fi = fkti[:, lm, :]
                        tt = pw.tile([P, KP], f32, tag="tt")
                        tu = pw.tile([P, KP], f32, tag="tu")
                        ztr = Ztr[:, lm, :]
                        zti = Zti[:, lm, :]
                        nc.vector.tensor_mul(out=ztr, in0=hrs[:], in1=fr)
                        nc.gpsimd.tensor_tensor(
                            tt[:], his[:], fi, mybir.AluOpType.mult
                        )
                        nc.vector.tensor_sub(out=ztr, in0=ztr, in1=tt[:])
                        nc.gpsimd.tensor_tensor(
                            tu[:], hrs[:], fi, mybir.AluOpType.mult
                        )
                        nc.vector.tensor_mul(out=zti, in0=his[:], in1=fr)
                        nc.vector.tensor_add(out=zti, in0=zti, in1=tu[:])

                    # fill the pointwise-latency bubble with the previous
                    # image's stage 4 (independent PE work)
                    if pending is not None:
                        emit_st4(*pending)
                        pending = None

                    # ---- Stage 3 (data-stationary, weight-paired direct):
                    # V_r = Ztr^T C + Zti^T (-S) ; V_i = Ztr^T S + Zti^T C
                    # k M-tiles: 0:128, 128:256, 130:258
                    Vr = vpool.tile([P, 3, N], mmdt, tag="Vr")
                    Vi = vpool.tile([P, 3, N], mmdt, tag="Vi")
                    for km in range(3):
                        koff = (0, 128, 130)[km]
                        ksl = slice(koff, koff + P)
                        nvr = psum.tile([P, N], f32, tag="ps", name="nvr")
                        nvi = psum.tile([P, N], f32, tag="ps", name="nvi")
                        for lt in range(4):
                            nc.tensor.matmul(
                                nvr[:], Ztr[:, lt, ksl], Cs[:, lt, :],
                                start=(lt == 0), stop=False,
                            )
                            nc.tensor.matmul(
                                nvi[:], Ztr[:, lt, ksl], Ss[:, lt, :],
                                start=(lt == 0), stop=False,
                            )
                        for lt in range(4):
                            nc.tensor.matmul(
                                nvr[:], Zti[:, lt, ksl], Sns[:, lt, :],
                                start=False, stop=(lt == 3),
                            )
                            nc.tensor.matmul(
                                nvi[:], Zti[:, lt, ksl], Cs[:, lt, :],
                                start=False, stop=(lt == 3),
                            )
                        nc.any.tensor_copy(out=Vr[:, km, :], in_=nvr[:])
                        nc.any.tensor_copy(out=Vi[:, km, :], in_=nvi[:])

                    pending = (Vr, Vi, img)


            if pending is not None:
                emit_st4(*pending)

    nc.compile()
    return nc


def _host_spectra(kernels):
    """Compose step kernels into 21 cumulative half-spectra, transposed to
    [l, k] layout with Hermitian weights, 1/N^2, and the stage-4
    double-count halving folded in. Returns (FKtr, FKti) f32 [21, 512, KP]."""
    kernels = np.asarray(kernels, dtype=np.float64)
    h = np.zeros((T_STEPS, N, N), np.float64)
    idx = (KS // 2 - np.arange(KS)) % N
    h[:, idx[:, None], idx[None, :]] = kernels
    s_step = np.fft.fft2(h)
    cum = np.ones((T_STEPS + 1, N, N), np.complex128)
    for i in range(1, T_STEPS + 1):
        cum[i] = cum[i - 1] * s_step[i - 1]
    w = np.zeros(KP)
    w[: N // 2 + 1] = 2.0
    w[0] = w[N // 2] = 1.0
    fkt = (cum[:, :KP, :] * w[None, :, None] / float(N * N)).transpose(0, 2, 1)
    half = np.ones(KP)
    half[130:256] = 0.5  # k rows 130..255 appear in both stage-4 k-tiles
    fkt = fkt * half[None, None, :]
    return (
        np.ascontiguousarray(fkt.real.astype(np.float32)),
        np.ascontiguousarray(fkt.imag.astype(np.float32)),
    )


def _dft_mats():
    j = np.arange(N)
    ang = 2.0 * np.pi * (np.outer(j, j) % N) / N
    cm = np.cos(ang).astype(np.float32)
    sm = np.sin(ang).astype(np.float32)
    return {
        "cmat": cm,
        "smat": sm,
        "snmat": np.ascontiguousarray(-sm),
        "nscmat": np.ascontiguousarray(-sm - cm),
        "cmsmat": np.ascontiguousarray(cm - sm),
    }


def _kernel_spectral(x0, tt, kernels):
    global LAST_EXEC_NS, LAST_TRACE
    from concourse import bass_utils

    x0 = np.ascontiguousarray(np.asarray(x0), dtype=np.float32)
    fktr_all, fkti_all = _host_spectra(kernels)
    mats = _dft_mats()

    if "spec" not in _PROGRAMS:
        _PROGRAMS["spec"] = _build_program_spec()
    nc = _PROGRAMS["spec"]

    in_maps = []
    for c in range(NCORES):
        sl = slice(c * SPC, (c + 1) * SPC)
        ts = tt[sl]
        im = {
            "x": np.ascontiguousarray(x0[sl].reshape(IMGS, N, N)),
            "fkr": np.ascontiguousarray(fktr_all[ts]),
            "fki": np.ascontiguousarray(fkti_all[ts]),
        }
        im.update(mats)
        in_maps.append(im)

    res = bass_utils.run_bass_kernel_spmd(
        nc, in_maps, core_ids=list(range(NCORES)), trace=TRACE
    )
    LAST_EXEC_NS = res.exec_time_ns
    if res.instructions_and_trace is not None:
        LAST_TRACE = res.instructions_and_trace[1]
    out = np.empty((BATCH, CHANNELS, N, N), np.float32)
    for c in range(NCORES):
        out[c * SPC : (c + 1) * SPC] = res.results[c]["out"].reshape(
            SPC, CHANNELS, N, N
        )
    return out


def kernel(x0, t, kernels):
    tt = np.asarray(t).astype(np.int64)
    k_sp = _composed_kernels(kernels)
    fac = _rank1_factors(k_sp)
    if fac is not None:
        return _kernel_separable(x0, tt, fac[0], fac[1], fac[2])
    return _kernel_spectral(x0, tt, kernels)


# revision 16
# speedup vs baseline: 6.7653x; 1.0379x over previous
"""BlurDegradation kernel for 8x TRN2 NeuronCores.

Math: t[b] successive 11x11 depthwise *circular* convolutions compose into a
single circular convolution with kernel k_t = h_1 (*) h_2 (*) ... (*) h_t
(circular 2D convolution of the per-step impulse responses). The host
composes k_t exactly with FFTs.

Fast path (separable): when every composed k_t is numerically rank-1
(k_t = outer(a_t, b_t) -- always true for the constant-Gaussian blur routine,
whose steps are separable), the 2D blur factors into a column circular conv
by a_t followed by a row circular conv by b_t. Each is one dense circulant
matmul, so an image costs just two 512x512x512 matmul chains on the PE array
(268M MACs vs 941M for the spectral pipeline). Both stages are
data-stationary so outputs chain orientation [h,w] -> [w,v] -> [v,x] with no
transposes:
  stage 1: T1[w,v] = sum_h X[h,w] * Gc[h,v],  Gc[h,v] = a[(v-h) mod N]
  stage 2: Z [v,x] = sum_w T1[w,v] * Gr[w,x], Gr[w,x] = b[(x-w) mod N]
Matmuls run in bf16 (1 cycle/row + fast weight load) with fp32 PSUM
accumulation; stage-2 of image i is emitted after stage-1 of image i+1 so the
PSUM->SBUF cast copies never stall the PE.

Fallback path (general kernels): the original spectral pipeline -- per-sample
cumulative half-spectra multiply between dense DFT matmuls.

Sharding: pure data parallel, 8 samples per core, no cross-core comms.
"""

import numpy as np

N = 512
P = 128
T_STEPS = 20
KS = 11
KP = 258            # padded half-spectrum k-dim (even for fp32r)
NCORES = 8
BATCH = 64
CHANNELS = 3
SPC = BATCH // NCORES  # samples per core
IMGS = SPC * CHANNELS  # images per core

USE_F32R = True

_PROGRAMS = {}
TRACE = False
LAST_EXEC_NS = None
LAST_TRACE = None


# --------------------------------------------------------------------------
# Separable fast path
# --------------------------------------------------------------------------

# The composed blur kernel after t steps has support <= +-5t, so the
# circulant G[a, b] = vec[(b-a) mod 512] is band-limited: the [128 x 512]
# tile for contraction block kk has nonzero columns only in the contiguous
# (mod 512) span [128kk - W, 128(kk+1) + W). Samples are globally sorted by
# t so that slot j on every core shares a compiled band halfwidth W_j.
# PSUM's per-element has_written bit makes partial-column accumulation work:
# the group's first matmul (start=True) clears the whole bank's bits, later
# matmuls overwrite where unwritten and accumulate where written.
def _plan_spans(W):
    """Per contraction tile kk: fused contiguous column ranges of its band
    (split only at the mod-512 wrap). Returns [(kk, c0, c1), ...]."""
    plans = []
    for kk in range(4):
        c0 = (P * kk - W) % N
        ln = P + 2 * W
        if ln >= N:
            plans.append((kk, 0, N))
        elif c0 + ln <= N:
            plans.append((kk, c0, c0 + ln))
        else:
            plans.append((kk, c0, N))
            plans.append((kk, 0, c0 + ln - N))
    return plans


def _plan_spans_1s(W):
    """One-sided band (filter pre-rolled to support [0, 2W], input pre-rolled
    by -W): tile kk covers cols [128kk, 128(kk+1)+2W) mod 512. Returns
    [(kk, c0, c1, q0), ...] with q0 the packed-G column offset."""
    plans = []
    for kk in range(4):
        base = P * kk
        ln = min(P + 2 * W, N)
        if base + ln <= N:
            plans.append((kk, base, base + ln, 0))
        else:
            plans.append((kk, base, N, 0))
            plans.append((kk, 0, base + ln - N, N - base))
    return plans


def _build_program_sep_shared(Wj):
    """Shared-G (symmetric factors) program: per-slot band-packed circulant
    G tile M[p, q] = a'[(q - p) mod 512] (identical for all 4 contraction
    tiles), kept resident in SBUF for all 8 slots."""
    import concourse.mybir as mybir
    import concourse.tile as tile
    from concourse import bacc

    f32 = mybir.dt.float32
    bf16 = mybir.dt.bfloat16

    nc = bacc.Bacc(
        "TRN2", target_bir_lowering=False, debug=False, num_devices=NCORES
    )
    x_d = nc.dram_tensor("x", [IMGS, N, N], bf16, kind="ExternalInput").ap()
    cws = [min(P + 2 * W, N) for W in Wj]
    g_d = [
        nc.dram_tensor(f"g{j}", [P, cws[j]], bf16, kind="ExternalInput").ap()
        for j in range(SPC)
    ]
    out_d = nc.dram_tensor("out", [IMGS, N, N], bf16, kind="ExternalOutput").ap()

    with tile.TileContext(nc) as tc:
        with (
            tc.tile_pool(name="gp", bufs=1) as gp,
            tc.tile_pool(name="xsp", bufs=4) as xsp,
            tc.tile_pool(name="t1p", bufs=3) as t1p,
            tc.tile_pool(name="outp", bufs=3) as outp,
            tc.tile_pool(name="psum", bufs=8, space="PSUM") as psum,
        ):
            # all per-slot G bands stay resident (~4KB/partition total)
            gt = []
            for j in range(SPC):
                g = gp.tile([P, cws[j]], bf16, name=f"g{j}_s")
                nc.sync.dma_start(g[:], g_d[j])
                gt.append(g)

            def banded_mms(pa, data, msl, g, plans):
                last = len(plans) - 1
                for i, (kk, c0, c1, q0) in enumerate(plans):
                    nc.tensor.matmul(
                        pa[:, c0:c1], data[:, kk, msl], g[:, q0 : q0 + c1 - c0],
                        start=(i == 0), stop=(i == last),
                    )

            def emit_st2(T1, g, img, plans):
                # stage 2 (row conv): Z[v,x] = sum_w T1[w,v] G[w,x]
                outs = outp.tile([P, 4, N], bf16, tag="outs")
                for m in range(4):
                    msl = slice(m * P, (m + 1) * P)
                    pb = psum.tile([P, N], f32, tag="ps", name="pb")
                    banded_mms(pb, T1, msl, g, plans)
                    if m % 2 == 0:
                        nc.vector.tensor_copy(out=outs[:, m, :], in_=pb[:])
                    else:
                        nc.scalar.copy(out=outs[:, m, :], in_=pb[:])
                nc.sync.dma_start(
                    out_d[img].rearrange("(i p) n -> p i n", p=P), outs[:]
                )

            pending = None  # (T1, g, img, plans) of the previous image

            for s in range(SPC):
                plans = _plan_spans_1s(Wj[s])
                for ch in range(CHANNELS):
                    img = s * CHANNELS + ch
                    xs = xsp.tile([P, 4, N], bf16, tag="xs")
                    nc.gpsimd.dma_start(
                        xs[:], x_d[img].rearrange("(i p) n -> p i n", p=P)
                    )
                    # stage 1 (col conv): T1[w,v] = sum_h X[h,w] G[h,v]
                    T1 = t1p.tile([P, 4, N], bf16, tag="T1")
                    for m in range(4):
                        msl = slice(m * P, (m + 1) * P)
                        pa = psum.tile([P, N], f32, tag="ps", name="pa")
                        banded_mms(pa, xs, msl, gt[s], plans)
                        if m % 2 == 0:
                            nc.vector.tensor_copy(out=T1[:, m, :], in_=pa[:])
                        else:
                            nc.scalar.copy(out=T1[:, m, :], in_=pa[:])
                    if pending is not None:
                        emit_st2(*pending)
                    pending = (T1, gt[s], img, plans)

            if pending is not None:
                emit_st2(*pending)

    nc.compile()
    return nc


def _build_program_sep(Wj, shared):
    import concourse.mybir as mybir
    import concourse.tile as tile
    from concourse import bacc

    f32 = mybir.dt.float32
    bf16 = mybir.dt.bfloat16

    nc = bacc.Bacc(
        "TRN2", target_bir_lowering=False, debug=False, num_devices=NCORES
    )
    x_d = nc.dram_tensor("x", [IMGS, N, N], bf16, kind="ExternalInput").ap()
    gc_d = nc.dram_tensor("gc", [SPC, N, N], bf16, kind="ExternalInput").ap()
    gr_d = None
    if not shared:
        gr_d = nc.dram_tensor("gr", [SPC, N, N], bf16, kind="ExternalInput").ap()
    out_d = nc.dram_tensor("out", [IMGS, N, N], bf16, kind="ExternalOutput").ap()

    with tile.TileContext(nc) as tc:
        with (
            tc.tile_pool(name="gp", bufs=4 if shared else 3) as gp,
            tc.tile_pool(name="xsp", bufs=4) as xsp,
            tc.tile_pool(name="t1p", bufs=3) as t1p,
            tc.tile_pool(name="outp", bufs=3) as outp,
            tc.tile_pool(name="psum", bufs=8, space="PSUM") as psum,
        ):
            def banded_mms(pa, data, msl, g, plans):
                # fused band spans; only the group's first matmul starts the
                # bank (whole-bank has_written clear), the rest accumulate
                # per-element. Consecutive same-kk matmuls share LDWEIGHTS.
                last = len(plans) - 1
                for i, (kk, c0, c1) in enumerate(plans):
                    nc.tensor.matmul(
                        pa[:, c0:c1], data[:, kk, msl], g[:, kk, c0:c1],
                        start=(i == 0), stop=(i == last),
                    )

            def emit_st2(T1, grs, img, plans):
                # stage 2 (row conv): Z[v,x] = sum_w T1[w,v] Gr[w,x]
                outs = outp.tile([P, 4, N], bf16, tag="outs")
                for m in range(4):
                    msl = slice(m * P, (m + 1) * P)
                    pb = psum.tile([P, N], f32, tag="ps", name="pb")
                    banded_mms(pb, T1, msl, grs, plans)
                    if m % 2 == 0:
                        nc.vector.tensor_copy(out=outs[:, m, :], in_=pb[:])
                    else:
                        nc.scalar.copy(out=outs[:, m, :], in_=pb[:])
                nc.sync.dma_start(
                    out_d[img].rearrange("(i p) n -> p i n", p=P), outs[:]
                )

            pending = None  # (T1, grs, img, plans) of the previous image

            for s in range(SPC):
                plans = _plan_spans(Wj[s])
                gcs = gp.tile([P, 4, N], bf16, tag="gcs")
                nc.sync.dma_start(
                    gcs[:], gc_d[s].rearrange("(i p) n -> p i n", p=P)
                )
                if shared:
                    grs = gcs
                else:
                    grs = gp.tile([P, 4, N], bf16, tag="grs")
                    nc.sync.dma_start(
                        grs[:], gr_d[s].rearrange("(i p) n -> p i n", p=P)
                    )
                for ch in range(CHANNELS):
                    img = s * CHANNELS + ch
                    xs = xsp.tile([P, 4, N], bf16, tag="xs")
                    nc.sync.dma_start(
                        xs[:], x_d[img].rearrange("(i p) n -> p i n", p=P)
                    )
                    # stage 1 (col conv): T1[w,v] = sum_h X[h,w] Gc[h,v]
                    T1 = t1p.tile([P, 4, N], bf16, tag="T1")
                    for m in range(4):
                        msl = slice(m * P, (m + 1) * P)
                        pa = psum.tile([P, N], f32, tag="ps", name="pa")
                        banded_mms(pa, xs, msl, gcs, plans)
                        if m % 2 == 0:
                            nc.vector.tensor_copy(out=T1[:, m, :], in_=pa[:])
                        else:
                            nc.scalar.copy(out=T1[:, m, :], in_=pa[:])
                    # fill the copy-latency bubble with the previous image's
                    # stage 2 (independent PE work)
                    if pending is not None:
                        emit_st2(*pending)
                    pending = (T1, grs, img, plans)

            if pending is not None:
                emit_st2(*pending)

    nc.compile()
    return nc


def _composed_kernels(kernels):
    """Exact composed spatial kernels k_t, [T+1, N, N] float64 (k_0 = delta).
    out_t = k_t (*) x as a 2D circular convolution."""
    kernels = np.asarray(kernels, dtype=np.float64)
    h = np.zeros((T_STEPS, N, N), np.float64)
    idx = (KS // 2 - np.arange(KS)) % N
    h[:, idx[:, None], idx[None, :]] = kernels
    s_step = np.fft.fft2(h)
    cum = np.empty((T_STEPS + 1, N, N), np.complex128)
    cum[0] = 1.0
    for i in range(1, T_STEPS + 1):
        cum[i] = cum[i - 1] * s_step[i - 1]
    return np.fft.ifft2(cum).real


def _rank1_factors(k_sp):
    """If every composed kernel is rank-1, return (A, B, shared) with
    k_sp[t] == outer(A[t], B[t]); shared=True when A==B for all t (symmetric
    kernels -- lets the device reuse one circulant for both stages).
    Returns None if any level is not rank-1."""
    A = np.zeros((T_STEPS + 1, N))
    B = np.zeros((T_STEPS + 1, N))
    shared = True
    for tl in range(T_STEPS + 1):
        K = k_sp[tl]
        am = np.abs(K).max()
        if am == 0.0:
            return None
        sym = False
        if np.abs(K - K.T).max() <= 1e-9 * am:
            i0 = int(np.argmax(np.diagonal(K)))
            piv = K[i0, i0]
            if piv > 1e-12 * am:
                a = K[:, i0] / np.sqrt(piv)
                b = a
                sym = True
        if not sym:
            shared = False
            i0, j0 = np.unravel_index(np.abs(K).argmax(), K.shape)
            a = K[:, j0] / K[i0, j0]
            b = K[i0, :]
        if np.abs(K - np.outer(a, b)).max() > 1e-6 * am:
            return None
        A[tl] = a
        B[tl] = b
    return A, B, shared


_SHIFT_IDX = (np.arange(N)[None, :] - np.arange(N)[:, None]) % N


def _circulant(vec):
    # M[h, v] = vec[(v - h) mod N]
    return vec[_SHIFT_IDX]


def _kernel_separable(x0, tt, A, B, shared):
    global LAST_EXEC_NS, LAST_TRACE
    from concourse import bass_utils
    import ml_dtypes

    # sort samples by descending t; core c slot j <- sorted position 8j + c,
    # so every core's slot j shares the compiled band halfwidth
    # W_j = 5 * max t. Heavy slots run first (DMA prefetch warms up behind
    # long PE bursts), cheap slots drain fast at the tail.
    order = np.argsort(-tt, kind="stable")
    Wj = tuple(min(5 * int(tt[order[NCORES * j]]), P - 1) for j in range(SPC))

    key = ("sep", Wj, shared)
    if key not in _PROGRAMS:
        _PROGRAMS[key] = (
            _build_program_sep_shared(Wj) if shared
            else _build_program_sep(Wj, shared)
        )
    nc = _PROGRAMS[key]

    bf = ml_dtypes.bfloat16
    xb = np.asarray(x0, dtype=np.float32).astype(bf)

    in_maps = []
    origs = []
    if shared:
        # band-packed G: M[p, q] = a'[(q - p) mod 512], a' = roll(a, W_slot)
        GP = {}
        for j in range(SPC):
            W = Wj[j]
            cw = min(P + 2 * W, N)
            GP[W] = (np.arange(cw)[None, :] - np.arange(P)[:, None]) % N
        for c in range(NCORES):
            orig = order[np.arange(SPC) * NCORES + c]
            origs.append(orig)
            im = {}
            xcore = np.empty((SPC, CHANNELS, N, N), bf)
            for j in range(SPC):
                W = Wj[j]
                tv = int(tt[orig[j]])
                Ws = min(5 * tv, P - 1)
                a = A[tv].copy()
                a[Ws + 1 : N - Ws] = 0.0
                ap_ = np.roll(a, W)
                im[f"g{j}"] = ap_[GP[W]].astype(bf)
                xcore[j] = np.roll(xb[orig[j]], (-W, -W), axis=(-2, -1))
            im["x"] = np.ascontiguousarray(xcore.reshape(IMGS, N, N))
            in_maps.append(im)
    else:
        GC = {}
        GR = {}
        for tv in np.unique(tt):
            tv = int(tv)
            W = min(5 * tv, P - 1)
            a = A[tv].copy()
            a[W + 1 : N - W] = 0.0  # drop FFT noise outside the band
            GC[tv] = _circulant(a).astype(bf)
            b = B[tv].copy()
            b[W + 1 : N - W] = 0.0
            GR[tv] = _circulant(b).astype(bf)
        for c in range(NCORES):
            orig = order[np.arange(SPC) * NCORES + c]
            origs.append(orig)
            ts = tt[orig]
            in_maps.append({
                "x": np.ascontiguousarray(xb[orig].reshape(IMGS, N, N)),
                "gc": np.stack([GC[int(tv)] for tv in ts]),
                "gr": np.stack([GR[int(tv)] for tv in ts]),
            })

    res = bass_utils.run_bass_kernel_spmd(
        nc, in_maps, core_ids=list(range(NCORES)), trace=TRACE
    )
    LAST_EXEC_NS = res.exec_time_ns
    if res.instructions_and_trace is not None:
        LAST_TRACE = res.instructions_and_trace[1]
    out = np.empty((BATCH, CHANNELS, N, N), np.float32)
    for c in range(NCORES):
        out[origs[c]] = (
            np.asarray(res.results[c]["out"])
            .astype(np.float32)
            .reshape(SPC, CHANNELS, N, N)
        )
    return out


# --------------------------------------------------------------------------
# Spectral fallback (general, possibly non-separable kernels)
# --------------------------------------------------------------------------

def _build_program_spec():
    import concourse.mybir as mybir
    import concourse.tile as tile
    from concourse import bacc

    f32 = mybir.dt.float32
    f32r = mybir.dt.float32r
    mmdt = f32r if USE_F32R else f32

    nc = bacc.Bacc(
        "TRN2", target_bir_lowering=False, debug=False, num_devices=NCORES
    )
    x_d = nc.dram_tensor("x", [IMGS, N, N], mmdt, kind="ExternalInput").ap()
    fkr_d = nc.dram_tensor("fkr", [SPC, N, KP], f32, kind="ExternalInput").ap()
    fki_d = nc.dram_tensor("fki", [SPC, N, KP], f32, kind="ExternalInput").ap()
    mat_names = ["cmat", "smat", "snmat", "nscmat", "cmsmat"]
    mat_d = {
        nm: nc.dram_tensor(nm, [N, N], mmdt, kind="ExternalInput").ap()
        for nm in mat_names
    }
    out_d = nc.dram_tensor("out", [IMGS, N, N], f32, kind="ExternalOutput").ap()

    with tile.TileContext(nc) as tc:
        with (
            tc.tile_pool(name="mats", bufs=1) as mats,
            tc.tile_pool(name="xsp", bufs=2) as xsp,
            tc.tile_pool(name="outp", bufs=2) as outp,
            tc.tile_pool(name="fkp", bufs=2) as fkp,
            tc.tile_pool(name="apool", bufs=2) as apool,
            tc.tile_pool(name="hpool", bufs=2) as hpool,
            tc.tile_pool(name="zpool", bufs=2) as zpool,
            tc.tile_pool(name="vpool", bufs=2) as vpool,
            tc.tile_pool(name="pw", bufs=3) as pw,
            tc.tile_pool(name="psum", bufs=8, space="PSUM") as psum,
        ):
            # resident DFT matrices, [p, tile, n] layout
            M = {}
            for nm in mat_names:
                mt = mats.tile([P, 4, N], mmdt, name=nm + "_s")
                nc.sync.dma_start(mt[:], mat_d[nm].rearrange("(i p) n -> p i n", p=P))
                M[nm] = mt
            Cs, Ss, Sns = M["cmat"], M["smat"], M["snmat"]
            nSCs, CmSs = M["nscmat"], M["cmsmat"]
            # C/-S rows 130..257, partition-aligned (stage-4 k tail tile)
            Ck2s = mats.tile([P, N], mmdt, name="ck2_s")
            Snk2s = mats.tile([P, N], mmdt, name="snk2_s")
            nc.sync.dma_start(Ck2s[:], mat_d["cmat"][130:258, :])
            nc.sync.dma_start(Snk2s[:], mat_d["snmat"][130:258, :])

            def emit_st4(Vr, Vi, img):
                # ---- Stage 4 (matrix-stationary, natural orientation):
                # out[y,x] = sum_k C[k,y] Vr[k,x] + (-S)[k,y] Vi[k,x]
                # k tiles: 0:128, 128:256, 130:258 (FK pre-halved on the
                # double-counted 130..255 range)
                outs = outp.tile([P, 4, N], f32, tag="outs")
                for ym in range(4):
                    ysl = slice(ym * P, (ym + 1) * P)
                    po = psum.tile([P, N], f32, tag="ps", name="po")
                    nc.tensor.matmul(
                        po[:], Cs[:, 0, ysl], Vr[:, 0, :],
                        start=True, stop=False,
                    )
                    nc.tensor.matmul(
                        po[:], Cs[:, 1, ysl], Vr[:, 1, :],
                        start=False, stop=False,
                    )
                    nc.tensor.matmul(
                        po[:], Ck2s[:, ysl], Vr[:, 2, :],
                        start=False, stop=False,
                    )
                    nc.tensor.matmul(
                        po[:], Sns[:, 0, ysl], Vi[:, 0, :],
                        start=False, stop=False,
                    )
                    nc.tensor.matmul(
                        po[:], Sns[:, 1, ysl], Vi[:, 1, :],
                        start=False, stop=False,
                    )
                    nc.tensor.matmul(
                        po[:], Snk2s[:, ysl], Vi[:, 2, :],
                        start=False, stop=True,
                    )
                    nc.any.tensor_copy(out=outs[:, ym, :], in_=po[:])
                nc.sync.dma_start(
                    out_d[img].rearrange("(i p) n -> p i n", p=P), outs[:]
                )

            pending = None  # (Vr, Vi, img) of the previous image

            for s in range(SPC):
                # per-sample spectrum, transposed [l, k] layout, k cols 0..257
                fktr = fkp.tile([P, 4, KP], f32, tag="fktr")
                fkti = fkp.tile([P, 4, KP], f32, tag="fkti")
                nc.sync.dma_start(
                    fktr[:], fkr_d[s].rearrange("(i p) n -> p i n", p=P)
                )
                nc.sync.dma_start(
                    fkti[:], fki_d[s].rearrange("(i p) n -> p i n", p=P)
                )

                for ch in range(CHANNELS):
                    img = s * CHANNELS + ch
                    xs = xsp.tile([P, 4, N], mmdt, tag="xs")
                    nc.sync.dma_start(
                        xs[:], x_d[img].rearrange("(i p) n -> p i n", p=P)
                    )

                    # ---- Stage 1 (data-stationary, weight-paired):
                    # A_r = x^T C[:, :258] ; A_i = x^T (-S)[:, :258]
                    Ar = apool.tile([P, 4, KP], mmdt, tag="Ar")
                    Ai = apool.tile([P, 4, KP], mmdt, tag="Ai")
                    Apb = apool.tile([P, 4, KP], mmdt, tag="Apb")
                    for m in range(4):
                        msl = slice(m * P, (m + 1) * P)
                        pa = psum.tile([P, N], f32, tag="ps", name="pa")[:, :KP]
                        pb = psum.tile([P, N], f32, tag="ps", name="pb")[:, :KP]
                        for kk in range(4):
                            nc.tensor.matmul(
                                pa[:], xs[:, kk, msl], Cs[:, kk, 0:KP],
                                start=(kk == 0), stop=(kk == 3),
                            )
                            nc.tensor.matmul(
                                pb[:], xs[:, kk, msl], Sns[:, kk, 0:KP],
                                start=(kk == 0), stop=(kk == 3),
                            )
                        nc.scalar.copy(out=Ar[:, m, :], in_=pa[:])
                        nc.scalar.copy(out=Ai[:, m, :], in_=pb[:])
                        nc.gpsimd.tensor_tensor(
                            Apb[:, m, :], Ar[:, m, :], Ai[:, m, :],
                            mybir.AluOpType.add,
                        )

                    # ---- Stage 2 (matrix-stationary, Gauss):
                    # m1 = C.(Ar+Ai) ; m2 = (-S-C).Ar ; m3 = (C-S).Ai
                    # Htr = m1 - m3 ; Hti = m1 + m2 ; pointwise per l-tile
                    Ztr = zpool.tile([P, 4, KP], mmdt, tag="Ztr")
                    Zti = zpool.tile([P, 4, KP], mmdt, tag="Zti")
                    for lm in range(4):
                        lsl = slice(lm * P, (lm + 1) * P)
                        m1 = psum.tile([P, N], f32, tag="ps", name="m1")[:, :KP]
                        m2 = psum.tile([P, N], f32, tag="ps", name="m2")[:, :KP]
                        m3 = psum.tile([P, N], f32, tag="ps", name="m3")[:, :KP]
                        for kk in range(4):
                            nc.tensor.matmul(
                                m1[:], Cs[:, kk, lsl], Apb[:, kk, :],
                                start=(kk == 0), stop=(kk == 3),
                            )
                        for kk in range(4):
                            nc.tensor.matmul(
                                m2[:], nSCs[:, kk, lsl], Ar[:, kk, :],
                                start=(kk == 0), stop=(kk == 3),
                            )
                        for kk in range(4):
                            nc.tensor.matmul(
                                m3[:], CmSs[:, kk, lsl], Ai[:, kk, :],
                                start=(kk == 0), stop=(kk == 3),
                            )
                        # Htr = m1 - m3 ; Hti = m1 + m2
                        m1s = pw.tile([P, KP], f32, tag="m1s")
                        hrs = pw.tile([P, KP], f32, tag="hrs")
                        his = pw.tile([P, KP], f32, tag="his")
                        nc.scalar.copy(out=m1s[:], in_=m1[:])
                        nc.vector.tensor_sub(out=hrs[:], in0=m1s[:], in1=m3[:])
                        nc.vector.tensor_add(out=his[:], in0=m1s[:], in1=m2[:])
                        # pointwise: Ztr = hr o fr - hi o fi
                        #            Zti = hr o fi + hi o fr
                        fr = fktr[:, lm, :]
                        fi = fkti[:, lm, :]
                        tt = pw.tile([P, KP], f32, tag="tt")
                        tu = pw.tile([P, KP], f32, tag="tu")
                        ztr = Ztr[:, lm, :]
                        zti = Zti[:, lm, :]
                        nc.vector.tensor_mul(out=ztr, in0=hrs[:], in1=fr)
                        nc.gpsimd.tensor_tensor(
                            tt[:], his[:], fi, mybir.AluOpType.mult
                        )
                        nc.vector.tensor_sub(out=ztr, in0=ztr, in1=tt[:])
                        nc.gpsimd.tensor_tensor(
                            tu[:], hrs[:], fi, mybir.AluOpType.mult
                        )
                        nc.vector.tensor_mul(out=zti, in0=his[:], in1=fr)
                        nc.vector.tensor_add(out=zti, in0=zti, in1=tu[:])

                    # fill the pointwise-latency bubble with the previous
                    # image's stage 4 (independent PE work)
                    if pending is not None:
                        emit_st4(*pending)
                        pending = None

                    # ---- Stage 3 (data-stationary, weight-paired direct):
                    # V_r = Ztr^T C + Zti^T (-S) ; V_i = Ztr^T S + Zti^T C
                    # k M-tiles: 0:128, 128:256, 130:258
                    Vr = vpool.tile([P, 3, N], mmdt, tag="Vr")
                    Vi = vpool.tile([P, 3, N], mmdt, tag="Vi")
                    for km in range(3):
                        koff = (0, 128, 130)[km]
                        ksl = slice(koff, koff + P)
                        nvr = psum.tile([P, N], f32, tag="ps", name="nvr")
                        nvi = psum.tile([P, N], f32, tag="ps", name="nvi")
                        for lt in range(4):
                            nc.tensor.matmul(
                                nvr[:], Ztr[:, lt, ksl], Cs[:, lt, :],
                                start=(lt == 0), stop=False,
                            )
                            nc.tensor.matmul(
                                nvi[:], Ztr[:, lt, ksl], Ss[:, lt, :],
                                start=(lt == 0), stop=False,
                            )
                        for lt in range(4):
                            nc.tensor.matmul(
                                nvr[:], Zti[:, lt, ksl], Sns[:, lt, :],
                                start=False, stop=(lt == 3),
                            )
                            nc.tensor.matmul(
                                nvi[:], Zti[:, lt, ksl], Cs[:, lt, :],
                                start=False, stop=(lt == 3),
                            )
                        nc.any.tensor_copy(out=Vr[:, km, :], in_=nvr[:])
                        nc.any.tensor_copy(out=Vi[:, km, :], in_=nvi[:])

                    pending = (Vr, Vi, img)


            if pending is not None:
                emit_st4(*pending)

    nc.compile()
    return nc


def _host_spectra(kernels):
    """Compose step kernels into 21 cumulative half-spectra, transposed to
    [l, k] layout with Hermitian weights, 1/N^2, and the stage-4
    double-count halving folded in. Returns (FKtr, FKti) f32 [21, 512, KP]."""
    kernels = np.asarray(kernels, dtype=np.float64)
    h = np.zeros((T_STEPS, N, N), np.float64)
    idx = (KS // 2 - np.arange(KS)) % N
    h[:, idx[:, None], idx[None, :]] = kernels
    s_step = np.fft.fft2(h)
    cum = np.ones((T_STEPS + 1, N, N), np.complex128)
    for i in range(1, T_STEPS + 1):
        cum[i] = cum[i - 1] * s_step[i - 1]
    w = np.zeros(KP)
    w[: N // 2 + 1] = 2.0
    w[0] = w[N // 2] = 1.0
    fkt = (cum[:, :KP, :] * w[None, :, None] / float(N * N)).transpose(0, 2, 1)
    half = np.ones(KP)
    half[130:256] = 0.5  # k rows 130..255 appear in both stage-4 k-tiles
    fkt = fkt * half[None, None, :]
    return (
        np.ascontiguousarray(fkt.real.astype(np.float32)),
        np.ascontiguousarray(fkt.imag.astype(np.float32)),
    )


def _dft_mats():
    j = np.arange(N)
    ang = 2.0 * np.pi * (np.outer(j, j) % N) / N
    cm = np.cos(ang).astype(np.float32)
    sm = np.sin(ang).astype(np.float32)
    return {
        "cmat": cm,
        "smat": sm,
        "snmat": np.ascontiguousarray(-sm),
        "nscmat": np.ascontiguousarray(-sm - cm),
        "cmsmat": np.ascontiguousarray(cm - sm),
    }


def _kernel_spectral(x0, tt, kernels):
    global LAST_EXEC_NS, LAST_TRACE
    from concourse import bass_utils

    x0 = np.ascontiguousarray(np.asarray(x0), dtype=np.float32)
    fktr_all, fkti_all = _host_spectra(kernels)
    mats = _dft_mats()

    if "spec" not in _PROGRAMS:
        _PROGRAMS["spec"] = _build_program_spec()
    nc = _PROGRAMS["spec"]

    in_maps = []
    for c in range(NCORES):
        sl = slice(c * SPC, (c + 1) * SPC)
        ts = tt[sl]
        im = {
            "x": np.ascontiguousarray(x0[sl].reshape(IMGS, N, N)),
            "fkr": np.ascontiguousarray(fktr_all[ts]),
            "fki": np.ascontiguousarray(fkti_all[ts]),
        }
        im.update(mats)
        in_maps.append(im)

    res = bass_utils.run_bass_kernel_spmd(
        nc, in_maps, core_ids=list(range(NCORES)), trace=TRACE
    )
    LAST_EXEC_NS = res.exec_time_ns
    if res.instructions_and_trace is not None:
        LAST_TRACE = res.instructions_and_trace[1]
    out = np.empty((BATCH, CHANNELS, N, N), np.float32)
    for c in range(NCORES):
        out[c * SPC : (c + 1) * SPC] = res.results[c]["out"].reshape(
            SPC, CHANNELS, N, N
        )
    return out


def kernel(x0, t, kernels):
    tt = np.asarray(t).astype(np.int64)
    k_sp = _composed_kernels(kernels)
    fac = _rank1_factors(k_sp)
    if fac is not None:
        return _kernel_separable(x0, tt, fac[0], fac[1], fac[2])
    return _kernel_spectral(x0, tt, kernels)


# revision 18
# speedup vs baseline: 7.7713x; 1.1487x over previous
"""BlurDegradation kernel for 8x TRN2 NeuronCores.

Math: t[b] successive 11x11 depthwise *circular* convolutions compose into a
single circular convolution with kernel k_t = h_1 (*) h_2 (*) ... (*) h_t
(circular 2D convolution of the per-step impulse responses). The host
composes k_t exactly with FFTs.

Fast path (separable): when every composed k_t is numerically rank-1
(k_t = outer(a_t, b_t) -- always true for the constant-Gaussian blur routine,
whose steps are separable), the 2D blur factors into a column circular conv
by a_t followed by a row circular conv by b_t. Each is one dense circulant
matmul, so an image costs just two 512x512x512 matmul chains on the PE array
(268M MACs vs 941M for the spectral pipeline). Both stages are
data-stationary so outputs chain orientation [h,w] -> [w,v] -> [v,x] with no
transposes:
  stage 1: T1[w,v] = sum_h X[h,w] * Gc[h,v],  Gc[h,v] = a[(v-h) mod N]
  stage 2: Z [v,x] = sum_w T1[w,v] * Gr[w,x], Gr[w,x] = b[(x-w) mod N]
Matmuls run in bf16 (1 cycle/row + fast weight load) with fp32 PSUM
accumulation; stage-2 of image i is emitted after stage-1 of image i+1 so the
PSUM->SBUF cast copies never stall the PE.

Fallback path (general kernels): the original spectral pipeline -- per-sample
cumulative half-spectra multiply between dense DFT matmuls.

Sharding: pure data parallel, 8 samples per core, no cross-core comms.
"""

import numpy as np

N = 512
P = 128
T_STEPS = 20
KS = 11
KP = 258            # padded half-spectrum k-dim (even for fp32r)
NCORES = 8
BATCH = 64
CHANNELS = 3
SPC = BATCH // NCORES  # samples per core
IMGS = SPC * CHANNELS  # images per core

USE_F32R = True

_PROGRAMS = {}
TRACE = False
LAST_EXEC_NS = None
LAST_TRACE = None


# --------------------------------------------------------------------------
# Separable fast path
# --------------------------------------------------------------------------

# The composed blur kernel after t steps has support <= +-5t, so the
# circulant G[a, b] = vec[(b-a) mod 512] is band-limited: the [128 x 512]
# tile for contraction block kk has nonzero columns only in the contiguous
# (mod 512) span [128kk - W, 128(kk+1) + W). Samples are globally sorted by
# t so that slot j on every core shares a compiled band halfwidth W_j.
# PSUM's per-element has_written bit makes partial-column accumulation work:
# the group's first matmul (start=True) clears the whole bank's bits, later
# matmuls overwrite where unwritten and accumulate where written.
def _plan_spans(W):
    """Per contraction tile kk: fused contiguous column ranges of its band
    (split only at the mod-512 wrap). Returns [(kk, c0, c1), ...]."""
    plans = []
    for kk in range(4):
        c0 = (P * kk - W) % N
        ln = P + 2 * W
        if ln >= N:
            plans.append((kk, 0, N))
        elif c0 + ln <= N:
            plans.append((kk, c0, c0 + ln))
        else:
            plans.append((kk, c0, N))
            plans.append((kk, 0, c0 + ln - N))
    return plans


def _plan_spans_1s(W):
    """One-sided band (filter pre-rolled to support [0, 2W], input pre-rolled
    by -W): tile kk covers cols [128kk, 128(kk+1)+2W) mod 512. Returns
    [(kk, c0, c1, q0), ...] with q0 the packed-G column offset."""
    plans = []
    for kk in range(4):
        base = P * kk
        ln = min(P + 2 * W, N)
        if base + ln <= N:
            plans.append((kk, base, base + ln, 0))
        else:
            plans.append((kk, base, N, 0))
            plans.append((kk, 0, base + ln - N, N - base))
    return plans


def _build_program_sep_shared(Wj):
    """Shared-G (symmetric factors) program: per-slot band-packed circulant
    G tile M[p, q] = a'[(q - p) mod 512] (identical for all 4 contraction
    tiles), kept resident in SBUF for all 8 slots."""
    import concourse.mybir as mybir
    import concourse.tile as tile
    from concourse import bacc

    f32 = mybir.dt.float32
    bf16 = mybir.dt.bfloat16

    nc = bacc.Bacc(
        "TRN2", target_bir_lowering=False, debug=False, num_devices=NCORES
    )
    x_d = nc.dram_tensor("x", [IMGS, N, N], bf16, kind="ExternalInput").ap()
    cws = [min(P + 2 * W, N) for W in Wj]
    g_d = [
        nc.dram_tensor(f"g{j}", [P, cws[j]], bf16, kind="ExternalInput").ap()
        for j in range(SPC)
    ]
    out_d = nc.dram_tensor("out", [IMGS, N, N], bf16, kind="ExternalOutput").ap()

    with tile.TileContext(nc) as tc:
        with (
            tc.tile_pool(name="gp", bufs=1) as gp,
            tc.tile_pool(name="xsp", bufs=4) as xsp,
            tc.tile_pool(name="t1p", bufs=3) as t1p,
            tc.tile_pool(name="outp", bufs=3) as outp,
            tc.tile_pool(name="psum", bufs=8, space="PSUM") as psum,
        ):
            # all per-slot G bands stay resident (~4KB/partition total)
            gt = []
            for j in range(SPC):
                g = gp.tile([P, cws[j]], bf16, name=f"g{j}_s")
                nc.sync.dma_start(g[:], g_d[j])
                gt.append(g)

            def banded_mms(pa, data, msl, g, plans):
                last = len(plans) - 1
                for i, (kk, c0, c1, q0) in enumerate(plans):
                    nc.tensor.matmul(
                        pa[:, c0:c1], data[:, kk, msl], g[:, q0 : q0 + c1 - c0],
                        start=(i == 0), stop=(i == last),
                    )

            def emit_st2(T1, g, img, plans):
                # stage 2 (row conv): Z[v,x] = sum_w T1[w,v] G[w,x]
                outs = outp.tile([P, 4, N], bf16, tag="outs")
                for m in range(4):
                    msl = slice(m * P, (m + 1) * P)
                    pb = psum.tile([P, N], f32, tag="ps", name="pb")
                    banded_mms(pb, T1, msl, g, plans)
                    if m % 2 == 0:
                        nc.vector.tensor_copy(out=outs[:, m, :], in_=pb[:])
                    else:
                        nc.scalar.copy(out=outs[:, m, :], in_=pb[:])
                nc.sync.dma_start(
                    out_d[img].rearrange("(i p) n -> p i n", p=P), outs[:]
                )

            pending = None  # (T1, g, img, plans) of the previous image

            for s in range(SPC):
                plans = _plan_spans_1s(Wj[s])
                for ch in range(CHANNELS):
                    img = s * CHANNELS + ch
                    xs = xsp.tile([P, 4, N], bf16, tag="xs")
                    nc.gpsimd.dma_start(
                        xs[:], x_d[img].rearrange("(i p) n -> p i n", p=P)
                    )
                    # stage 1 (col conv): T1[w,v] = sum_h X[h,w] G[h,v]
                    T1 = t1p.tile([P, 4, N], bf16, tag="T1")
                    for m in range(4):
                        msl = slice(m * P, (m + 1) * P)
                        pa = psum.tile([P, N], f32, tag="ps", name="pa")
                        banded_mms(pa, xs, msl, gt[s], plans)
                        if m % 2 == 0:
                            nc.vector.tensor_copy(out=T1[:, m, :], in_=pa[:])
                        else:
                            nc.scalar.copy(out=T1[:, m, :], in_=pa[:])
                    if pending is not None:
                        emit_st2(*pending)
                    pending = (T1, gt[s], img, plans)

            if pending is not None:
                emit_st2(*pending)

    nc.compile()
    return nc


def _build_program_sep(Wj, shared):
    import concourse.mybir as mybir
    import concourse.tile as tile
    from concourse import bacc

    f32 = mybir.dt.float32
    bf16 = mybir.dt.bfloat16

    nc = bacc.Bacc(
        "TRN2", target_bir_lowering=False, debug=False, num_devices=NCORES
    )
    x_d = nc.dram_tensor("x", [IMGS, N, N], bf16, kind="ExternalInput").ap()
    gc_d = nc.dram_tensor("gc", [SPC, N, N], bf16, kind="ExternalInput").ap()
    gr_d = None
    if not shared:
        gr_d = nc.dram_tensor("gr", [SPC, N, N], bf16, kind="ExternalInput").ap()
    out_d = nc.dram_tensor("out", [IMGS, N, N], bf16, kind="ExternalOutput").ap()

    with tile.TileContext(nc) as tc:
        with (
            tc.tile_pool(name="gp", bufs=4 if shared else 3) as gp,
            tc.tile_pool(name="xsp", bufs=4) as xsp,
            tc.tile_pool(name="t1p", bufs=3) as t1p,
            tc.tile_pool(name="outp", bufs=3) as outp,
            tc.tile_pool(name="psum", bufs=8, space="PSUM") as psum,
        ):
            def banded_mms(pa, data, msl, g, plans):
                # fused band spans; only the group's first matmul starts the
                # bank (whole-bank has_written clear), the rest accumulate
                # per-element. Consecutive same-kk matmuls share LDWEIGHTS.
                last = len(plans) - 1
                for i, (kk, c0, c1) in enumerate(plans):
                    nc.tensor.matmul(
                        pa[:, c0:c1], data[:, kk, msl], g[:, kk, c0:c1],
                        start=(i == 0), stop=(i == last),
                    )

            def emit_st2(T1, grs, img, plans):
                # stage 2 (row conv): Z[v,x] = sum_w T1[w,v] Gr[w,x]
                outs = outp.tile([P, 4, N], bf16, tag="outs")
                for m in range(4):
                    msl = slice(m * P, (m + 1) * P)
                    pb = psum.tile([P, N], f32, tag="ps", name="pb")
                    banded_mms(pb, T1, msl, grs, plans)
                    if m % 2 == 0:
                        nc.vector.tensor_copy(out=outs[:, m, :], in_=pb[:])
                    else:
                        nc.scalar.copy(out=outs[:, m, :], in_=pb[:])
                nc.sync.dma_start(
                    out_d[img].rearrange("(i p) n -> p i n", p=P), outs[:]
                )

            pending = None  # (T1, grs, img, plans) of the previous image

            for s in range(SPC):
                plans = _plan_spans(Wj[s])
                gcs = gp.tile([P, 4, N], bf16, tag="gcs")
                nc.sync.dma_start(
                    gcs[:], gc_d[s].rearrange("(i p) n -> p i n", p=P)
                )
                if shared:
                    grs = gcs
                else:
                    grs = gp.tile([P, 4, N], bf16, tag="grs")
                    nc.sync.dma_start(
                        grs[:], gr_d[s].rearrange("(i p) n -> p i n", p=P)
                    )
                for ch in range(CHANNELS):
                    img = s * CHANNELS + ch
                    xs = xsp.tile([P, 4, N], bf16, tag="xs")
                    nc.sync.dma_start(
                        xs[:], x_d[img].rearrange("(i p) n -> p i n", p=P)
                    )
                    # stage 1 (col conv): T1[w,v] = sum_h X[h,w] Gc[h,v]
                    T1 = t1p.tile([P, 4, N], bf16, tag="T1")
                    for m in range(4):
                        msl = slice(m * P, (m + 1) * P)
                        pa = psum.tile([P, N], f32, tag="ps", name="pa")
                        banded_mms(pa, xs, msl, gcs, plans)
                        if m % 2 == 0:
                            nc.vector.tensor_copy(out=T1[:, m, :], in_=pa[:])
                        else:
                            nc.scalar.copy(out=T1[:, m, :], in_=pa[:])
                    # fill the copy-latency bubble with the previous image's
                    # stage 2 (independent PE work)
                    if pending is not None:
                        emit_st2(*pending)
                    pending = (T1, grs, img, plans)

            if pending is not None:
                emit_st2(*pending)

    nc.compile()
    return nc


def _composed_kernels(kernels):
    """Exact composed spatial kernels k_t, [T+1, N, N] float64 (k_0 = delta).
    out_t = k_t (*) x as a 2D circular convolution."""
    kernels = np.asarray(kernels, dtype=np.float64)
    h = np.zeros((T_STEPS, N, N), np.float64)
    idx = (KS // 2 - np.arange(KS)) % N
    h[:, idx[:, None], idx[None, :]] = kernels
    s_step = np.fft.fft2(h)
    cum = np.empty((T_STEPS + 1, N, N), np.complex128)
    cum[0] = 1.0
    for i in range(1, T_STEPS + 1):
        cum[i] = cum[i - 1] * s_step[i - 1]
    return np.fft.ifft2(cum).real


def _rank1_factors(k_sp):
    """If every composed kernel is rank-1, return (A, B, shared) with
    k_sp[t] == outer(A[t], B[t]); shared=True when A==B for all t (symmetric
    kernels -- lets the device reuse one circulant for both stages).
    Returns None if any level is not rank-1."""
    A = np.zeros((T_STEPS + 1, N))
    B = np.zeros((T_STEPS + 1, N))
    shared = True
    for tl in range(T_STEPS + 1):
        K = k_sp[tl]
        am = np.abs(K).max()
        if am == 0.0:
            return None
        sym = False
        if np.abs(K - K.T).max() <= 1e-9 * am:
            i0 = int(np.argmax(np.diagonal(K)))
            piv = K[i0, i0]
            if piv > 1e-12 * am:
                a = K[:, i0] / np.sqrt(piv)
                b = a
                sym = True
        if not sym:
            shared = False
            i0, j0 = np.unravel_index(np.abs(K).argmax(), K.shape)
            a = K[:, j0] / K[i0, j0]
            b = K[i0, :]
        if np.abs(K - np.outer(a, b)).max() > 1e-6 * am:
            return None
        A[tl] = a
        B[tl] = b
    return A, B, shared


_SHIFT_IDX = (np.arange(N)[None, :] - np.arange(N)[:, None]) % N


def _circulant(vec):
    # M[h, v] = vec[(v - h) mod N]
    return vec[_SHIFT_IDX]


def _kernel_separable(x0, tt, A, B, shared):
    global LAST_EXEC_NS, LAST_TRACE
    from concourse import bass_utils
    import ml_dtypes

    # Effective band halfwidth per t: the smallest W whose tail mass (both
    # factors) is <= 3e-5 of the filter's L1 norm -- the induced output error
    # is orders of magnitude below the 2e-2 gate, and Gaussian tails shrink
    # W(t=20) from 100 to ~60.
    Weff = {}
    for tv in np.unique(tt):
        tv = int(tv)
        Ws = min(5 * tv, P - 1)
        W = Ws
        for cand in range(Ws + 1):
            tol = 3e-5
            ta = np.abs(A[tv][cand + 1 : N - cand]).sum() if cand < N // 2 else 0.0
            tb = np.abs(B[tv][cand + 1 : N - cand]).sum() if cand < N // 2 else 0.0
            if ta <= tol * np.abs(A[tv]).sum() and tb <= tol * np.abs(B[tv]).sum():
                W = cand
                break
        Weff[tv] = W

    # sort samples by descending effective W; core c slot j <- sorted
    # position 8j + c, so every core's slot j shares the compiled band
    # halfwidth W_j = max Weff. Heavy slots run first (DMA prefetch warms up
    # behind long PE bursts), cheap slots drain fast at the tail.
    wt = np.array([Weff[int(tv)] for tv in tt])
    order = np.argsort(-wt, kind="stable")
    Wj = tuple(int(wt[order[NCORES * j]]) for j in range(SPC))

    key = ("sep", Wj, shared)
    if key not in _PROGRAMS:
        _PROGRAMS[key] = (
            _build_program_sep_shared(Wj) if shared
            else _build_program_sep(Wj, shared)
        )
    nc = _PROGRAMS[key]

    bf = ml_dtypes.bfloat16
    xb = np.asarray(x0, dtype=np.float32).astype(bf)

    in_maps = []
    origs = []
    if shared:
        # band-packed G: M[p, q] = a'[(q - p) mod 512], a' = roll(a, W_slot)
        GP = {}
        for j in range(SPC):
            W = Wj[j]
            cw = min(P + 2 * W, N)
            GP[W] = (np.arange(cw)[None, :] - np.arange(P)[:, None]) % N
        for c in range(NCORES):
            orig = order[np.arange(SPC) * NCORES + c]
            origs.append(orig)
            im = {}
            xcore = np.empty((SPC, CHANNELS, N, N), bf)
            for j in range(SPC):
                W = Wj[j]
                tv = int(tt[orig[j]])
                Ws = Weff[tv]
                a = A[tv].copy()
                if Ws < N // 2:
                    a[Ws + 1 : N - Ws] = 0.0
                ap_ = np.roll(a, W)
                im[f"g{j}"] = ap_[GP[W]].astype(bf)
                xcore[j] = np.roll(xb[orig[j]], (-W, -W), axis=(-2, -1))
            im["x"] = np.ascontiguousarray(xcore.reshape(IMGS, N, N))
            in_maps.append(im)
    else:
        GC = {}
        GR = {}
        for tv in np.unique(tt):
            tv = int(tv)
            W = min(5 * tv, P - 1)
            a = A[tv].copy()
            a[W + 1 : N - W] = 0.0  # drop FFT noise outside the band
            GC[tv] = _circulant(a).astype(bf)
            b = B[tv].copy()
            b[W + 1 : N - W] = 0.0
            GR[tv] = _circulant(b).astype(bf)
        for c in range(NCORES):
            orig = order[np.arange(SPC) * NCORES + c]
            origs.append(orig)
            ts = tt[orig]
            in_maps.append({
                "x": np.ascontiguousarray(xb[orig].reshape(IMGS, N, N)),
                "gc": np.stack([GC[int(tv)] for tv in ts]),
                "gr": np.stack([GR[int(tv)] for tv in ts]),
            })

    res = bass_utils.run_bass_kernel_spmd(
        nc, in_maps, core_ids=list(range(NCORES)), trace=TRACE
    )
    LAST_EXEC_NS = res.exec_time_ns
    if res.instructions_and_trace is not None:
        LAST_TRACE = res.instructions_and_trace[1]
    out = np.empty((BATCH, CHANNELS, N, N), np.float32)
    for c in range(NCORES):
        out[origs[c]] = (
            np.asarray(res.results[c]["out"])
            .astype(np.float32)
            .reshape(SPC, CHANNELS, N, N)
        )
    return out


# --------------------------------------------------------------------------
# Spectral fallback (general, possibly non-separable kernels)
# --------------------------------------------------------------------------

def _build_program_spec():
    import concourse.mybir as mybir
    import concourse.tile as tile
    from concourse import bacc

    f32 = mybir.dt.float32
    f32r = mybir.dt.float32r
    mmdt = f32r if USE_F32R else f32

    nc = bacc.Bacc(
        "TRN2", target_bir_lowering=False, debug=False, num_devices=NCORES
    )
    x_d = nc.dram_tensor("x", [IMGS, N, N], mmdt, kind="ExternalInput").ap()
    fkr_d = nc.dram_tensor("fkr", [SPC, N, KP], f32, kind="ExternalInput").ap()
    fki_d = nc.dram_tensor("fki", [SPC, N, KP], f32, kind="ExternalInput").ap()
    mat_names = ["cmat", "smat", "snmat", "nscmat", "cmsmat"]
    mat_d = {
        nm: nc.dram_tensor(nm, [N, N], mmdt, kind="ExternalInput").ap()
        for nm in mat_names
    }
    out_d = nc.dram_tensor("out", [IMGS, N, N], f32, kind="ExternalOutput").ap()

    with tile.TileContext(nc) as tc:
        with (
            tc.tile_pool(name="mats", bufs=1) as mats,
            tc.tile_pool(name="xsp", bufs=2) as xsp,
            tc.tile_pool(name="outp", bufs=2) as outp,
            tc.tile_pool(name="fkp", bufs=2) as fkp,
            tc.tile_pool(name="apool", bufs=2) as apool,
            tc.tile_pool(name="hpool", bufs=2) as hpool,
            tc.tile_pool(name="zpool", bufs=2) as zpool,
            tc.tile_pool(name="vpool", bufs=2) as vpool,
            tc.tile_pool(name="pw", bufs=3) as pw,
            tc.tile_pool(name="psum", bufs=8, space="PSUM") as psum,
        ):
            # resident DFT matrices, [p, tile, n] layout
            M = {}
            for nm in mat_names:
                mt = mats.tile([P, 4, N], mmdt, name=nm + "_s")
                nc.sync.dma_start(mt[:], mat_d[nm].rearrange("(i p) n -> p i n", p=P))
                M[nm] = mt
            Cs, Ss, Sns = M["cmat"], M["smat"], M["snmat"]
            nSCs, CmSs = M["nscmat"], M["cmsmat"]
            # C/-S rows 130..257, partition-aligned (stage-4 k tail tile)
            Ck2s = mats.tile([P, N], mmdt, name="ck2_s")
            Snk2s = mats.tile([P, N], mmdt, name="snk2_s")
            nc.sync.dma_start(Ck2s[:], mat_d["cmat"][130:258, :])
            nc.sync.dma_start(Snk2s[:], mat_d["snmat"][130:258, :])

            def emit_st4(Vr, Vi, img):
                # ---- Stage 4 (matrix-stationary, natural orientation):
                # out[y,x] = sum_k C[k,y] Vr[k,x] + (-S)[k,y] Vi[k,x]
                # k tiles: 0:128, 128:256, 130:258 (FK pre-halved on the
                # double-counted 130..255 range)
                outs = outp.tile([P, 4, N], f32, tag="outs")
                for ym in range(4):
                    ysl = slice(ym * P, (ym + 1) * P)
                    po = psum.tile([P, N], f32, tag="ps", name="po")
                    nc.tensor.matmul(
                        po[:], Cs[:, 0, ysl], Vr[:, 0, :],
                        start=True, stop=False,
                    )
                    nc.tensor.matmul(
                        po[:], Cs[:, 1, ysl], Vr[:, 1, :],
                        start=False, stop=False,
                    )
                    nc.tensor.matmul(
                        po[:], Ck2s[:, ysl], Vr[:, 2, :],
                        start=False, stop=False,
                    )
                    nc.tensor.matmul(
                        po[:], Sns[:, 0, ysl], Vi[:, 0, :],
                        start=False, stop=False,
                    )
                    nc.tensor.matmul(
                        po[:], Sns[:, 1, ysl], Vi[:, 1, :],
                        start=False, stop=False,
                    )
                    nc.tensor.matmul(
                        po[:], Snk2s[:, ysl], Vi[:, 2, :],
                        start=False, stop=True,
                    )
                    nc.any.tensor_copy(out=outs[:, ym, :], in_=po[:])
                nc.sync.dma_start(
                    out_d[img].rearrange("(i p) n -> p i n", p=P), outs[:]
                )

            pending = None  # (Vr, Vi, img) of the previous image

            for s in range(SPC):
                # per-sample spectrum, transposed [l, k] layout, k cols 0..257
                fktr = fkp.tile([P, 4, KP], f32, tag="fktr")
                fkti = fkp.tile([P, 4, KP], f32, tag="fkti")
                nc.sync.dma_start(
                    fktr[:], fkr_d[s].rearrange("(i p) n -> p i n", p=P)
                )
                nc.sync.dma_start(
                    fkti[:], fki_d[s].rearrange("(i p) n -> p i n", p=P)
                )

                for ch in range(CHANNELS):
                    img = s * CHANNELS + ch
                    xs = xsp.tile([P, 4, N], mmdt, tag="xs")
                    nc.sync.dma_start(
                        xs[:], x_d[img].rearrange("(i p) n -> p i n", p=P)
                    )

                    # ---- Stage 1 (data-stationary, weight-paired):
                    # A_r = x^T C[:, :258] ; A_i = x^T (-S)[:, :258]
                    Ar = apool.tile([P, 4, KP], mmdt, tag="Ar")
                    Ai = apool.tile([P, 4, KP], mmdt, tag="Ai")
                    Apb = apool.tile([P, 4, KP], mmdt, tag="Apb")
                    for m in range(4):
                        msl = slice(m * P, (m + 1) * P)
                        pa = psum.tile([P, N], f32, tag="ps", name="pa")[:, :KP]
                        pb = psum.tile([P, N], f32, tag="ps", name="pb")[:, :KP]
                        for kk in range(4):
                            nc.tensor.matmul(
                                pa[:], xs[:, kk, msl], Cs[:, kk, 0:KP],
                                start=(kk == 0), stop=(kk == 3),
                            )
                            nc.tensor.matmul(
                                pb[:], xs[:, kk, msl], Sns[:, kk, 0:KP],
                                start=(kk == 0), stop=(kk == 3),
                            )
                        nc.scalar.copy(out=Ar[:, m, :], in_=pa[:])
                        nc.scalar.copy(out=Ai[:, m, :], in_=pb[:])
                        nc.gpsimd.tensor_tensor(
                            Apb[:, m, :], Ar[:, m, :], Ai[:, m, :],
                            mybir.AluOpType.add,
                        )

                    # ---- Stage 2 (matrix-stationary, Gauss):
                    # m1 = C.(Ar+Ai) ; m2 = (-S-C).Ar ; m3 = (C-S).Ai
                    # Htr = m1 - m3 ; Hti = m1 + m2 ; pointwise per l-tile
                    Ztr = zpool.tile([P, 4, KP], mmdt, tag="Ztr")
                    Zti = zpool.tile([P, 4, KP], mmdt, tag="Zti")
                    for lm in range(4):
                        lsl = slice(lm * P, (lm + 1) * P)
                        m1 = psum.tile([P, N], f32, tag="ps", name="m1")[:, :KP]
                        m2 = psum.tile([P, N], f32, tag="ps", name="m2")[:, :KP]
                        m3 = psum.tile([P, N], f32, tag="ps", name="m3")[:, :KP]
                        for kk in range(4):
                            nc.tensor.matmul(
                                m1[:], Cs[:, kk, lsl], Apb[:, kk, :],
                                start=(kk == 0), stop=(kk == 3),
                            )
                        for kk in range(4):
                            nc.tensor.matmul(
                                m2[:], nSCs[:, kk, lsl], Ar[:, kk, :],
                                start=(kk == 0), stop=(kk == 3),
                            )
                        for kk in range(4):
                            nc.tensor.matmul(
                                m3[:], CmSs[:, kk, lsl], Ai[:, kk, :],
                                start=(kk == 0), stop=(kk == 3),
                            )
                        # Htr = m1 - m3 ; Hti = m1 + m2
                        m1s = pw.tile([P, KP], f32, tag="m1s")
                        hrs = pw.tile([P, KP], f32, tag="hrs")
                        his = pw.tile([P, KP], f32, tag="his")
                        nc.scalar.copy(out=m1s[:], in_=m1[:])
                        nc.vector.tensor_sub(out=hrs[:], in0=m1s[:], in1=m3[:])
                        nc.vector.tensor_add(out=his[:], in0=m1s[:], in1=m2[:])
                        # pointwise: Ztr = hr o fr - hi o fi
                        #            Zti = hr o fi + hi o fr
                        fr = fktr[:, lm, :]
                        fi = fkti[:, lm, :]
                        tt = pw.tile([P, KP], f32, tag="tt")
                        tu = pw.tile([P, KP], f32, tag="tu")
                        ztr = Ztr[:, lm, :]
                        zti = Zti[:, lm, :]
                        nc.vector.tensor_mul(out=ztr, in0=hrs[:], in1=fr)
                        nc.gpsimd.tensor_tensor(
                            tt[:], his[:], fi, mybir.AluOpType.mult
                        )
                        nc.vector.tensor_sub(out=ztr, in0=ztr, in1=tt[:])
                        nc.gpsimd.tensor_tensor(
                            tu[:], hrs[:], fi, mybir.AluOpType.mult
                        )
                        nc.vector.tensor_mul(out=zti, in0=his[:], in1=fr)
                        nc.vector.tensor_add(out=zti, in0=zti, in1=tu[:])

                    # fill the pointwise-latency bubble with the previous
                    # image's stage 4 (independent PE work)
                    if pending is not None:
                        emit_st4(*pending)
                        pending = None

                    # ---- Stage 3 (data-stationary, weight-paired direct):
                    # V_r = Ztr^T C + Zti^T (-S) ; V_i = Ztr^T S + Zti^T C
                    # k M-tiles: 0:128, 128:256, 130:258
                    Vr = vpool.tile([P, 3, N], mmdt, tag="Vr")
                    Vi = vpool.tile([P, 3, N], mmdt, tag="Vi")
                    for km in range(3):
                        koff = (0, 128, 130)[km]
                        ksl = slice(koff, koff + P)
                        nvr = psum.tile([P, N], f32, tag="ps", name="nvr")
                        nvi = psum.tile([P, N], f32, tag="ps", name="nvi")
                        for lt in range(4):
                            nc.tensor.matmul(
                                nvr[:], Ztr[:, lt, ksl], Cs[:, lt, :],
                                start=(lt == 0), stop=False,
                            )
                            nc.tensor.matmul(
                                nvi[:], Ztr[:, lt, ksl], Ss[:, lt, :],
                                start=(lt == 0), stop=False,
                            )
                        for lt in range(4):
                            nc.tensor.matmul(
                                nvr[:], Zti[:, lt, ksl], Sns[:, lt, :],
                                start=False, stop=(lt == 3),
                            )
                            nc.tensor.matmul(
                                nvi[:], Zti[:, lt, ksl], Cs[:, lt, :],
                                start=False, stop=(lt == 3),
                            )
                        nc.any.tensor_copy(out=Vr[:, km, :], in_=nvr[:])
                        nc.any.tensor_copy(out=Vi[:, km, :], in_=nvi[:])

                    pending = (Vr, Vi, img)


            if pending is not None:
                emit_st4(*pending)

    nc.compile()
    return nc


def _host_spectra(kernels):
    """Compose step kernels into 21 cumulative half-spectra, transposed to
    [l, k] layout with Hermitian weights, 1/N^2, and the stage-4
    double-count halving folded in. Returns (FKtr, FKti) f32 [21, 512, KP]."""
    kernels = np.asarray(kernels, dtype=np.float64)
    h = np.zeros((T_STEPS, N, N), np.float64)
    idx = (KS // 2 - np.arange(KS)) % N
    h[:, idx[:, None], idx[None, :]] = kernels
    s_step = np.fft.fft2(h)
    cum = np.ones((T_STEPS + 1, N, N), np.complex128)
    for i in range(1, T_STEPS + 1):
        cum[i] = cum[i - 1] * s_step[i - 1]
    w = np.zeros(KP)
    w[: N // 2 + 1] = 2.0
    w[0] = w[N // 2] = 1.0
    fkt = (cum[:, :KP, :] * w[None, :, None] / float(N * N)).transpose(0, 2, 1)
    half = np.ones(KP)
    half[130:256] = 0.5  # k rows 130..255 appear in both stage-4 k-tiles
    fkt = fkt * half[None, None, :]
    return (
        np.ascontiguousarray(fkt.real.astype(np.float32)),
        np.ascontiguousarray(fkt.imag.astype(np.float32)),
    )


def _dft_mats():
    j = np.arange(N)
    ang = 2.0 * np.pi * (np.outer(j, j) % N) / N
    cm = np.cos(ang).astype(np.float32)
    sm = np.sin(ang).astype(np.float32)
    return {
        "cmat": cm,
        "smat": sm,
        "snmat": np.ascontiguousarray(-sm),
        "nscmat": np.ascontiguousarray(-sm - cm),
        "cmsmat": np.ascontiguousarray(cm - sm),
    }


def _kernel_spectral(x0, tt, kernels):
    global LAST_EXEC_NS, LAST_TRACE
    from concourse import bass_utils

    x0 = np.ascontiguousarray(np.asarray(x0), dtype=np.float32)
    fktr_all, fkti_all = _host_spectra(kernels)
    mats = _dft_mats()

    if "spec" not in _PROGRAMS:
        _PROGRAMS["spec"] = _build_program_spec()
    nc = _PROGRAMS["spec"]

    in_maps = []
    for c in range(NCORES):
        sl = slice(c * SPC, (c + 1) * SPC)
        ts = tt[sl]
        im = {
            "x": np.ascontiguousarray(x0[sl].reshape(IMGS, N, N)),
            "fkr": np.ascontiguousarray(fktr_all[ts]),
            "fki": np.ascontiguousarray(fkti_all[ts]),
        }
        im.update(mats)
        in_maps.append(im)

    res = bass_utils.run_bass_kernel_spmd(
        nc, in_maps, core_ids=list(range(NCORES)), trace=TRACE
    )
    LAST_EXEC_NS = res.exec_time_ns
    if res.instructions_and_trace is not None:
        LAST_TRACE = res.instructions_and_trace[1]
    out = np.empty((BATCH, CHANNELS, N, N), np.float32)
    for c in range(NCORES):
        out[c * SPC : (c + 1) * SPC] = res.results[c]["out"].reshape(
            SPC, CHANNELS, N, N
        )
    return out


def kernel(x0, t, kernels):
    tt = np.asarray(t).astype(np.int64)
    k_sp = _composed_kernels(kernels)
    fac = _rank1_factors(k_sp)
    if fac is not None:
        return _kernel_separable(x0, tt, fac[0], fac[1], fac[2])
    return _kernel_spectral(x0, tt, kernels)


# revision 19
# speedup vs baseline: 7.8914x; 1.0155x over previous
"""BlurDegradation kernel for 8x TRN2 NeuronCores.

Math: t[b] successive 11x11 depthwise *circular* convolutions compose into a
single circular convolution with kernel k_t = h_1 (*) h_2 (*) ... (*) h_t
(circular 2D convolution of the per-step impulse responses). The host
composes k_t exactly with FFTs.

Fast path (separable): when every composed k_t is numerically rank-1
(k_t = outer(a_t, b_t) -- always true for the constant-Gaussian blur routine,
whose steps are separable), the 2D blur factors into a column circular conv
by a_t followed by a row circular conv by b_t. Each is one dense circulant
matmul, so an image costs just two 512x512x512 matmul chains on the PE array
(268M MACs vs 941M for the spectral pipeline). Both stages are
data-stationary so outputs chain orientation [h,w] -> [w,v] -> [v,x] with no
transposes:
  stage 1: T1[w,v] = sum_h X[h,w] * Gc[h,v],  Gc[h,v] = a[(v-h) mod N]
  stage 2: Z [v,x] = sum_w T1[w,v] * Gr[w,x], Gr[w,x] = b[(x-w) mod N]
Matmuls run in bf16 (1 cycle/row + fast weight load) with fp32 PSUM
accumulation; stage-2 of image i is emitted after stage-1 of image i+1 so the
PSUM->SBUF cast copies never stall the PE.

Fallback path (general kernels): the original spectral pipeline -- per-sample
cumulative half-spectra multiply between dense DFT matmuls.

Sharding: pure data parallel, 8 samples per core, no cross-core comms.
"""

import numpy as np

N = 512
P = 128
T_STEPS = 20
KS = 11
KP = 258            # padded half-spectrum k-dim (even for fp32r)
NCORES = 8
BATCH = 64
CHANNELS = 3
SPC = BATCH // NCORES  # samples per core
IMGS = SPC * CHANNELS  # images per core

USE_F32R = True

_PROGRAMS = {}
TRACE = False
LAST_EXEC_NS = None
LAST_TRACE = None


# --------------------------------------------------------------------------
# Separable fast path
# --------------------------------------------------------------------------

# The composed blur kernel after t steps has support <= +-5t, so the
# circulant G[a, b] = vec[(b-a) mod 512] is band-limited: the [128 x 512]
# tile for contraction block kk has nonzero columns only in the contiguous
# (mod 512) span [128kk - W, 128(kk+1) + W). Samples are globally sorted by
# t so that slot j on every core shares a compiled band halfwidth W_j.
# PSUM's per-element has_written bit makes partial-column accumulation work:
# the group's first matmul (start=True) clears the whole bank's bits, later
# matmuls overwrite where unwritten and accumulate where written.
def _plan_spans(W):
    """Per contraction tile kk: fused contiguous column ranges of its band
    (split only at the mod-512 wrap). Returns [(kk, c0, c1), ...]."""
    plans = []
    for kk in range(4):
        c0 = (P * kk - W) % N
        ln = P + 2 * W
        if ln >= N:
            plans.append((kk, 0, N))
        elif c0 + ln <= N:
            plans.append((kk, c0, c0 + ln))
        else:
            plans.append((kk, c0, N))
            plans.append((kk, 0, c0 + ln - N))
    return plans


def _plan_spans_1s(W):
    """One-sided band (filter pre-rolled to support [0, 2W], input pre-rolled
    by -W): tile kk covers cols [128kk, 128(kk+1)+2W) mod 512. Returns
    [(kk, c0, c1, q0), ...] with q0 the packed-G column offset."""
    plans = []
    for kk in range(4):
        base = P * kk
        ln = min(P + 2 * W, N)
        if base + ln <= N:
            plans.append((kk, base, base + ln, 0))
        else:
            plans.append((kk, base, N, 0))
            plans.append((kk, 0, base + ln - N, N - base))
    return plans


def _build_program_sep_shared(Wj):
    """Shared-G (symmetric factors) program: per-slot band-packed circulant
    G tile M[p, q] = a'[(q - p) mod 512] (identical for all 4 contraction
    tiles), kept resident in SBUF for all 8 slots."""
    import concourse.mybir as mybir
    import concourse.tile as tile
    from concourse import bacc

    f32 = mybir.dt.float32
    bf16 = mybir.dt.bfloat16

    nc = bacc.Bacc(
        "TRN2", target_bir_lowering=False, debug=False, num_devices=NCORES
    )
    x_d = nc.dram_tensor("x", [IMGS, N, N], bf16, kind="ExternalInput").ap()
    cws = [min(P + 2 * W, N) for W in Wj]
    g_d = [
        nc.dram_tensor(f"g{j}", [P, cws[j]], bf16, kind="ExternalInput").ap()
        for j in range(SPC)
    ]
    out_d = nc.dram_tensor("out", [IMGS, N, N], bf16, kind="ExternalOutput").ap()

    with tile.TileContext(nc) as tc:
        with (
            tc.tile_pool(name="gp", bufs=1) as gp,
            tc.tile_pool(name="xsp", bufs=6) as xsp,
            tc.tile_pool(name="t1p", bufs=3) as t1p,
            tc.tile_pool(name="outp", bufs=4) as outp,
            tc.tile_pool(name="psum", bufs=8, space="PSUM") as psum,
        ):
            # all per-slot G bands stay resident (~4KB/partition total)
            gt = []
            for j in range(SPC):
                g = gp.tile([P, cws[j]], bf16, name=f"g{j}_s")
                nc.sync.dma_start(g[:], g_d[j])
                gt.append(g)

            def banded_mms(pa, data, msl, g, plans):
                last = len(plans) - 1
                for i, (kk, c0, c1, q0) in enumerate(plans):
                    nc.tensor.matmul(
                        pa[:, c0:c1], data[:, kk, msl], g[:, q0 : q0 + c1 - c0],
                        start=(i == 0), stop=(i == last),
                    )

            def emit_st2(T1, g, img, plans):
                # stage 2 (row conv): Z[v,x] = sum_w T1[w,v] G[w,x]
                outs = outp.tile([P, 4, N], bf16, tag="outs")
                for m in range(4):
                    msl = slice(m * P, (m + 1) * P)
                    pb = psum.tile([P, N], f32, tag="ps", name="pb")
                    banded_mms(pb, T1, msl, g, plans)
                    if m % 2 == 0:
                        nc.vector.tensor_copy(out=outs[:, m, :], in_=pb[:])
                    else:
                        nc.scalar.copy(out=outs[:, m, :], in_=pb[:])
                nc.sync.dma_start(
                    out_d[img].rearrange("(i p) n -> p i n", p=P), outs[:]
                )

            pending = None  # (T1, g, img, plans) of the previous image

            for s in range(SPC):
                plans = _plan_spans_1s(Wj[s])
                for ch in range(CHANNELS):
                    img = s * CHANNELS + ch
                    xs = xsp.tile([P, 4, N], bf16, tag="xs")
                    nc.gpsimd.dma_start(
                        xs[:], x_d[img].rearrange("(i p) n -> p i n", p=P)
                    )
                    # stage 1 (col conv): T1[w,v] = sum_h X[h,w] G[h,v]
                    T1 = t1p.tile([P, 4, N], bf16, tag="T1")
                    for m in range(4):
                        msl = slice(m * P, (m + 1) * P)
                        pa = psum.tile([P, N], f32, tag="ps", name="pa")
                        banded_mms(pa, xs, msl, gt[s], plans)
                        if m % 2 == 0:
                            nc.vector.tensor_copy(out=T1[:, m, :], in_=pa[:])
                        else:
                            nc.scalar.copy(out=T1[:, m, :], in_=pa[:])
                    if pending is not None:
                        emit_st2(*pending)
                    pending = (T1, gt[s], img, plans)

            if pending is not None:
                emit_st2(*pending)

    nc.compile()
    return nc


def _build_program_sep(Wj, shared):
    import concourse.mybir as mybir
    import concourse.tile as tile
    from concourse import bacc

    f32 = mybir.dt.float32
    bf16 = mybir.dt.bfloat16

    nc = bacc.Bacc(
        "TRN2", target_bir_lowering=False, debug=False, num_devices=NCORES
    )
    x_d = nc.dram_tensor("x", [IMGS, N, N], bf16, kind="ExternalInput").ap()
    gc_d = nc.dram_tensor("gc", [SPC, N, N], bf16, kind="ExternalInput").ap()
    gr_d = None
    if not shared:
        gr_d = nc.dram_tensor("gr", [SPC, N, N], bf16, kind="ExternalInput").ap()
    out_d = nc.dram_tensor("out", [IMGS, N, N], bf16, kind="ExternalOutput").ap()

    with tile.TileContext(nc) as tc:
        with (
            tc.tile_pool(name="gp", bufs=4 if shared else 3) as gp,
            tc.tile_pool(name="xsp", bufs=4) as xsp,
            tc.tile_pool(name="t1p", bufs=3) as t1p,
            tc.tile_pool(name="outp", bufs=3) as outp,
            tc.tile_pool(name="psum", bufs=8, space="PSUM") as psum,
        ):
            def banded_mms(pa, data, msl, g, plans):
                # fused band spans; only the group's first matmul starts the
                # bank (whole-bank has_written clear), the rest accumulate
                # per-element. Consecutive same-kk matmuls share LDWEIGHTS.
                last = len(plans) - 1
                for i, (kk, c0, c1) in enumerate(plans):
                    nc.tensor.matmul(
                        pa[:, c0:c1], data[:, kk, msl], g[:, kk, c0:c1],
                        start=(i == 0), stop=(i == last),
                    )

            def emit_st2(T1, grs, img, plans):
                # stage 2 (row conv): Z[v,x] = sum_w T1[w,v] Gr[w,x]
                outs = outp.tile([P, 4, N], bf16, tag="outs")
                for m in range(4):
                    msl = slice(m * P, (m + 1) * P)
                    pb = psum.tile([P, N], f32, tag="ps", name="pb")
                    banded_mms(pb, T1, msl, grs, plans)
                    if m % 2 == 0:
                        nc.vector.tensor_copy(out=outs[:, m, :], in_=pb[:])
                    else:
                        nc.scalar.copy(out=outs[:, m, :], in_=pb[:])
                nc.sync.dma_start(
                    out_d[img].rearrange("(i p) n -> p i n", p=P), outs[:]
                )

            pending = None  # (T1, grs, img, plans) of the previous image

            for s in range(SPC):
                plans = _plan_spans(Wj[s])
                gcs = gp.tile([P, 4, N], bf16, tag="gcs")
                nc.sync.dma_start(
                    gcs[:], gc_d[s].rearrange("(i p) n -> p i n", p=P)
                )
                if shared:
                    grs = gcs
                else:
                    grs = gp.tile([P, 4, N], bf16, tag="grs")
                    nc.sync.dma_start(
                        grs[:], gr_d[s].rearrange("(i p) n -> p i n", p=P)
                    )
                for ch in range(CHANNELS):
                    img = s * CHANNELS + ch
                    xs = xsp.tile([P, 4, N], bf16, tag="xs")
                    nc.sync.dma_start(
                        xs[:], x_d[img].rearrange("(i p) n -> p i n", p=P)
                    )
                    # stage 1 (col conv): T1[w,v] = sum_h X[h,w] Gc[h,v]
                    T1 = t1p.tile([P, 4, N], bf16, tag="T1")
                    for m in range(4):
                        msl = slice(m * P, (m + 1) * P)
                        pa = psum.tile([P, N], f32, tag="ps", name="pa")
                        banded_mms(pa, xs, msl, gcs, plans)
                        if m % 2 == 0:
                            nc.vector.tensor_copy(out=T1[:, m, :], in_=pa[:])
                        else:
                            nc.scalar.copy(out=T1[:, m, :], in_=pa[:])
                    # fill the copy-latency bubble with the previous image's
                    # stage 2 (independent PE work)
                    if pending is not None:
                        emit_st2(*pending)
                    pending = (T1, grs, img, plans)

            if pending is not None:
                emit_st2(*pending)

    nc.compile()
    return nc


def _composed_kernels(kernels):
    """Exact composed spatial kernels k_t, [T+1, N, N] float64 (k_0 = delta).
    out_t = k_t (*) x as a 2D circular convolution."""
    kernels = np.asarray(kernels, dtype=np.float64)
    h = np.zeros((T_STEPS, N, N), np.float64)
    idx = (KS // 2 - np.arange(KS)) % N
    h[:, idx[:, None], idx[None, :]] = kernels
    s_step = np.fft.fft2(h)
    cum = np.empty((T_STEPS + 1, N, N), np.complex128)
    cum[0] = 1.0
    for i in range(1, T_STEPS + 1):
        cum[i] = cum[i - 1] * s_step[i - 1]
    return np.fft.ifft2(cum).real


def _rank1_factors(k_sp):
    """If every composed kernel is rank-1, return (A, B, shared) with
    k_sp[t] == outer(A[t], B[t]); shared=True when A==B for all t (symmetric
    kernels -- lets the device reuse one circulant for both stages).
    Returns None if any level is not rank-1."""
    A = np.zeros((T_STEPS + 1, N))
    B = np.zeros((T_STEPS + 1, N))
    shared = True
    for tl in range(T_STEPS + 1):
        K = k_sp[tl]
        am = np.abs(K).max()
        if am == 0.0:
            return None
        sym = False
        if np.abs(K - K.T).max() <= 1e-9 * am:
            i0 = int(np.argmax(np.diagonal(K)))
            piv = K[i0, i0]
            if piv > 1e-12 * am:
                a = K[:, i0] / np.sqrt(piv)
                b = a
                sym = True
        if not sym:
            shared = False
            i0, j0 = np.unravel_index(np.abs(K).argmax(), K.shape)
            a = K[:, j0] / K[i0, j0]
            b = K[i0, :]
        if np.abs(K - np.outer(a, b)).max() > 1e-6 * am:
            return None
        A[tl] = a
        B[tl] = b
    return A, B, shared


_SHIFT_IDX = (np.arange(N)[None, :] - np.arange(N)[:, None]) % N


def _circulant(vec):
    # M[h, v] = vec[(v - h) mod N]
    return vec[_SHIFT_IDX]


def _kernel_separable(x0, tt, A, B, shared):
    global LAST_EXEC_NS, LAST_TRACE
    from concourse import bass_utils
    import ml_dtypes

    # Effective band halfwidth per t: the smallest W whose tail mass (both
    # factors) is <= 3e-5 of the filter's L1 norm -- the induced output error
    # is orders of magnitude below the 2e-2 gate, and Gaussian tails shrink
    # W(t=20) from 100 to ~60.
    Weff = {}
    for tv in np.unique(tt):
        tv = int(tv)
        Ws = min(5 * tv, P - 1)
        W = Ws
        for cand in range(Ws + 1):
            tol = 3e-5
            ta = np.abs(A[tv][cand + 1 : N - cand]).sum() if cand < N // 2 else 0.0
            tb = np.abs(B[tv][cand + 1 : N - cand]).sum() if cand < N // 2 else 0.0
            if ta <= tol * np.abs(A[tv]).sum() and tb <= tol * np.abs(B[tv]).sum():
                W = cand
                break
        Weff[tv] = W

    # sort samples by descending effective W; core c slot j <- sorted
    # position 8j + c, so every core's slot j shares the compiled band
    # halfwidth W_j = max Weff. Heavy slots run first (DMA prefetch warms up
    # behind long PE bursts), cheap slots drain fast at the tail.
    wt = np.array([Weff[int(tv)] for tv in tt])
    order = np.argsort(-wt, kind="stable")
    Wj = tuple(int(wt[order[NCORES * j]]) for j in range(SPC))

    key = ("sep", Wj, shared)
    if key not in _PROGRAMS:
        _PROGRAMS[key] = (
            _build_program_sep_shared(Wj) if shared
            else _build_program_sep(Wj, shared)
        )
    nc = _PROGRAMS[key]

    bf = ml_dtypes.bfloat16
    xb = np.asarray(x0, dtype=np.float32).astype(bf)

    in_maps = []
    origs = []
    if shared:
        # band-packed G: M[p, q] = a'[(q - p) mod 512], a' = roll(a, W_slot)
        GP = {}
        for j in range(SPC):
            W = Wj[j]
            cw = min(P + 2 * W, N)
            GP[W] = (np.arange(cw)[None, :] - np.arange(P)[:, None]) % N
        for c in range(NCORES):
            orig = order[np.arange(SPC) * NCORES + c]
            origs.append(orig)
            im = {}
            xcore = np.empty((SPC, CHANNELS, N, N), bf)
            for j in range(SPC):
                W = Wj[j]
                tv = int(tt[orig[j]])
                Ws = Weff[tv]
                a = A[tv].copy()
                if Ws < N // 2:
                    a[Ws + 1 : N - Ws] = 0.0
                ap_ = np.roll(a, W)
                im[f"g{j}"] = ap_[GP[W]].astype(bf)
                xcore[j] = np.roll(xb[orig[j]], (-W, -W), axis=(-2, -1))
            im["x"] = np.ascontiguousarray(xcore.reshape(IMGS, N, N))
            in_maps.append(im)
    else:
        GC = {}
        GR = {}
        for tv in np.unique(tt):
            tv = int(tv)
            W = min(5 * tv, P - 1)
            a = A[tv].copy()
            a[W + 1 : N - W] = 0.0  # drop FFT noise outside the band
            GC[tv] = _circulant(a).astype(bf)
            b = B[tv].copy()
            b[W + 1 : N - W] = 0.0
            GR[tv] = _circulant(b).astype(bf)
        for c in range(NCORES):
            orig = order[np.arange(SPC) * NCORES + c]
            origs.append(orig)
            ts = tt[orig]
            in_maps.append({
                "x": np.ascontiguousarray(xb[orig].reshape(IMGS, N, N)),
                "gc": np.stack([GC[int(tv)] for tv in ts]),
                "gr": np.stack([GR[int(tv)] for tv in ts]),
            })

    res = bass_utils.run_bass_kernel_spmd(
        nc, in_maps, core_ids=list(range(NCORES)), trace=TRACE
    )
    LAST_EXEC_NS = res.exec_time_ns
    if res.instructions_and_trace is not None:
        LAST_TRACE = res.instructions_and_trace[1]
    out = np.empty((BATCH, CHANNELS, N, N), np.float32)
    for c in range(NCORES):
        out[origs[c]] = (
            np.asarray(res.results[c]["out"])
            .astype(np.float32)
            .reshape(SPC, CHANNELS, N, N)
        )
    return out


# --------------------------------------------------------------------------
# Spectral fallback (general, possibly non-separable kernels)
# --------------------------------------------------------------------------

def _build_program_spec():
    import concourse.mybir as mybir
    import concourse.tile as tile
    from concourse import bacc

    f32 = mybir.dt.float32
    f32r = mybir.dt.float32r
    mmdt = f32r if USE_F32R else f32

    nc = bacc.Bacc(
        "TRN2", target_bir_lowering=False, debug=False, num_devices=NCORES
    )
    x_d = nc.dram_tensor("x", [IMGS, N, N], mmdt, kind="ExternalInput").ap()
    fkr_d = nc.dram_tensor("fkr", [SPC, N, KP], f32, kind="ExternalInput").ap()
    fki_d = nc.dram_tensor("fki", [SPC, N, KP], f32, kind="ExternalInput").ap()
    mat_names = ["cmat", "smat", "snmat", "nscmat", "cmsmat"]
    mat_d = {
        nm: nc.dram_tensor(nm, [N, N], mmdt, kind="ExternalInput").ap()
        for nm in mat_names
    }
    out_d = nc.dram_tensor("out", [IMGS, N, N], f32, kind="ExternalOutput").ap()

    with tile.TileContext(nc) as tc:
        with (
            tc.tile_pool(name="mats", bufs=1) as mats,
            tc.tile_pool(name="xsp", bufs=2) as xsp,
            tc.tile_pool(name="outp", bufs=2) as outp,
            tc.tile_pool(name="fkp", bufs=2) as fkp,
            tc.tile_pool(name="apool", bufs=2) as apool,
            tc.tile_pool(name="hpool", bufs=2) as hpool,
            tc.tile_pool(name="zpool", bufs=2) as zpool,
            tc.tile_pool(name="vpool", bufs=2) as vpool,
            tc.tile_pool(name="pw", bufs=3) as pw,
            tc.tile_pool(name="psum", bufs=8, space="PSUM") as psum,
        ):
            # resident DFT matrices, [p, tile, n] layout
            M = {}
            for nm in mat_names:
                mt = mats.tile([P, 4, N], mmdt, name=nm + "_s")
                nc.sync.dma_start(mt[:], mat_d[nm].rearrange("(i p) n -> p i n", p=P))
                M[nm] = mt
            Cs, Ss, Sns = M["cmat"], M["smat"], M["snmat"]
            nSCs, CmSs = M["nscmat"], M["cmsmat"]
            # C/-S rows 130..257, partition-aligned (stage-4 k tail tile)
            Ck2s = mats.tile([P, N], mmdt, name="ck2_s")
            Snk2s = mats.tile([P, N], mmdt, name="snk2_s")
            nc.sync.dma_start(Ck2s[:], mat_d["cmat"][130:258, :])
            nc.sync.dma_start(Snk2s[:], mat_d["snmat"][130:258, :])

            def emit_st4(Vr, Vi, img):
                # ---- Stage 4 (matrix-stationary, natural orientation):
                # out[y,x] = sum_k C[k,y] Vr[k,x] + (-S)[k,y] Vi[k,x]
                # k tiles: 0:128, 128:256, 130:258 (FK pre-halved on the
                # double-counted 130..255 range)
                outs = outp.tile([P, 4, N], f32, tag="outs")
                for ym in range(4):
                    ysl = slice(ym * P, (ym + 1) * P)
                    po = psum.tile([P, N], f32, tag="ps", name="po")
                    nc.tensor.matmul(
                        po[:], Cs[:, 0, ysl], Vr[:, 0, :],
                        start=True, stop=False,
                    )
                    nc.tensor.matmul(
                        po[:], Cs[:, 1, ysl], Vr[:, 1, :],
                        start=False, stop=False,
                    )
                    nc.tensor.matmul(
                        po[:], Ck2s[:, ysl], Vr[:, 2, :],
                        start=False, stop=False,
                    )
                    nc.tensor.matmul(
                        po[:], Sns[:, 0, ysl], Vi[:, 0, :],
                        start=False, stop=False,
                    )
                    nc.tensor.matmul(
                        po[:], Sns[:, 1, ysl], Vi[:, 1, :],
                        start=False, stop=False,
                    )
                    nc.tensor.matmul(
                        po[:], Snk2s[:, ysl], Vi[:, 2, :],
                        start=False, stop=True,
                    )
                    nc.any.tensor_copy(out=outs[:, ym, :], in_=po[:])
                nc.sync.dma_start(
                    out_d[img].rearrange("(i p) n -> p i n", p=P), outs[:]
                )

            pending = None  # (Vr, Vi, img) of the previous image

            for s in range(SPC):
                # per-sample spectrum, transposed [l, k] layout, k cols 0..257
                fktr = fkp.tile([P, 4, KP], f32, tag="fktr")
                fkti = fkp.tile([P, 4, KP], f32, tag="fkti")
                nc.sync.dma_start(
                    fktr[:], fkr_d[s].rearrange("(i p) n -> p i n", p=P)
                )
                nc.sync.dma_start(
                    fkti[:], fki_d[s].rearrange("(i p) n -> p i n", p=P)
                )

                for ch in range(CHANNELS):
                    img = s * CHANNELS + ch
                    xs = xsp.tile([P, 4, N], mmdt, tag="xs")
                    nc.sync.dma_start(
                        xs[:], x_d[img].rearrange("(i p) n -> p i n", p=P)
                    )

                    # ---- Stage 1 (data-stationary, weight-paired):
                    # A_r = x^T C[:, :258] ; A_i = x^T (-S)[:, :258]
                    Ar = apool.tile([P, 4, KP], mmdt, tag="Ar")
                    Ai = apool.tile([P, 4, KP], mmdt, tag="Ai")
                    Apb = apool.tile([P, 4, KP], mmdt, tag="Apb")
                    for m in range(4):
                        msl = slice(m * P, (m + 1) * P)
                        pa = psum.tile([P, N], f32, tag="ps", name="pa")[:, :KP]
                        pb = psum.tile([P, N], f32, tag="ps", name="pb")[:, :KP]
                        for kk in range(4):
                            nc.tensor.matmul(
                                pa[:], xs[:, kk, msl], Cs[:, kk, 0:KP],
                                start=(kk == 0), stop=(kk == 3),
                            )
                            nc.tensor.matmul(
                                pb[:], xs[:, kk, msl], Sns[:, kk, 0:KP],
                                start=(kk == 0), stop=(kk == 3),
                            )
                        nc.scalar.copy(out=Ar[:, m, :], in_=pa[:])
                        nc.scalar.copy(out=Ai[:, m, :], in_=pb[:])
                        nc.gpsimd.tensor_tensor(
                            Apb[:, m, :], Ar[:, m, :], Ai[:, m, :],
                            mybir.AluOpType.add,
                        )

                    # ---- Stage 2 (matrix-stationary, Gauss):
                    # m1 = C.(Ar+Ai) ; m2 = (-S-C).Ar ; m3 = (C-S).Ai
                    # Htr = m1 - m3 ; Hti = m1 + m2 ; pointwise per l-tile
                    Ztr = zpool.tile([P, 4, KP], mmdt, tag="Ztr")
                    Zti = zpool.tile([P, 4, KP], mmdt, tag="Zti")
                    for lm in range(4):
                        lsl = slice(lm * P, (lm + 1) * P)
                        m1 = psum.tile([P, N], f32, tag="ps", name="m1")[:, :KP]
                        m2 = psum.tile([P, N], f32, tag="ps", name="m2")[:, :KP]
                        m3 = psum.tile([P, N], f32, tag="ps", name="m3")[:, :KP]
                        for kk in range(4):
                            nc.tensor.matmul(
                                m1[:], Cs[:, kk, lsl], Apb[:, kk, :],
                                start=(kk == 0), stop=(kk == 3),
                            )
                        for kk in range(4):
                            nc.tensor.matmul(
                                m2[:], nSCs[:, kk, lsl], Ar[:, kk, :],
                                start=(kk == 0), stop=(kk == 3),
                            )
                        for kk in range(4):
                            nc.tensor.matmul(
                                m3[:], CmSs[:, kk, lsl], Ai[:, kk, :],
                                start=(kk == 0), stop=(kk == 3),
                            )
                        # Htr = m1 - m3 ; Hti = m1 + m2
                        m1s = pw.tile([P, KP], f32, tag="m1s")
                        hrs = pw.tile([P, KP], f32, tag="hrs")
                        his = pw.tile([P, KP], f32, tag="his")
                        nc.scalar.copy(out=m1s[:], in_=m1[:])
                        nc.vector.tensor_sub(out=hrs[:], in0=m1s[:], in1=m3[:])
                        nc.vector.tensor_add(out=his[:], in0=m1s[:], in1=m2[:])
                        # pointwise: Ztr = hr o fr - hi o fi
                        #            Zti = hr o fi + hi o fr
                        fr = fktr[:, lm, :]
                        fi = fkti[:, lm, :]
                        tt = pw.tile([P, KP], f32, tag="tt")
                        tu = pw.tile([P, KP], f32, tag="tu")
                        ztr = Ztr[:, lm, :]
                        zti = Zti[:, lm, :]
                        nc.vector.tensor_mul(out=ztr, in0=hrs[:], in1=fr)
                        nc.gpsimd.tensor_tensor(
                            tt[:], his[:], fi, mybir.AluOpType.mult
                        )
                        nc.vector.tensor_sub(out=ztr, in0=ztr, in1=tt[:])
                        nc.gpsimd.tensor_tensor(
                            tu[:], hrs[:], fi, mybir.AluOpType.mult
                        )
                        nc.vector.tensor_mul(out=zti, in0=his[:], in1=fr)
                        nc.vector.tensor_add(out=zti, in0=zti, in1=tu[:])

                    # fill the pointwise-latency bubble with the previous
                    # image's stage 4 (independent PE work)
                    if pending is not None:
                        emit_st4(*pending)
                        pending = None

                    # ---- Stage 3 (data-stationary, weight-paired direct):
                    # V_r = Ztr^T C + Zti^T (-S) ; V_i = Ztr^T S + Zti^T C
                    # k M-tiles: 0:128, 128:256, 130:258
                    Vr = vpool.tile([P, 3, N], mmdt, tag="Vr")
                    Vi = vpool.tile([P, 3, N], mmdt, tag="Vi")
                    for km in range(3):
                        koff = (0, 128, 130)[km]
                        ksl = slice(koff, koff + P)
                        nvr = psum.tile([P, N], f32, tag="ps", name="nvr")
                        nvi = psum.tile([P, N], f32, tag="ps", name="nvi")
                        for lt in range(4):
                            nc.tensor.matmul(
                                nvr[:], Ztr[:, lt, ksl], Cs[:, lt, :],
                                start=(lt == 0), stop=False,
                            )
                            nc.tensor.matmul(
                                nvi[:], Ztr[:, lt, ksl], Ss[:, lt, :],
                                start=(lt == 0), stop=False,
                            )
                        for lt in range(4):
                            nc.tensor.matmul(
                                nvr[:], Zti[:, lt, ksl], Sns[:, lt, :],
                                start=False, stop=(lt == 3),
                            )
                            nc.tensor.matmul(
                                nvi[:], Zti[:, lt, ksl], Cs[:, lt, :],
                                start=False, stop=(lt == 3),
                            )
                        nc.any.tensor_copy(out=Vr[:, km, :], in_=nvr[:])
                        nc.any.tensor_copy(out=Vi[:, km, :], in_=nvi[:])

                    pending = (Vr, Vi, img)


            if pending is not None:
                emit_st4(*pending)

    nc.compile()
    return nc


def _host_spectra(kernels):
    """Compose step kernels into 21 cumulative half-spectra, transposed to
    [l, k] layout with Hermitian weights, 1/N^2, and the stage-4
    double-count halving folded in. Returns (FKtr, FKti) f32 [21, 512, KP]."""
    kernels = np.asarray(kernels, dtype=np.float64)
    h = np.zeros((T_STEPS, N, N), np.float64)
    idx = (KS // 2 - np.arange(KS)) % N
    h[:, idx[:, None], idx[None, :]] = kernels
    s_step = np.fft.fft2(h)
    cum = np.ones((T_STEPS + 1, N, N), np.complex128)
    for i in range(1, T_STEPS + 1):
        cum[i] = cum[i - 1] * s_step[i - 1]
    w = np.zeros(KP)
    w[: N // 2 + 1] = 2.0
    w[0] = w[N // 2] = 1.0
    fkt = (cum[:, :KP, :] * w[None, :, None] / float(N * N)).transpose(0, 2, 1)
    half = np.ones(KP)
    half[130:256] = 0.5  # k rows 130..255 appear in both stage-4 k-tiles
    fkt = fkt * half[None, None, :]
    return (
        np.ascontiguousarray(fkt.real.astype(np.float32)),
        np.ascontiguousarray(fkt.imag.astype(np.float32)),
    )


def _dft_mats():
    j = np.arange(N)
    ang = 2.0 * np.pi * (np.outer(j, j) % N) / N
    cm = np.cos(ang).astype(np.float32)
    sm = np.sin(ang).astype(np.float32)
    return {
        "cmat": cm,
        "smat": sm,
        "snmat": np.ascontiguousarray(-sm),
        "nscmat": np.ascontiguousarray(-sm - cm),
        "cmsmat": np.ascontiguousarray(cm - sm),
    }


def _kernel_spectral(x0, tt, kernels):
    global LAST_EXEC_NS, LAST_TRACE
    from concourse import bass_utils

    x0 = np.ascontiguousarray(np.asarray(x0), dtype=np.float32)
    fktr_all, fkti_all = _host_spectra(kernels)
    mats = _dft_mats()

    if "spec" not in _PROGRAMS:
        _PROGRAMS["spec"] = _build_program_spec()
    nc = _PROGRAMS["spec"]

    in_maps = []
    for c in range(NCORES):
        sl = slice(c * SPC, (c + 1) * SPC)
        ts = tt[sl]
        im = {
            "x": np.ascontiguousarray(x0[sl].reshape(IMGS, N, N)),
            "fkr": np.ascontiguousarray(fktr_all[ts]),
            "fki": np.ascontiguousarray(fkti_all[ts]),
        }
        im.update(mats)
        in_maps.append(im)

    res = bass_utils.run_bass_kernel_spmd(
        nc, in_maps, core_ids=list(range(NCORES)), trace=TRACE
    )
    LAST_EXEC_NS = res.exec_time_ns
    if res.instructions_and_trace is not None:
        LAST_TRACE = res.instructions_and_trace[1]
    out = np.empty((BATCH, CHANNELS, N, N), np.float32)
    for c in range(NCORES):
        out[c * SPC : (c + 1) * SPC] = res.results[c]["out"].reshape(
            SPC, CHANNELS, N, N
        )
    return out


def kernel(x0, t, kernels):
    tt = np.asarray(t).astype(np.int64)
    k_sp = _composed_kernels(kernels)
    fac = _rank1_factors(k_sp)
    if fac is not None:
        return _kernel_separable(x0, tt, fac[0], fac[1], fac[2])
    return _kernel_spectral(x0, tt, kernels)


# revision 22
# speedup vs baseline: 7.9050x; 1.0017x over previous
"""BlurDegradation kernel for 8x TRN2 NeuronCores.

Math: t[b] successive 11x11 depthwise *circular* convolutions compose into a
single circular convolution with kernel k_t = h_1 (*) h_2 (*) ... (*) h_t
(circular 2D convolution of the per-step impulse responses). The host
composes k_t exactly with FFTs.

Fast path (separable, shared): when every composed k_t is numerically rank-1
(k_t = outer(a_t, b_t) -- always true for the constant-Gaussian blur routine)
the 2D blur factors into a column circular conv then a row circular conv.
Each is a banded circulant matmul chain on the PE array; both stages are
data-stationary so orientation chains [h,w] -> [w,v] -> [v,x] with no
transposes:
  stage 1: T1[w,v] = sum_h X[h,w] * G[h,v],  G[h,v] = a'[(v-h) mod N]
  stage 2: Z [v,x] = sum_w T1[w,v] * G[w,x]
Key optimizations (723us baseline spectral pipeline -> ~92us):
 - bf16 matmuls/IO, fp32 PSUM accumulation (tolerance is 2e-2).
 - Band-limited matmuls: W_eff(t) = smallest halfwidth holding all but 3e-5
   of the factor's L1 mass (Gaussian tails die fast: W(20)=55 not 100). Only
   the in-band psum column spans are streamed; PSUM's per-element
   has_written bit lets the group's first matmul (start=True) clear the
   bank while later partial-column matmuls overwrite-or-accumulate.
 - Samples globally sorted by W_eff; core c slot j takes sorted sample
   8j + c so all cores share one compiled per-slot band plan.
 - One-sided bands: x is pre-rolled by (-W,-W) on host and the filter by +W,
   so each contraction tile's span starts 128-aligned (fewer wrap splits).
 - The band-packed circulant tile M[p,q] = a'[(q-p) mod 512] is identical
   for all 4 contraction tiles and both stages (symmetric factors), so all
   8 slots' G bands stay resident in SBUF (~4KB/partition, ~0.5MB DMA).
 - Stage-2 of image i is emitted after stage-1 of image i+1 (PSUM copy
   latency hides behind independent PE work); psum<->sbuf copies alternate
   Vector/Scalar; x DMAs issue from the idle GpSimd queue to keep the Sync
   queue free for G/out.

Fallback paths: general rank-1 (non-symmetric) kernels use a two-sided-band
two-matrix variant; non-separable kernels use the original spectral pipeline
(per-sample cumulative half-spectra between dense DFT matmuls).

Sharding: pure data parallel, 8 samples per core, no cross-core comms.
"""

import numpy as np

N = 512
P = 128
T_STEPS = 20
KS = 11
KP = 258            # padded half-spectrum k-dim (even for fp32r)
NCORES = 8
BATCH = 64
CHANNELS = 3
SPC = BATCH // NCORES  # samples per core
IMGS = SPC * CHANNELS  # images per core

USE_F32R = True

_PROGRAMS = {}
TRACE = False
LAST_EXEC_NS = None
LAST_TRACE = None


# --------------------------------------------------------------------------
# Separable fast path
# --------------------------------------------------------------------------

# The composed blur kernel after t steps has support <= +-5t, so the
# circulant G[a, b] = vec[(b-a) mod 512] is band-limited: the [128 x 512]
# tile for contraction block kk has nonzero columns only in the contiguous
# (mod 512) span [128kk - W, 128(kk+1) + W). Samples are globally sorted by
# t so that slot j on every core shares a compiled band halfwidth W_j.
# PSUM's per-element has_written bit makes partial-column accumulation work:
# the group's first matmul (start=True) clears the whole bank's bits, later
# matmuls overwrite where unwritten and accumulate where written.
def _plan_spans(W):
    """Per contraction tile kk: fused contiguous column ranges of its band
    (split only at the mod-512 wrap). Returns [(kk, c0, c1), ...]."""
    plans = []
    for kk in range(4):
        c0 = (P * kk - W) % N
        ln = P + 2 * W
        if ln >= N:
            plans.append((kk, 0, N))
        elif c0 + ln <= N:
            plans.append((kk, c0, c0 + ln))
        else:
            plans.append((kk, c0, N))
            plans.append((kk, 0, c0 + ln - N))
    return plans


def _plan_spans_1s(W):
    """One-sided band (filter pre-rolled to support [0, 2W], input pre-rolled
    by -W): tile kk covers cols [128kk, 128(kk+1)+2W) mod 512. Returns
    [(kk, c0, c1, q0), ...] with q0 the packed-G column offset."""
    plans = []
    for kk in range(4):
        base = P * kk
        ln = min(P + 2 * W, N)
        if base + ln <= N:
            plans.append((kk, base, base + ln, 0))
        else:
            plans.append((kk, base, N, 0))
            plans.append((kk, 0, base + ln - N, N - base))
    return plans


def _build_program_sep_shared(Wj):
    """Shared-G (symmetric factors) program: per-slot band-packed circulant
    G tile M[p, q] = a'[(q - p) mod 512] (identical for all 4 contraction
    tiles), kept resident in SBUF for all 8 slots."""
    import concourse.mybir as mybir
    import concourse.tile as tile
    from concourse import bacc

    f32 = mybir.dt.float32
    bf16 = mybir.dt.bfloat16

    nc = bacc.Bacc(
        "TRN2", target_bir_lowering=False, debug=False, num_devices=NCORES
    )
    x_d = nc.dram_tensor("x", [IMGS, N, N], bf16, kind="ExternalInput").ap()
    cws = [min(P + 2 * W, N) for W in Wj]
    g_d = [
        nc.dram_tensor(f"g{j}", [P, cws[j]], bf16, kind="ExternalInput").ap()
        for j in range(SPC)
    ]
    out_d = nc.dram_tensor("out", [IMGS, N, N], bf16, kind="ExternalOutput").ap()

    with tile.TileContext(nc) as tc:
        with (
            tc.tile_pool(name="gp", bufs=1) as gp,
            tc.tile_pool(name="xsp", bufs=6) as xsp,
            tc.tile_pool(name="t1p", bufs=3) as t1p,
            tc.tile_pool(name="outp", bufs=4) as outp,
            tc.tile_pool(name="psum", bufs=8, space="PSUM") as psum,
        ):
            # all per-slot G bands stay resident (~4KB/partition total);
            # only slot 0's G gates the first matmul -- the rest are DMA'd
            # after the first image's x so they don't delay startup
            gt = [gp.tile([P, cws[j]], bf16, name=f"g{j}_s") for j in range(SPC)]
            nc.sync.dma_start(gt[0][:], g_d[0])

            def banded_mms(pa, data, msl, g, plans):
                last = len(plans) - 1
                for i, (kk, c0, c1, q0) in enumerate(plans):
                    nc.tensor.matmul(
                        pa[:, c0:c1], data[:, kk, msl], g[:, q0 : q0 + c1 - c0],
                        start=(i == 0), stop=(i == last),
                    )

            def emit_st2(T1, g, img, plans):
                # stage 2 (row conv): Z[v,x] = sum_w T1[w,v] G[w,x]
                outs = outp.tile([P, 4, N], bf16, tag="outs")
                for m in range(4):
                    msl = slice(m * P, (m + 1) * P)
                    pb = psum.tile([P, N], f32, tag="ps", name="pb")
                    banded_mms(pb, T1, msl, g, plans)
                    if m % 2 == 0:
                        nc.vector.tensor_copy(out=outs[:, m, :], in_=pb[:])
                    else:
                        nc.scalar.copy(out=outs[:, m, :], in_=pb[:])
                nc.sync.dma_start(
                    out_d[img].rearrange("(i p) n -> p i n", p=P), outs[:]
                )

            pending = None  # (T1, g, img, plans) of the previous image

            for s in range(SPC):
                plans = _plan_spans_1s(Wj[s])
                for ch in range(CHANNELS):
                    img = s * CHANNELS + ch
                    xs = xsp.tile([P, 4, N], bf16, tag="xs")
                    nc.gpsimd.dma_start(
                        xs[:], x_d[img].rearrange("(i p) n -> p i n", p=P)
                    )
                    if img == 0:
                        for j in range(1, SPC):
                            nc.sync.dma_start(gt[j][:], g_d[j])
                    # stage 1 (col conv): T1[w,v] = sum_h X[h,w] G[h,v]
                    T1 = t1p.tile([P, 4, N], bf16, tag="T1")
                    for m in range(4):
                        msl = slice(m * P, (m + 1) * P)
                        pa = psum.tile([P, N], f32, tag="ps", name="pa")
                        banded_mms(pa, xs, msl, gt[s], plans)
                        if m % 2 == 0:
                            nc.vector.tensor_copy(out=T1[:, m, :], in_=pa[:])
                        else:
                            nc.scalar.copy(out=T1[:, m, :], in_=pa[:])
                    if pending is not None:
                        emit_st2(*pending)
                    pending = (T1, gt[s], img, plans)

            if pending is not None:
                emit_st2(*pending)

    nc.compile()
    return nc


def _build_program_sep(Wj, shared):
    import concourse.mybir as mybir
    import concourse.tile as tile
    from concourse import bacc

    f32 = mybir.dt.float32
    bf16 = mybir.dt.bfloat16

    nc = bacc.Bacc(
        "TRN2", target_bir_lowering=False, debug=False, num_devices=NCORES
    )
    x_d = nc.dram_tensor("x", [IMGS, N, N], bf16, kind="ExternalInput").ap()
    gc_d = nc.dram_tensor("gc", [SPC, N, N], bf16, kind="ExternalInput").ap()
    gr_d = None
    if not shared:
        gr_d = nc.dram_tensor("gr", [SPC, N, N], bf16, kind="ExternalInput").ap()
    out_d = nc.dram_tensor("out", [IMGS, N, N], bf16, kind="ExternalOutput").ap()

    with tile.TileContext(nc) as tc:
        with (
            tc.tile_pool(name="gp", bufs=4 if shared else 3) as gp,
            tc.tile_pool(name="xsp", bufs=4) as xsp,
            tc.tile_pool(name="t1p", bufs=3) as t1p,
            tc.tile_pool(name="outp", bufs=3) as outp,
            tc.tile_pool(name="psum", bufs=8, space="PSUM") as psum,
        ):
            def banded_mms(pa, data, msl, g, plans):
                # fused band spans; only the group's first matmul starts the
                # bank (whole-bank has_written clear), the rest accumulate
                # per-element. Consecutive same-kk matmuls share LDWEIGHTS.
                last = len(plans) - 1
                for i, (kk, c0, c1) in enumerate(plans):
                    nc.tensor.matmul(
                        pa[:, c0:c1], data[:, kk, msl], g[:, kk, c0:c1],
                        start=(i == 0), stop=(i == last),
                    )

            def emit_st2(T1, grs, img, plans):
                # stage 2 (row conv): Z[v,x] = sum_w T1[w,v] Gr[w,x]
                outs = outp.tile([P, 4, N], bf16, tag="outs")
                for m in range(4):
                    msl = slice(m * P, (m + 1) * P)
                    pb = psum.tile([P, N], f32, tag="ps", name="pb")
                    banded_mms(pb, T1, msl, grs, plans)
                    if m % 2 == 0:
                        nc.vector.tensor_copy(out=outs[:, m, :], in_=pb[:])
                    else:
                        nc.scalar.copy(out=outs[:, m, :], in_=pb[:])
                nc.sync.dma_start(
                    out_d[img].rearrange("(i p) n -> p i n", p=P), outs[:]
                )

            pending = None  # (T1, grs, img, plans) of the previous image

            for s in range(SPC):
                plans = _plan_spans(Wj[s])
                gcs = gp.tile([P, 4, N], bf16, tag="gcs")
                nc.sync.dma_start(
                    gcs[:], gc_d[s].rearrange("(i p) n -> p i n", p=P)
                )
                if shared:
                    grs = gcs
                else:
                    grs = gp.tile([P, 4, N], bf16, tag="grs")
                    nc.sync.dma_start(
                        grs[:], gr_d[s].rearrange("(i p) n -> p i n", p=P)
                    )
                for ch in range(CHANNELS):
                    img = s * CHANNELS + ch
                    xs = xsp.tile([P, 4, N], bf16, tag="xs")
                    nc.sync.dma_start(
                        xs[:], x_d[img].rearrange("(i p) n -> p i n", p=P)
                    )
                    # stage 1 (col conv): T1[w,v] = sum_h X[h,w] Gc[h,v]
                    T1 = t1p.tile([P, 4, N], bf16, tag="T1")
                    for m in range(4):
                        msl = slice(m * P, (m + 1) * P)
                        pa = psum.tile([P, N], f32, tag="ps", name="pa")
                        banded_mms(pa, xs, msl, gcs, plans)
                        if m % 2 == 0:
                            nc.vector.tensor_copy(out=T1[:, m, :], in_=pa[:])
                        else:
                            nc.scalar.copy(out=T1[:, m, :], in_=pa[:])
                    # fill the copy-latency bubble with the previous image's
                    # stage 2 (independent PE work)
                    if pending is not None:
                        emit_st2(*pending)
                    pending = (T1, grs, img, plans)

            if pending is not None:
                emit_st2(*pending)

    nc.compile()
    return nc


def _composed_kernels(kernels):
    """Exact composed spatial kernels k_t, [T+1, N, N] float64 (k_0 = delta).
    out_t = k_t (*) x as a 2D circular convolution."""
    kernels = np.asarray(kernels, dtype=np.float64)
    h = np.zeros((T_STEPS, N, N), np.float64)
    idx = (KS // 2 - np.arange(KS)) % N
    h[:, idx[:, None], idx[None, :]] = kernels
    s_step = np.fft.fft2(h)
    cum = np.empty((T_STEPS + 1, N, N), np.complex128)
    cum[0] = 1.0
    for i in range(1, T_STEPS + 1):
        cum[i] = cum[i - 1] * s_step[i - 1]
    return np.fft.ifft2(cum).real


def _rank1_factors(k_sp):
    """If every composed kernel is rank-1, return (A, B, shared) with
    k_sp[t] == outer(A[t], B[t]); shared=True when A==B for all t (symmetric
    kernels -- lets the device reuse one circulant for both stages).
    Returns None if any level is not rank-1."""
    A = np.zeros((T_STEPS + 1, N))
    B = np.zeros((T_STEPS + 1, N))
    shared = True
    for tl in range(T_STEPS + 1):
        K = k_sp[tl]
        am = np.abs(K).max()
        if am == 0.0:
            return None
        sym = False
        if np.abs(K - K.T).max() <= 1e-9 * am:
            i0 = int(np.argmax(np.diagonal(K)))
            piv = K[i0, i0]
            if piv > 1e-12 * am:
                a = K[:, i0] / np.sqrt(piv)
                b = a
                sym = True
        if not sym:
            shared = False
            i0, j0 = np.unravel_index(np.abs(K).argmax(), K.shape)
            a = K[:, j0] / K[i0, j0]
            b = K[i0, :]
        if np.abs(K - np.outer(a, b)).max() > 1e-6 * am:
            return None
        A[tl] = a
        B[tl] = b
    return A, B, shared


_SHIFT_IDX = (np.arange(N)[None, :] - np.arange(N)[:, None]) % N


def _circulant(vec):
    # M[h, v] = vec[(v - h) mod N]
    return vec[_SHIFT_IDX]


def _kernel_separable(x0, tt, A, B, shared):
    global LAST_EXEC_NS, LAST_TRACE
    from concourse import bass_utils
    import ml_dtypes

    # Effective band halfwidth per t: the smallest W whose tail mass (both
    # factors) is <= 3e-5 of the filter's L1 norm -- the induced output error
    # is orders of magnitude below the 2e-2 gate, and Gaussian tails shrink
    # W(t=20) from 100 to ~60.
    Weff = {}
    for tv in np.unique(tt):
        tv = int(tv)
        Ws = min(5 * tv, P - 1)
        W = Ws
        for cand in range(Ws + 1):
            tol = 3e-5
            ta = np.abs(A[tv][cand + 1 : N - cand]).sum() if cand < N // 2 else 0.0
            tb = np.abs(B[tv][cand + 1 : N - cand]).sum() if cand < N // 2 else 0.0
            if ta <= tol * np.abs(A[tv]).sum() and tb <= tol * np.abs(B[tv]).sum():
                W = cand
                break
        Weff[tv] = W

    # sort samples by descending effective W; core c slot j <- sorted
    # position 8j + c, so every core's slot j shares the compiled band
    # halfwidth W_j = max Weff. Heavy slots run first (DMA prefetch warms up
    # behind long PE bursts), cheap slots drain fast at the tail.
    wt = np.array([Weff[int(tv)] for tv in tt])
    order = np.argsort(-wt, kind="stable")
    Wj = tuple(int(wt[order[NCORES * j]]) for j in range(SPC))

    key = ("sep", Wj, shared)
    if key not in _PROGRAMS:
        _PROGRAMS[key] = (
            _build_program_sep_shared(Wj) if shared
            else _build_program_sep(Wj, shared)
        )
    nc = _PROGRAMS[key]

    bf = ml_dtypes.bfloat16
    xb = np.asarray(x0, dtype=np.float32).astype(bf)

    in_maps = []
    origs = []
    if shared:
        # band-packed G: M[p, q] = a'[(q - p) mod 512], a' = roll(a, W_slot)
        GP = {}
        for j in range(SPC):
            W = Wj[j]
            cw = min(P + 2 * W, N)
            GP[W] = (np.arange(cw)[None, :] - np.arange(P)[:, None]) % N
        for c in range(NCORES):
            orig = order[np.arange(SPC) * NCORES + c]
            origs.append(orig)
            im = {}
            xcore = np.empty((SPC, CHANNELS, N, N), bf)
            for j in range(SPC):
                W = Wj[j]
                tv = int(tt[orig[j]])
                Ws = Weff[tv]
                a = A[tv].copy()
                if Ws < N // 2:
                    a[Ws + 1 : N - Ws] = 0.0
                ap_ = np.roll(a, W)
                im[f"g{j}"] = ap_[GP[W]].astype(bf)
                xcore[j] = np.roll(xb[orig[j]], (-W, -W), axis=(-2, -1))
            im["x"] = np.ascontiguousarray(xcore.reshape(IMGS, N, N))
            in_maps.append(im)
    else:
        GC = {}
        GR = {}
        for tv in np.unique(tt):
            tv = int(tv)
            W = min(5 * tv, P - 1)
            a = A[tv].copy()
            a[W + 1 : N - W] = 0.0  # drop FFT noise outside the band
            GC[tv] = _circulant(a).astype(bf)
            b = B[tv].copy()
            b[W + 1 : N - W] = 0.0
            GR[tv] = _circulant(b).astype(bf)
        for c in range(NCORES):
            orig = order[np.arange(SPC) * NCORES + c]
            origs.append(orig)
            ts = tt[orig]
            in_maps.append({
                "x": np.ascontiguousarray(xb[orig].reshape(IMGS, N, N)),
                "gc": np.stack([GC[int(tv)] for tv in ts]),
                "gr": np.stack([GR[int(tv)] for tv in ts]),
            })

    res = bass_utils.run_bass_kernel_spmd(
        nc, in_maps, core_ids=list(range(NCORES)), trace=TRACE
    )
    LAST_EXEC_NS = res.exec_time_ns
    if res.instructions_and_trace is not None:
        LAST_TRACE = res.instructions_and_trace[1]
    out = np.empty((BATCH, CHANNELS, N, N), np.float32)
    for c in range(NCORES):
        out[origs[c]] = (
            np.asarray(res.results[c]["out"])
            .astype(np.float32)
            .reshape(SPC, CHANNELS, N, N)
        )
    return out


# --------------------------------------------------------------------------
# Spectral fallback (general, possibly non-separable kernels)
# --------------------------------------------------------------------------

def _build_program_spec():
    import concourse.mybir as mybir
    import concourse.tile as tile
    from concourse import bacc

    f32 = mybir.dt.float32
    f32r = mybir.dt.float32r
    mmdt = f32r if USE_F32R else f32

    nc = bacc.Bacc(
        "TRN2", target_bir_lowering=False, debug=False, num_devices=NCORES
    )
    x_d = nc.dram_tensor("x", [IMGS, N, N], mmdt, kind="ExternalInput").ap()
    fkr_d = nc.dram_tensor("fkr", [SPC, N, KP], f32, kind="ExternalInput").ap()
    fki_d = nc.dram_tensor("fki", [SPC, N, KP], f32, kind="ExternalInput").ap()
    mat_names = ["cmat", "smat", "snmat", "nscmat", "cmsmat"]
    mat_d = {
        nm: nc.dram_tensor(nm, [N, N], mmdt, kind="ExternalInput").ap()
        for nm in mat_names
    }
    out_d = nc.dram_tensor("out", [IMGS, N, N], f32, kind="ExternalOutput").ap()

    with tile.TileContext(nc) as tc:
        with (
            tc.tile_pool(name="mats", bufs=1) as mats,
            tc.tile_pool(name="xsp", bufs=2) as xsp,
            tc.tile_pool(name="outp", bufs=2) as outp,
            tc.tile_pool(name="fkp", bufs=2) as fkp,
            tc.tile_pool(name="apool", bufs=2) as apool,
            tc.tile_pool(name="hpool", bufs=2) as hpool,
            tc.tile_pool(name="zpool", bufs=2) as zpool,
            tc.tile_pool(name="vpool", bufs=2) as vpool,
            tc.tile_pool(name="pw", bufs=3) as pw,
            tc.tile_pool(name="psum", bufs=8, space="PSUM") as psum,
        ):
            # resident DFT matrices, [p, tile, n] layout
            M = {}
            for nm in mat_names:
                mt = mats.tile([P, 4, N], mmdt, name=nm + "_s")
                nc.sync.dma_start(mt[:], mat_d[nm].rearrange("(i p) n -> p i n", p=P))
                M[nm] = mt
            Cs, Ss, Sns = M["cmat"], M["smat"], M["snmat"]
            nSCs, CmSs = M["nscmat"], M["cmsmat"]
            # C/-S rows 130..257, partition-aligned (stage-4 k tail tile)
            Ck2s = mats.tile([P, N], mmdt, name="ck2_s")
            Snk2s = mats.tile([P, N], mmdt, name="snk2_s")
            nc.sync.dma_start(Ck2s[:], mat_d["cmat"][130:258, :])
            nc.sync.dma_start(Snk2s[:], mat_d["snmat"][130:258, :])

            def emit_st4(Vr, Vi, img):
                # ---- Stage 4 (matrix-stationary, natural orientation):
                # out[y,x] = sum_k C[k,y] Vr[k,x] + (-S)[k,y] Vi[k,x]
                # k tiles: 0:128, 128:256, 130:258 (FK pre-halved on the
                # double-counted 130..255 range)
                outs = outp.tile([P, 4, N], f32, tag="outs")
                for ym in range(4):
                    ysl = slice(ym * P, (ym + 1) * P)
                    po = psum.tile([P, N], f32, tag="ps", name="po")
                    nc.tensor.matmul(
                        po[:], Cs[:, 0, ysl], Vr[:, 0, :],
                        start=True, stop=False,
                    )
                    nc.tensor.matmul(
                        po[:], Cs[:, 1, ysl], Vr[:, 1, :],
                        start=False, stop=False,
                    )
                    nc.tensor.matmul(
                        po[:], Ck2s[:, ysl], Vr[:, 2, :],
                        start=False, stop=False,
                    )
                    nc.tensor.matmul(
                        po[:], Sns[:, 0, ysl], Vi[:, 0, :],
                        start=False, stop=False,
                    )
                    nc.tensor.matmul(
                        po[:], Sns[:, 1, ysl], Vi[:, 1, :],
                        start=False, stop=False,
                    )
                    nc.tensor.matmul(
                        po[:], Snk2s[:, ysl], Vi[:, 2, :],
                        start=False, stop=True,
                    )
                    nc.any.tensor_copy(out=outs[:, ym, :], in_=po[:])
                nc.sync.dma_start(
                    out_d[img].rearrange("(i p) n -> p i n", p=P), outs[:]
                )

            pending = None  # (Vr, Vi, img) of the previous image

            for s in range(SPC):
                # per-sample spectrum, transposed [l, k] layout, k cols 0..257
                fktr = fkp.tile([P, 4, KP], f32, tag="fktr")
                fkti = fkp.tile([P, 4, KP], f32, tag="fkti")
                nc.sync.dma_start(
                    fktr[:], fkr_d[s].rearrange("(i p) n -> p i n", p=P)
                )
                nc.sync.dma_start(
                    fkti[:], fki_d[s].rearrange("(i p) n -> p i n", p=P)
                )

                for ch in range(CHANNELS):
                    img = s * CHANNELS + ch
                    xs = xsp.tile([P, 4, N], mmdt, tag="xs")
                    nc.sync.dma_start(
                        xs[:], x_d[img].rearrange("(i p) n -> p i n", p=P)
                    )

                    # ---- Stage 1 (data-stationary, weight-paired):
                    # A_r = x^T C[:, :258] ; A_i = x^T (-S)[:, :258]
                    Ar = apool.tile([P, 4, KP], mmdt, tag="Ar")
                    Ai = apool.tile([P, 4, KP], mmdt, tag="Ai")
                    Apb = apool.tile([P, 4, KP], mmdt, tag="Apb")
                    for m in range(4):
                        msl = slice(m * P, (m + 1) * P)
                        pa = psum.tile([P, N], f32, tag="ps", name="pa")[:, :KP]
                        pb = psum.tile([P, N], f32, tag="ps", name="pb")[:, :KP]
                        for kk in range(4):
                            nc.tensor.matmul(
                                pa[:], xs[:, kk, msl], Cs[:, kk, 0:KP],
                                start=(kk == 0), stop=(kk == 3),
                            )
                            nc.tensor.matmul(
                                pb[:], xs[:, kk, msl], Sns[:, kk, 0:KP],
                                start=(kk == 0), stop=(kk == 3),
                            )
                        nc.scalar.copy(out=Ar[:, m, :], in_=pa[:])
                        nc.scalar.copy(out=Ai[:, m, :], in_=pb[:])
                        nc.gpsimd.tensor_tensor(
                            Apb[:, m, :], Ar[:, m, :], Ai[:, m, :],
                            mybir.AluOpType.add,
                        )

                    # ---- Stage 2 (matrix-stationary, Gauss):
                    # m1 = C.(Ar+Ai) ; m2 = (-S-C).Ar ; m3 = (C-S).Ai
                    # Htr = m1 - m3 ; Hti = m1 + m2 ; pointwise per l-tile
                    Ztr = zpool.tile([P, 4, KP], mmdt, tag="Ztr")
                    Zti = zpool.tile([P, 4, KP], mmdt, tag="Zti")
                    for lm in range(4):
                        lsl = slice(lm * P, (lm + 1) * P)
                        m1 = psum.tile([P, N], f32, tag="ps", name="m1")[:, :KP]
                        m2 = psum.tile([P, N], f32, tag="ps", name="m2")[:, :KP]
                        m3 = psum.tile([P, N], f32, tag="ps", name="m3")[:, :KP]
                        for kk in range(4):
                            nc.tensor.matmul(
                                m1[:], Cs[:, kk, lsl], Apb[:, kk, :],
                                start=(kk == 0), stop=(kk == 3),
                            )
                        for kk in range(4):
                            nc.tensor.matmul(
                                m2[:], nSCs[:, kk, lsl], Ar[:, kk, :],
                                start=(kk == 0), stop=(kk == 3),
                            )
                        for kk in range(4):
                            nc.tensor.matmul(
                                m3[:], CmSs[:, kk, lsl], Ai[:, kk, :],
                                start=(kk == 0), stop=(kk == 3),
                            )
                        # Htr = m1 - m3 ; Hti = m1 + m2
                        m1s = pw.tile([P, KP], f32, tag="m1s")
                        hrs = pw.tile([P, KP], f32, tag="hrs")
                        his = pw.tile([P, KP], f32, tag="his")
                        nc.scalar.copy(out=m1s[:], in_=m1[:])
                        nc.vector.tensor_sub(out=hrs[:], in0=m1s[:], in1=m3[:])
                        nc.vector.tensor_add(out=his[:], in0=m1s[:], in1=m2[:])
                        # pointwise: Ztr = hr o fr - hi o fi
                        #            Zti = hr o fi + hi o fr
                        fr = fktr[:, lm, :]
                        fi = fkti[:, lm, :]
                        tt = pw.tile([P, KP], f32, tag="tt")
                        tu = pw.tile([P, KP], f32, tag="tu")
                        ztr = Ztr[:, lm, :]
                        zti = Zti[:, lm, :]
                        nc.vector.tensor_mul(out=ztr, in0=hrs[:], in1=fr)
                        nc.gpsimd.tensor_tensor(
                            tt[:], his[:], fi, mybir.AluOpType.mult
                        )
                        nc.vector.tensor_sub(out=ztr, in0=ztr, in1=tt[:])
                        nc.gpsimd.tensor_tensor(
                            tu[:], hrs[:], fi, mybir.AluOpType.mult
                        )
                        nc.vector.tensor_mul(out=zti, in0=his[:], in1=fr)
                        nc.vector.tensor_add(out=zti, in0=zti, in1=tu[:])

                    # fill the pointwise-latency bubble with the previous
                    # image's stage 4 (independent PE work)
                    if pending is not None:
                        emit_st4(*pending)
                        pending = None

                    # ---- Stage 3 (data-stationary, weight-paired direct):
                    # V_r = Ztr^T C + Zti^T (-S) ; V_i = Ztr^T S + Zti^T C
                    # k M-tiles: 0:128, 128:256, 130:258
                    Vr = vpool.tile([P, 3, N], mmdt, tag="Vr")
                    Vi = vpool.tile([P, 3, N], mmdt, tag="Vi")
                    for km in range(3):
                        koff = (0, 128, 130)[km]
                        ksl = slice(koff, koff + P)
                        nvr = psum.tile([P, N], f32, tag="ps", name="nvr")
                        nvi = psum.tile([P, N], f32, tag="ps", name="nvi")
                        for lt in range(4):
                            nc.tensor.matmul(
                                nvr[:], Ztr[:, lt, ksl], Cs[:, lt, :],
                                start=(lt == 0), stop=False,
                            )
                            nc.tensor.matmul(
                                nvi[:], Ztr[:, lt, ksl], Ss[:, lt, :],
                                start=(lt == 0), stop=False,
                            )
                        for lt in range(4):
                            nc.tensor.matmul(
                                nvr[:], Zti[:, lt, ksl], Sns[:, lt, :],
                                start=False, stop=(lt == 3),
                            )
                            nc.tensor.matmul(
                                nvi[:], Zti[:, lt, ksl], Cs[:, lt, :],
                                start=False, stop=(lt == 3),
                            )
                        nc.any.tensor_copy(out=Vr[:, km, :], in_=nvr[:])
                        nc.any.tensor_copy(out=Vi[:, km, :], in_=nvi[:])

                    pending = (Vr, Vi, img)


            if pending is not None:
                emit_st4(*pending)

    nc.compile()
    return nc


def _host_spectra(kernels):
    """Compose step kernels into 21 cumulative half-spectra, transposed to
    [l, k] layout with Hermitian weights, 1/N^2, and the stage-4
    double-count halving folded in. Returns (FKtr, FKti) f32 [21, 512, KP]."""
    kernels = np.asarray(kernels, dtype=np.float64)
    h = np.zeros((T_STEPS, N, N), np.float64)
    idx = (KS // 2 - np.arange(KS)) % N
    h[:, idx[:, None], idx[None, :]] = kernels
    s_step = np.fft.fft2(h)
    cum = np.ones((T_STEPS + 1, N, N), np.complex128)
    for i in range(1, T_STEPS + 1):
        cum[i] = cum[i - 1] * s_step[i - 1]
    w = np.zeros(KP)
    w[: N // 2 + 1] = 2.0
    w[0] = w[N // 2] = 1.0
    fkt = (cum[:, :KP, :] * w[None, :, None] / float(N * N)).transpose(0, 2, 1)
    half = np.ones(KP)
    half[130:256] = 0.5  # k rows 130..255 appear in both stage-4 k-tiles
    fkt = fkt * half[None, None, :]
    return (
        np.ascontiguousarray(fkt.real.astype(np.float32)),
        np.ascontiguousarray(fkt.imag.astype(np.float32)),
    )


def _dft_mats():
    j = np.arange(N)
    ang = 2.0 * np.pi * (np.outer(j, j) % N) / N
    cm = np.cos(ang).astype(np.float32)
    sm = np.sin(ang).astype(np.float32)
    return {
        "cmat": cm,
        "smat": sm,
        "snmat": np.ascontiguousarray(-sm),
        "nscmat": np.ascontiguousarray(-sm - cm),
        "cmsmat": np.ascontiguousarray(cm - sm),
    }


def _kernel_spectral(x0, tt, kernels):
    global LAST_EXEC_NS, LAST_TRACE
    from concourse import bass_utils

    x0 = np.ascontiguousarray(np.asarray(x0), dtype=np.float32)
    fktr_all, fkti_all = _host_spectra(kernels)
    mats = _dft_mats()

    if "spec" not in _PROGRAMS:
        _PROGRAMS["spec"] = _build_program_spec()
    nc = _PROGRAMS["spec"]

    in_maps = []
    for c in range(NCORES):
        sl = slice(c * SPC, (c + 1) * SPC)
        ts = tt[sl]
        im = {
            "x": np.ascontiguousarray(x0[sl].reshape(IMGS, N, N)),
            "fkr": np.ascontiguousarray(fktr_all[ts]),
            "fki": np.ascontiguousarray(fkti_all[ts]),
        }
        im.update(mats)
        in_maps.append(im)

    res = bass_utils.run_bass_kernel_spmd(
        nc, in_maps, core_ids=list(range(NCORES)), trace=TRACE
    )
    LAST_EXEC_NS = res.exec_time_ns
    if res.instructions_and_trace is not None:
        LAST_TRACE = res.instructions_and_trace[1]
    out = np.empty((BATCH, CHANNELS, N, N), np.float32)
    for c in range(NCORES):
        out[c * SPC : (c + 1) * SPC] = res.results[c]["out"].reshape(
            SPC, CHANNELS, N, N
        )
    return out


def kernel(x0, t, kernels):
    tt = np.asarray(t).astype(np.int64)
    k_sp = _composed_kernels(kernels)
    fac = _rank1_factors(k_sp)
    if fac is not None:
        return _kernel_separable(x0, tt, fac[0], fac[1], fac[2])
    return _kernel_spectral(x0, tt, kernels)
